# revision 1
# baseline (speedup 1.0000x reference)
"""Trainium2 Bass kernel for a 2-layer GATv2 encoder (nn_GATv2Encoder).

Strategy (8 NeuronCores, SPMD):
- Host sorts edges by dst; dst nodes are partitioned into contiguous 256-node
  blocks, 25 blocks per core -> each core owns a disjoint contiguous node
  range, so segment softmax/scatter are core-local (no cross-core segment
  reductions needed).
- Each block's edges are split into two sections by src < HALF (int16 index
  range for dma_gather), each section padded to a fixed tile capacity.
- Per layer: node-parallel GEMMs produce a combined [fs|fd] table (512B rows);
  fs tables are AllGather'd; fs[src] and fd[dst] are fetched per edge with
  dma_gather (256B elements). Logits l = sum_f attn*leakyrelu(fs+fd) computed
  without segment-max (logits are tiny); softmax normalization is folded into
  a final num/den division. Scatter-add is a one-hot matmul accumulated in
  PSUM per block (bf16 one-hot + bf16 vals, fp32 accumulation).
- BatchNorm stats are computed per-core over the feature-partition layout and
  AllReduce'd; padded rows are corrected with host-precomputed constants.
"""

import os
import numpy as np

# ---------------------------------------------------------------- constants
N_REAL = 50000
E_REAL = 800000
IN_DIM, HID, OUT_DIM = 128, 64, 64
H1, F1 = 8, 8
H2, F2 = 4, 16
SLOPE = 0.2
EPS = 1e-5
DEN_EPS = 1e-30

NCORES = 8
BLK = 256                      # nodes per block (one-hot / psum free width)
NB = 25                        # blocks per core
CORE_NODES = BLK * NB          # 6400
NPAD = NCORES * CORE_NODES     # 51200
HALF = NPAD // 2               # 25600 (int16-safe gather split)
TILE = 128                     # edges per matmul tile
SEC_T = 18                     # tiles per section (A and B)
CAP = SEC_T * TILE             # 2304 edge slots per section
T_BLK = 2 * SEC_T              # 36 tiles per block
SLOTS = 2 * CAP                # 4608 edge slots per block
SEC_COLS = CAP // 16           # idx cols per section (144)
BLK_COLS = SLOTS // 16         # idx cols per block (288)


# ---------------------------------------------------------------- host prep
def _wrap16(idx_list, cols):
    """int16 indices in dma_gather layout: [128, cols]; idx j at
    (partition j%16, col j//16), replicated across the 8 Q7 core groups."""
    flat = np.zeros(16 * cols, np.int16)
    flat[: len(idx_list)] = np.asarray(idx_list, np.int16)
    a = np.ascontiguousarray(flat.reshape(cols, 16).T)  # idx j -> (j%16, j//16)
    return np.tile(a, (8, 1))


def host_prep(src, dst):
    src = np.asarray(src).astype(np.int64)
    dst = np.asarray(dst).astype(np.int64)
    order = np.argsort(dst, kind="stable")
    s_src, s_dst = src[order], dst[order]
    counts = np.bincount(s_dst, minlength=NPAD)
    assert counts[:N_REAL].min() > 0, "zero in-degree node"
    cum = np.concatenate([[0], np.cumsum(counts)])

    cores = []
    for c in range(NCORES):
        base = c * CORE_NODES
        fs_cols, fd_cols = [], []
        dloc = np.full((128, NB * T_BLK), -1.0, np.float32)
        for b in range(NB):
            nb0 = base + b * BLK
            e0, e1 = cum[nb0], cum[min(nb0 + BLK, NPAD)]
            bs, bd = s_src[e0:e1], s_dst[e0:e1]
            selA = bs < HALF
            A_src, A_dst = bs[selA], bd[selA]
            B_src, B_dst = bs[~selA], bd[~selA]
            nA, nB_ = len(A_src), len(B_src)
            assert nA <= CAP and nB_ <= CAP, (c, b, nA, nB_)
            fs_cols.append(_wrap16(A_src, SEC_COLS))
            fs_cols.append(_wrap16(B_src - HALF, SEC_COLS))
            fd_cols.append(_wrap16(A_dst - base, SEC_COLS))
            fd_cols.append(_wrap16(B_dst - base, SEC_COLS))
            dl = np.full(SLOTS, -1.0, np.float32)
            dl[:nA] = A_dst - nb0
            dl[CAP:CAP + nB_] = B_dst - nb0
            dloc[:, b * T_BLK:(b + 1) * T_BLK] = dl.reshape(T_BLK, 128).T
        cores.append(dict(
            fsidx=np.concatenate(fs_cols, 1),   # [128, NB*288] int16
            fdidx=np.concatenate(fd_cols, 1),   # [128, NB*288] int16
            dloc=dloc,                          # [128, NB*36] f32
        ))
    return cores


def _elu_np(x):
    return np.where(x > 0, x, np.exp(np.minimum(x, 0)) - 1).astype(np.float32)


def make_inputs(inputs):
    """Build the 8 per-core input maps for the bass program."""
    f32 = np.float32
    x = np.asarray(inputs["x"], f32)
    cores = host_prep(inputs["src"], inputs["dst"])

    xp = np.zeros((NPAD, IN_DIM), f32)
    xp[:N_REAL] = x
    xT = np.ascontiguousarray(xp.T)                       # [128, NPAD]

    w1 = np.concatenate([np.asarray(inputs["Wsrc1"], f32),
                         np.asarray(inputs["Wdst1"], f32)], 1)   # [128,128]
    w2 = np.concatenate([np.asarray(inputs["Wsrc2"], f32),
                         np.asarray(inputs["Wdst2"], f32)], 1)   # [64,128]
    b1 = np.concatenate([np.asarray(inputs["bsrc1"], f32),
                         np.asarray(inputs["bdst1"], f32)])      # [128]
    b2 = np.concatenate([np.asarray(inputs["bsrc2"], f32),
                         np.asarray(inputs["bdst2"], f32)])
    b1full = np.tile(b1[None, :], (128, 1)).astype(f32)
    b2full = np.tile(b2[None, :], (128, 1)).astype(f32)

    # lrelu(z) = (1+s)/2*z + (1-s)/2*|z|; the (1+s)/2 factor is folded into
    # the attention constants, the |z| path uses ACT Abs with scale (1-s)/(1+s)
    lr_a = (1.0 + SLOPE) / 2.0
    attn1f = np.tile(np.asarray(inputs["attn1"], f32).reshape(1, -1) * lr_a,
                     (128, 1))
    attn2f = np.tile(np.asarray(inputs["attn2"], f32).reshape(1, -1) * lr_a,
                     (128, 1))
    iota = np.tile(np.arange(BLK, dtype=f32)[None, :], (128, 1))

    r1 = np.zeros((H1, HID), f32)
    for h in range(H1):
        r1[h, h * F1:(h + 1) * F1] = 1.0
    r2 = np.zeros((H2, OUT_DIM), f32)
    for h in range(H2):
        r2[h, h * F2:(h + 1) * F2] = 1.0

    npad_rows = NPAD - N_REAL
    bias1 = np.asarray(inputs["bias1"], f32)
    bias2 = np.asarray(inputs["bias2"], f32)
    cpad1 = _elu_np(bias1)
    bn1 = np.zeros((HID, 8), f32)
    bn1[:, 0] = bias1
    bn1[:, 1] = np.asarray(inputs["gamma1"], f32)
    bn1[:, 2] = np.asarray(inputs["beta1"], f32)
    bn1[:, 3] = npad_rows * cpad1
    bn1[:, 4] = npad_rows * cpad1 ** 2
    bn1[:, 5] = EPS
    bn2 = np.zeros((OUT_DIM, 8), f32)
    bn2[:, 0] = bias2
    bn2[:, 1] = np.asarray(inputs["gamma2"], f32)
    bn2[:, 2] = np.asarray(inputs["beta2"], f32)
    bn2[:, 3] = npad_rows * bias2
    bn2[:, 4] = npad_rows * bias2 ** 2
    bn2[:, 5] = EPS

    in_maps = []
    for c in range(NCORES):
        in_maps.append({
            "xT": np.ascontiguousarray(xT[:, c * CORE_NODES:(c + 1) * CORE_NODES]),
            "w1": w1, "w2": w2, "b1full": b1full, "b2full": b2full,
            "attn1f": attn1f.astype(f32), "attn2f": attn2f.astype(f32),
            "iota": iota, "r1": r1, "r2": r2, "bn1": bn1, "bn2": bn2,
            "fsidx": cores[c]["fsidx"], "fdidx": cores[c]["fdidx"],
            "dloc": cores[c]["dloc"],
        })
    return in_maps


# ---------------------------------------------------------------- bass program
def build_program():
    import concourse.bacc as bacc
    import concourse.tile as tile
    from concourse import mybir

    f32 = mybir.dt.float32
    bf16 = mybir.dt.bfloat16
    i16 = mybir.dt.int16
    Alu = mybir.AluOpType
    Act = mybir.ActivationFunctionType

    nc = bacc.Bacc(None, target_bir_lowering=False, num_devices=NCORES)
    RG = [list(range(NCORES))]

    # ---- I/O ----
    xT_d = nc.dram_tensor("xT", [IN_DIM, CORE_NODES], f32, kind="ExternalInput")
    w1_d = nc.dram_tensor("w1", [IN_DIM, 128], f32, kind="ExternalInput")
    w2_d = nc.dram_tensor("w2", [HID, 128], f32, kind="ExternalInput")
    b1f_d = nc.dram_tensor("b1full", [128, 128], f32, kind="ExternalInput")
    b2f_d = nc.dram_tensor("b2full", [128, 128], f32, kind="ExternalInput")
    a1_d = nc.dram_tensor("attn1f", [128, HID], f32, kind="ExternalInput")
    a2_d = nc.dram_tensor("attn2f", [128, OUT_DIM], f32, kind="ExternalInput")
    iota_d = nc.dram_tensor("iota", [128, BLK], f32, kind="ExternalInput")
    r1_d = nc.dram_tensor("r1", [H1, HID], f32, kind="ExternalInput")
    r2_d = nc.dram_tensor("r2", [H2, OUT_DIM], f32, kind="ExternalInput")
    bn1_d = nc.dram_tensor("bn1", [HID, 8], f32, kind="ExternalInput")
    bn2_d = nc.dram_tensor("bn2", [OUT_DIM, 8], f32, kind="ExternalInput")
    fsidx_d = nc.dram_tensor("fsidx", [128, NB * BLK_COLS], i16, kind="ExternalInput")
    fdidx_d = nc.dram_tensor("fdidx", [128, NB * BLK_COLS], i16, kind="ExternalInput")
    dloc_d = nc.dram_tensor("dloc", [128, NB * T_BLK], f32, kind="ExternalInput")
    out_d = nc.dram_tensor("outT", [OUT_DIM, CORE_NODES], f32, kind="ExternalOutput")

    # ---- internal DRAM ----
    fsfd1_loc = nc.dram_tensor("fsfd1_loc", [CORE_NODES, 128], f32)
    fsfd1_full = nc.dram_tensor("fsfd1_full", [NPAD, 128], f32, addr_space="Shared")
    fsfd2_loc = nc.dram_tensor("fsfd2_loc", [CORE_NODES, 128], f32)
    fsfd2_full = nc.dram_tensor("fsfd2_full", [NPAD, 128], f32, addr_space="Shared")
    bnin = [nc.dram_tensor(f"bnin{i}", [64, 2], f32) for i in (1, 2)]
    bnout = [nc.dram_tensor(f"bnout{i}", [64, 2], f32, addr_space="Shared")
             for i in (1, 2)]

    with tile.TileContext(nc) as tc:
        with (
            tc.tile_pool(name="const", bufs=1) as cpool,
            tc.tile_pool(name="gath", bufs=2) as gpool,
            tc.tile_pool(name="work", bufs=2) as wpool,
            tc.tile_pool(name="small", bufs=2) as spool,
            tc.tile_pool(name="node", bufs=1) as npool,
            tc.tile_pool(name="psA", bufs=2, space="PSUM") as psA,
            tc.tile_pool(name="psB", bufs=2, space="PSUM") as psB,
            tc.tile_pool(name="psG", bufs=2, space="PSUM") as psG,
        ):
            # ---- load constants ----
            def load(dram, shape, dtype=f32, pool=cpool):
                t = pool.tile(shape, dtype, tag=f"c_{dram.name}")
                nc.sync.dma_start(out=t[:], in_=dram[:, :])
                return t

            w1_s = load(w1_d, [IN_DIM, 128])
            w2_s = load(w2_d, [HID, 128])
            b1f_s = load(b1f_d, [128, 128])
            b2f_s = load(b2f_d, [128, 128])
            a1_s = load(a1_d, [128, HID])
            a2_s = load(a2_d, [128, OUT_DIM])
            iota_s = load(iota_d, [128, BLK])
            r1_s = load(r1_d, [H1, HID])
            r2_s = load(r2_d, [H2, OUT_DIM])
            bn1_s = load(bn1_d, [HID, 8])
            bn2_s = load(bn2_d, [OUT_DIM, 8])
            fsidx_s = load(fsidx_d, [128, NB * BLK_COLS], i16)
            fdidx_s = load(fdidx_d, [128, NB * BLK_COLS], i16)
            dloc_s = load(dloc_d, [128, NB * T_BLK])

            h1_s = npool.tile([HID, CORE_NODES], f32, tag="h1")
            h_s = npool.tile([HID, CORE_NODES], f32, tag="h")

            NT = CORE_NODES // 128  # node tiles per core for GEMMs

            def gemm_layer(get_lhs, K, w_s, bfull_s, loc_dram, full_dram):
                for t in range(NT):
                    ps = psG.tile([128, 128], f32)
                    nc.tensor.matmul(
                        out=ps[:], lhsT=get_lhs(t),
                        rhs=w_s[:K, :], start=True, stop=True)
                    gs = spool.tile([128, 128], f32, tag="gemm")
                    nc.vector.tensor_tensor(
                        out=gs[:], in0=ps[:], in1=bfull_s[:], op=Alu.add)
                    nc.sync.dma_start(
                        out=loc_dram[t * 128:(t + 1) * 128, :], in_=gs[:])
                nc.gpsimd.collective_compute(
                    "AllGather", Alu.bypass, replica_groups=RG,
                    ins=[loc_dram.ap().opt()], outs=[full_dram.ap().opt()])

            def gemm1_lhs(t):
                xg = spool.tile([IN_DIM, 128], f32, tag="xg")
                nc.sync.dma_start(out=xg[:], in_=xT_d[:, t * 128:(t + 1) * 128])
                return xg[:]

            def edge_layer(Hh, Ff, full_dram, loc_dram, attn_s, rX_s, bn_s,
                           hout_s, do_elu):
                HF = Hh * Ff
                VW = Hh + HF  # vals width per tile
                for b in range(NB):
                    ps_s = psA.tile([VW, BLK], f32, tag="scat")
                    for sec in range(2):
                        cA = b * BLK_COLS + sec * SEC_COLS
                        fs_e = gpool.tile([128, SEC_T * 64], f32, tag="fs")
                        fd_e = gpool.tile([128, SEC_T * 64], f32, tag="fd")
                        tab = full_dram[:, 0:64] if sec == 0 else \
                            full_dram[HALF:, 0:64]
                        nc.gpsimd.dma_gather(
                            out_ap=fs_e[:].rearrange("p (t e) -> p t e", e=64),
                            in_ap=tab,
                            idxs_ap=fsidx_s[:, cA:cA + SEC_COLS],
                            num_idxs=CAP, num_idxs_reg=CAP,
                            elem_size=64, elem_step=128, single_packet=False)
                        nc.gpsimd.dma_gather(
                            out_ap=fd_e[:].rearrange("p (t e) -> p t e", e=64),
                            in_ap=loc_dram[:, 64:128],
                            idxs_ap=fdidx_s[:, cA:cA + SEC_COLS],
                            num_idxs=CAP, num_idxs_reg=CAP,
                            elem_size=64, elem_step=128, single_packet=False)

                        # one-hot (bf16): O[p, t, n] = (dloc[p, t] == n)
                        dcol = b * T_BLK + sec * SEC_T
                        O_t = wpool.tile([128, SEC_T * BLK], bf16, tag="O")
                        nc.vector.tensor_tensor(
                            out=O_t[:].rearrange("p (t n) -> p t n", n=BLK),
                            in0=dloc_s[:, dcol:dcol + SEC_T, None]
                                .to_broadcast([128, SEC_T, BLK]),
                            in1=iota_s[:, None, :]
                                .to_broadcast([128, SEC_T, BLK]),
                            op=Alu.is_equal)

                        # z = fs + fd ; lrelu(z)*attn = (z + c|z|) * attn06
                        z_t = wpool.tile([128, SEC_T * 64], f32, tag="z")
                        wz_t = wpool.tile([128, SEC_T * 64], f32, tag="wz")
                        nc.vector.tensor_tensor(
                            out=z_t[:], in0=fs_e[:], in1=fd_e[:], op=Alu.add)
                        nc.scalar.activation(
                            out=wz_t[:], in_=z_t[:], func=Act.Abs,
                            scale=(1.0 - SLOPE) / (1.0 + SLOPE))
                        nc.vector.tensor_tensor(
                            out=wz_t[:], in0=z_t[:], in1=wz_t[:], op=Alu.add)
                        nc.vector.tensor_tensor(
                            out=z_t[:].rearrange("p (t e) -> p t e", e=64),
                            in0=wz_t[:].rearrange("p (t e) -> p t e", e=64),
                            in1=attn_s[:, None, :]
                                .to_broadcast([128, SEC_T, 64]),
                            op=Alu.mult)
                        # l = sum_f wz ; p = exp(l) into vals
                        l_t = spool.tile([128, SEC_T * Hh], f32, tag="l")
                        nc.vector.tensor_reduce(
                            out=l_t[:],
                            in_=z_t[:].rearrange("p (t h f) -> p t h f",
                                                 h=Hh, f=Ff),
                            axis=mybir.AxisListType.X, op=Alu.add)
                        # vals layout: [p*fs (HF cols) | p (Hh cols)] so that
                        # psum num rows start at partition 0, den at HF (=64)
                        vals = wpool.tile([128, SEC_T * VW], bf16, tag="vals")
                        vals3 = vals[:].rearrange("p (t v) -> p t v", v=VW)
                        nc.scalar.activation(
                            out=vals3[:, :, HF:VW],
                            in_=l_t[:].rearrange("p (t h) -> p t h", h=Hh),
                            func=Act.Exp)
                        nc.vector.tensor_tensor(
                            out=vals3[:, :, 0:HF].rearrange(
                                "p t (h f) -> p t h f", f=Ff),
                            in0=fs_e[:].rearrange("p (t h f) -> p t h f",
                                                  h=Hh, f=Ff),
                            in1=vals3[:, :, HF:VW][:, :, :, None]
                                .to_broadcast([128, SEC_T, Hh, Ff]),
                            op=Alu.mult)

                        # scatter: psum[v, n] += sum_e vals[e, v] * O[e, n]
                        for t in range(SEC_T):
                            nc.tensor.matmul(
                                out=ps_s[:],
                                lhsT=vals[:, t * VW:(t + 1) * VW],
                                rhs=O_t[:, t * BLK:(t + 1) * BLK],
                                start=(sec == 0 and t == 0),
                                stop=(sec == 1 and t == SEC_T - 1))

                    # normalize: out = num * (1/den) + bias
                    den = spool.tile([Hh, BLK], f32, tag="den")
                    nc.vector.tensor_scalar(
                        out=den[:], in0=ps_s[HF:VW, :], scalar1=DEN_EPS,
                        scalar2=None, op0=Alu.add)
                    rcp = spool.tile([Hh, BLK], f32, tag="rcp")
                    nc.vector.reciprocal(out=rcp[:], in_=den[:])
                    ps_r = psB.tile([HF, BLK], f32, tag="rrep")
                    nc.tensor.matmul(out=ps_r[:], lhsT=rX_s[:], rhs=rcp[:],
                                     start=True, stop=True)
                    rr = spool.tile([HF, BLK], f32, tag="rr")
                    nc.vector.tensor_copy(out=rr[:], in_=ps_r[:])
                    o1 = spool.tile([HF, BLK], f32, tag="o1")
                    nc.vector.tensor_tensor(
                        out=o1[:], in0=ps_s[0:HF, :], in1=rr[:], op=Alu.mult)
                    nsl = slice(b * BLK, (b + 1) * BLK)
                    if do_elu:
                        ob = spool.tile([HF, BLK], f32, tag="ob")
                        nc.vector.tensor_scalar(
                            out=ob[:], in0=o1[:], scalar1=bn_s[:, 0:1],
                            scalar2=None, op0=Alu.add)
                        m_t = spool.tile([HF, BLK], f32, tag="elum")
                        nc.vector.tensor_scalar(
                            out=m_t[:], in0=ob[:], scalar1=0.0,
                            scalar2=None, op0=Alu.min)
                        e_t = spool.tile([HF, BLK], f32, tag="elue")
                        nc.scalar.activation(out=e_t[:], in_=m_t[:],
                                             func=Act.Exp)
                        nc.vector.tensor_scalar(
                            out=m_t[:], in0=ob[:], scalar1=0.0,
                            scalar2=None, op0=Alu.max)
                        t_t = spool.tile([HF, BLK], f32, tag="elut")
                        nc.vector.tensor_tensor(
                            out=t_t[:], in0=e_t[:], in1=m_t[:], op=Alu.add)
                        nc.vector.tensor_scalar(
                            out=hout_s[:, nsl], in0=t_t[:], scalar1=-1.0,
                            scalar2=None, op0=Alu.add)
                    else:
                        nc.vector.tensor_scalar(
                            out=hout_s[:, nsl], in0=o1[:], scalar1=bn_s[:, 0:1],
                            scalar2=None, op0=Alu.add)

            def bn_norm(hin_s, bn_s, bnin_d, bnout_d, D):
                """BN stats (blockwise) + AllReduce; returns (scale, shift)."""
                s_cols = spool.tile([D, NB], f32, tag="bnscols")
                q_cols = spool.tile([D, NB], f32, tag="bnqcols")
                for b in range(NB):
                    nsl = slice(b * BLK, (b + 1) * BLK)
                    nc.vector.tensor_reduce(
                        out=s_cols[:, b:b + 1], in_=hin_s[:, nsl],
                        axis=mybir.AxisListType.X, op=Alu.add)
                    scr = spool.tile([D, BLK], f32, tag="bnscr")
                    nc.scalar.activation(
                        out=scr[:], in_=hin_s[:, nsl], func=Act.Square,
                        accum_out=q_cols[:, b:b + 1])
                st = spool.tile([D, 2], f32, tag="bnst")
                nc.vector.tensor_reduce(out=st[:, 0:1], in_=s_cols[:],
                                        axis=mybir.AxisListType.X, op=Alu.add)
                nc.vector.tensor_reduce(out=st[:, 1:2], in_=q_cols[:],
                                        axis=mybir.AxisListType.X, op=Alu.add)
                nc.sync.dma_start(out=bnin_d[:, :], in_=st[:])
                nc.gpsimd.collective_compute(
                    "AllReduce", Alu.add, replica_groups=RG,
                    ins=[bnin_d.ap().opt()], outs=[bnout_d.ap().opt()])
                g = spool.tile([D, 2], f32, tag="bng")
                nc.sync.dma_start(out=g[:], in_=bnout_d[:, :])
                # mu = (S - corr)/N ; var = (SQ - corrsq)/N - mu^2
                t_a = spool.tile([D, 1], f32, tag="bnta")
                nc.vector.tensor_tensor(out=t_a[:], in0=g[:, 0:1],
                                        in1=bn_s[:, 3:4], op=Alu.subtract)
                mu = spool.tile([D, 1], f32, tag="bnmu")
                nc.vector.tensor_scalar(out=mu[:], in0=t_a[:],
                                        scalar1=1.0 / N_REAL, scalar2=None,
                                        op0=Alu.mult)
                t_b = spool.tile([D, 1], f32, tag="bntb")
                nc.vector.tensor_tensor(out=t_b[:], in0=g[:, 1:2],
                                        in1=bn_s[:, 4:5], op=Alu.subtract)
                msq = spool.tile([D, 1], f32, tag="bnmsq")
                nc.vector.tensor_scalar(out=msq[:], in0=t_b[:],
                                        scalar1=1.0 / N_REAL, scalar2=None,
                                        op0=Alu.mult)
                mu2 = spool.tile([D, 1], f32, tag="bnmu2")
                nc.vector.tensor_tensor(out=mu2[:], in0=mu[:], in1=mu[:],
                                        op=Alu.mult)
                var = spool.tile([D, 1], f32, tag="bnvar")
                nc.vector.tensor_tensor(out=var[:], in0=msq[:], in1=mu2[:],
                                        op=Alu.subtract)
                sd = spool.tile([D, 1], f32, tag="bnsd")
                nc.scalar.activation(out=sd[:], in_=var[:], func=Act.Sqrt,
                                     bias=bn_s[:, 5:6])
                rs = spool.tile([D, 1], f32, tag="bnrs")
                nc.vector.reciprocal(out=rs[:], in_=sd[:])
                scl = spool.tile([D, 1], f32, tag="bnscl")
                nc.vector.tensor_tensor(out=scl[:], in0=bn_s[:, 1:2],
                                        in1=rs[:], op=Alu.mult)
                t_c = spool.tile([D, 1], f32, tag="bntc")
                nc.vector.tensor_tensor(out=t_c[:], in0=mu[:], in1=scl[:],
                                        op=Alu.mult)
                shf = spool.tile([D, 1], f32, tag="bnshf")
                nc.vector.tensor_tensor(out=shf[:], in0=bn_s[:, 2:3],
                                        in1=t_c[:], op=Alu.subtract)
                return scl, shf

            def norm_elu_blockwise(dst_s, src_s, scl, shf, D, do_elu):
                for b in range(NB):
                    nsl = slice(b * BLK, (b + 1) * BLK)
                    if not do_elu:
                        nc.vector.tensor_scalar(
                            out=dst_s[:, nsl], in0=src_s[:, nsl],
                            scalar1=scl[:], scalar2=shf[:],
                            op0=Alu.mult, op1=Alu.add)
                        continue
                    hb = spool.tile([D, BLK], f32, tag="nrmh")
                    nc.vector.tensor_scalar(
                        out=hb[:], in0=src_s[:, nsl], scalar1=scl[:],
                        scalar2=shf[:], op0=Alu.mult, op1=Alu.add)
                    m_t = spool.tile([D, BLK], f32, tag="nrmm")
                    nc.vector.tensor_scalar(out=m_t[:], in0=hb[:],
                                            scalar1=0.0, scalar2=None,
                                            op0=Alu.min)
                    e_t = spool.tile([D, BLK], f32, tag="nrme")
                    nc.scalar.activation(out=e_t[:], in_=m_t[:], func=Act.Exp)
                    nc.vector.tensor_scalar(out=m_t[:], in0=hb[:],
                                            scalar1=0.0, scalar2=None,
                                            op0=Alu.max)
                    t_t = spool.tile([D, BLK], f32, tag="nrmt")
                    nc.vector.tensor_tensor(out=t_t[:], in0=e_t[:],
                                            in1=m_t[:], op=Alu.add)
                    nc.vector.tensor_scalar(out=dst_s[:, nsl], in0=t_t[:],
                                            scalar1=-1.0, scalar2=None,
                                            op0=Alu.add)

            # ================= layer 1 =================
            gemm_layer(gemm1_lhs, IN_DIM, w1_s, b1f_s, fsfd1_loc, fsfd1_full)
            edge_layer(H1, F1, fsfd1_full, fsfd1_loc, a1_s, r1_s, bn1_s,
                       h1_s, do_elu=True)
            scl1, shf1 = bn_norm(h1_s, bn1_s, bnin[0], bnout[0], HID)
            norm_elu_blockwise(h_s, h1_s, scl1, shf1, HID, do_elu=True)

            # ================= layer 2 =================
            gemm_layer(lambda t: h_s[:, t * 128:(t + 1) * 128], HID, w2_s,
                       b2f_s, fsfd2_loc, fsfd2_full)
            edge_layer(H2, F2, fsfd2_full, fsfd2_loc, a2_s, r2_s, bn2_s,
                       h1_s, do_elu=False)  # reuse h1_s as h2 buffer
            scl2, shf2 = bn_norm(h1_s, bn2_s, bnin[1], bnout[1], OUT_DIM)
            outb = npool.tile([OUT_DIM, CORE_NODES], f32, tag="h")  # reuse h
            norm_elu_blockwise(outb, h1_s, scl2, shf2, OUT_DIM, do_elu=False)
            nc.sync.dma_start(out=out_d[:, :], in_=outb[:])

    return nc


_PROGRAM_CACHE = {}


def kernel(**inputs) -> np.ndarray:
    import sys
    for p in ("/opt/trn_rl_repo",):
        if os.path.isdir(p) and p not in sys.path:
            sys.path.insert(0, p)
    from concourse.bass_utils import run_bass_kernel_spmd

    in_maps = make_inputs(inputs)
    if "nc" not in _PROGRAM_CACHE:
        nc = build_program()
        nc.finalize()
        _PROGRAM_CACHE["nc"] = nc
    nc = _PROGRAM_CACHE["nc"]
    res = run_bass_kernel_spmd(nc, in_maps, core_ids=list(range(NCORES)))
    outs = [res.results[c]["outT"].T for c in range(NCORES)]  # [6400, 64] each
    return np.ascontiguousarray(np.concatenate(outs, 0)[:N_REAL]).astype(
        np.float32)


if __name__ == "__main__":
    import jax
    with jax.default_device(jax.devices("cpu")[0]):
        import reference
        inputs = {k: np.asarray(v) for k, v in reference.setup_inputs().items()}
        expected = np.asarray(reference.reference(**inputs))
    actual = kernel(**inputs)
    rel = np.linalg.norm(actual - expected) / np.linalg.norm(expected)
    print("Relative error:", rel)



# revision 5
# speedup vs baseline: 32.7415x; 32.7415x over previous
"""Trainium2 Bass kernel for a 2-layer GATv2 encoder (nn_GATv2Encoder).

Strategy (8 NeuronCores, SPMD):
- Host sorts edges by dst; dst nodes are partitioned into contiguous 256-node
  blocks, 25 blocks per core -> each core owns a disjoint contiguous node
  range, so segment softmax/scatter are core-local (no cross-core segment
  reductions needed).
- Each block's edges are split into two sections by src < HALF (int16 index
  range for dma_gather), each section padded to a fixed tile capacity.
- Per layer: node-parallel GEMMs produce a combined [fs|fd] table (512B rows);
  fs tables are AllGather'd; fs[src] and fd[dst] are fetched per edge with
  dma_gather (256B elements). Logits l = sum_f attn*leakyrelu(fs+fd) computed
  without segment-max (logits are tiny); softmax normalization is folded into
  a final num/den division. Scatter-add is a one-hot matmul accumulated in
  PSUM per block (bf16 one-hot + bf16 vals, fp32 accumulation).
- BatchNorm stats are computed per-core over the feature-partition layout and
  AllReduce'd; padded rows are corrected with host-precomputed constants.

Host<->device traffic is minimized (the axon tunnel is ~45 MB/s): x ships as
fp16, gather indices ship un-replicated (int16) and are replicated across the
8 gpsimd partition groups on device, all replicated constant tiles are built
on device from a small packed table, output returns as fp16, and the
ExternalOutput staging zeros are materialized on device instead of uploaded.
"""

import os
import sys
import numpy as np

# ---------------------------------------------------------------- constants
N_REAL = 50000
E_REAL = 800000
IN_DIM, HID, OUT_DIM = 128, 64, 64
H1, F1 = 8, 8
H2, F2 = 4, 16
SLOPE = 0.2
EPS = 1e-5
DEN_EPS = 1e-30

NCORES = 8
BLK = 256                      # nodes per block (one-hot / psum free width)
NB = 25                        # blocks per core
CORE_NODES = BLK * NB          # 6400
NPAD = NCORES * CORE_NODES     # 51200
HALF = NPAD // 2               # 25600 (int16-safe gather split)
NBLK_TOT = NCORES * NB         # 200
TILE = 128                     # edges per matmul tile
SEC_T = 18                     # tiles per section (A and B)
CAP = SEC_T * TILE             # 2304 edge slots per section
T_BLK = 2 * SEC_T              # 36 tiles per block
SLOTS = 2 * CAP                # 4608 edge slots per block
SEC_COLS = CAP // 16           # idx cols per section (144)
BLK_COLS = SLOTS // 16         # idx cols per block (288)
WROWS = 209                    # rows in the packed f32 const table


# ---------------------------------------------------------------- host prep
def host_prep(src, dst):
    """Vectorized edge bucketing. Returns global (concat-over-core) arrays:
    fsc/fdc [NCORES*16, NB*BLK_COLS] i16 (un-replicated dma_gather indices),
    dl [NCORES*128, NB*T_BLK] i16 (dst-local slot->node map, -1 = padding)."""
    src = np.asarray(src).astype(np.int32, copy=False)
    dst = np.asarray(dst).astype(np.int32, copy=False)
    order = np.argsort(dst, kind="stable")
    s_src, s_dst = src[order], dst[order]

    blk = (s_dst >> 8).astype(np.int64)             # block id per edge
    isB = s_src >= HALF
    blk_counts = np.bincount(blk, minlength=NBLK_TOT)
    blk_start = np.zeros(NBLK_TOT, np.int64)
    np.cumsum(blk_counts[:-1], out=blk_start[1:])
    start_e = blk_start[blk]
    nB_bef = np.cumsum(isB) - isB                    # B-count before edge e
    posB = nB_bef - nB_bef[start_e]
    pos_in_blk = np.arange(len(s_src), dtype=np.int64) - start_e
    posA = pos_in_blk - posB

    nB_per = np.bincount(blk[isB], minlength=NBLK_TOT)
    nA_per = blk_counts - nB_per
    assert nA_per.max() <= CAP and nB_per.max() <= CAP, (
        nA_per.max(), nB_per.max())

    slot = np.where(isB, CAP + posB, posA)
    gslot = blk * SLOTS + slot
    fsv = np.where(isB, s_src - HALF, s_src).astype(np.int16)
    fdv = (s_dst - (blk // NB) * CORE_NODES).astype(np.int16)
    dlv = (s_dst & 255).astype(np.int16)

    fs_flat = np.zeros(NBLK_TOT * SLOTS, np.int16)
    fd_flat = np.zeros(NBLK_TOT * SLOTS, np.int16)
    dl_flat = np.full(NBLK_TOT * SLOTS, -1, np.int16)
    fs_flat[gslot] = fsv
    fd_flat[gslot] = fdv
    dl_flat[gslot] = dlv

    # dma_gather layout: idx j of a section at (partition j%16, col j//16)
    def wrap16(flat):
        a = flat.reshape(NCORES, NB * 2, SEC_COLS, 16)
        return np.ascontiguousarray(a.transpose(0, 3, 1, 2)).reshape(
            NCORES * 16, NB * BLK_COLS)

    # one-hot layout: slot s of block b at (partition s%128, col b*T_BLK+s//128)
    d4 = dl_flat.reshape(NCORES, NB, T_BLK, 128)
    dl_g = np.ascontiguousarray(d4.transpose(0, 3, 1, 2)).reshape(
        NCORES * 128, NB * T_BLK)
    return wrap16(fs_flat), wrap16(fd_flat), dl_g


def _elu_np(x):
    return np.where(x > 0, x, np.exp(np.minimum(x, 0)) - 1).astype(np.float32)


def make_operands(inputs):
    """Build the 5 global operand arrays (axis 0 = concat over cores)."""
    f32 = np.float32
    x = np.asarray(inputs["x"], f32)

    xh = np.zeros((IN_DIM, NPAD), np.float16)
    xh[:, :N_REAL] = x.T
    xh_g = np.ascontiguousarray(
        xh.reshape(IN_DIM, NCORES, CORE_NODES).transpose(1, 0, 2)).reshape(
        NCORES * IN_DIM, CORE_NODES)

    fsc_g, fdc_g, dl_g = host_prep(inputs["src"], inputs["dst"])

    # packed f32 const table, replicated per core
    lr_a = (1.0 + SLOPE) / 2.0
    wp = np.zeros((WROWS, 128), f32)
    wp[0:128] = np.concatenate([np.asarray(inputs["Wsrc1"], f32),
                                np.asarray(inputs["Wdst1"], f32)], 1)
    wp[128:192] = np.concatenate([np.asarray(inputs["Wsrc2"], f32),
                                  np.asarray(inputs["Wdst2"], f32)], 1)
    wp[192] = np.concatenate([np.asarray(inputs["bsrc1"], f32),
                              np.asarray(inputs["bdst1"], f32)])
    wp[193] = np.concatenate([np.asarray(inputs["bsrc2"], f32),
                              np.asarray(inputs["bdst2"], f32)])
    wp[194, 0:64] = np.asarray(inputs["attn1"], f32).reshape(-1) * lr_a
    wp[194, 64:128] = np.asarray(inputs["attn2"], f32).reshape(-1) * lr_a

    npad_rows = NPAD - N_REAL
    bias1 = np.asarray(inputs["bias1"], f32)
    bias2 = np.asarray(inputs["bias2"], f32)
    cpad1 = _elu_np(bias1)
    bn1 = np.zeros((HID, 8), f32)
    bn1[:, 0] = bias1
    bn1[:, 1] = np.asarray(inputs["gamma1"], f32)
    bn1[:, 2] = np.asarray(inputs["beta1"], f32)
    bn1[:, 3] = npad_rows * cpad1
    bn1[:, 4] = npad_rows * cpad1 ** 2
    bn1[:, 5] = EPS
    bn2 = np.zeros((OUT_DIM, 8), f32)
    bn2[:, 0] = bias2
    bn2[:, 1] = np.asarray(inputs["gamma2"], f32)
    bn2[:, 2] = np.asarray(inputs["beta2"], f32)
    bn2[:, 3] = npad_rows * bias2
    bn2[:, 4] = npad_rows * bias2 ** 2
    bn2[:, 5] = EPS
    wp[195:199] = bn1.reshape(4, 128)
    wp[199:203] = bn2.reshape(4, 128)

    r1 = np.zeros((H1, HID), f32)
    for h in range(H1):
        r1[h, h * F1:(h + 1) * F1] = 1.0
    r2 = np.zeros((H2, OUT_DIM), f32)
    for h in range(H2):
        r2[h, h * F2:(h + 1) * F2] = 1.0
    wp[203:207] = r1.reshape(4, 128)
    wp[207:209] = r2.reshape(2, 128)
    wp_g = np.tile(wp, (NCORES, 1))

    return [xh_g, fsc_g, fdc_g, dl_g, wp_g]


OPERAND_NAMES = ["xh", "fsc", "fdc", "dl", "wp"]


# ---------------------------------------------------------------- bass program
def build_program():
    import concourse.bacc as bacc
    import concourse.tile as tile
    from concourse import mybir

    f32 = mybir.dt.float32
    f16 = mybir.dt.float16
    bf16 = mybir.dt.bfloat16
    i16 = mybir.dt.int16
    Alu = mybir.AluOpType
    Act = mybir.ActivationFunctionType

    nc = bacc.Bacc(None, target_bir_lowering=False, num_devices=NCORES)
    RG = [list(range(NCORES))]

    # ---- I/O ----
    xh_d = nc.dram_tensor("xh", [IN_DIM, CORE_NODES], f16, kind="ExternalInput")
    fsc_d = nc.dram_tensor("fsc", [16, NB * BLK_COLS], i16, kind="ExternalInput")
    fdc_d = nc.dram_tensor("fdc", [16, NB * BLK_COLS], i16, kind="ExternalInput")
    dl_d = nc.dram_tensor("dl", [128, NB * T_BLK], i16, kind="ExternalInput")
    wp_d = nc.dram_tensor("wp", [WROWS, 128], f32, kind="ExternalInput")
    out_d = nc.dram_tensor("outT", [OUT_DIM, CORE_NODES], f16,
                           kind="ExternalOutput")

    # ---- internal DRAM ----
    fsfd1_loc = nc.dram_tensor("fsfd1_loc", [CORE_NODES, 128], f32)
    fsfd1_full = nc.dram_tensor("fsfd1_full", [NPAD, 128], f32,
                                addr_space="Shared")
    fsfd2_loc = nc.dram_tensor("fsfd2_loc", [CORE_NODES, 128], f32)
    fsfd2_full = nc.dram_tensor("fsfd2_full", [NPAD, 128], f32,
                                addr_space="Shared")
    bnin = [nc.dram_tensor(f"bnin{i}", [64, 2], f32) for i in (1, 2)]
    bnout = [nc.dram_tensor(f"bnout{i}", [64, 2], f32, addr_space="Shared")
             for i in (1, 2)]

    with tile.TileContext(nc) as tc:
        with (
            tc.tile_pool(name="const", bufs=1) as cpool,
            tc.tile_pool(name="gath", bufs=2) as gpool,
            tc.tile_pool(name="work", bufs=2) as wpool,
            tc.tile_pool(name="small", bufs=2) as spool,
            tc.tile_pool(name="node", bufs=1) as npool,
            tc.tile_pool(name="psA", bufs=2, space="PSUM") as psA,
            tc.tile_pool(name="psB", bufs=2, space="PSUM") as psB,
            tc.tile_pool(name="psG", bufs=2, space="PSUM") as psG,
        ):
            # ---- load + derive constants ----
            w1_s = cpool.tile([IN_DIM, 128], f32, tag="w1")
            nc.sync.dma_start(out=w1_s[:], in_=wp_d[0:128, :])
            w1h_s = cpool.tile([IN_DIM, 128], f16, tag="w1h")
            nc.vector.tensor_copy(out=w1h_s[:], in_=w1_s[:])
            w2_s = cpool.tile([HID, 128], f32, tag="w2")
            nc.sync.dma_start(out=w2_s[:], in_=wp_d[128:192, :])
            brow1_s = cpool.tile([1, 128], f32, tag="brow1")
            nc.sync.dma_start(out=brow1_s[:], in_=wp_d[192:193, :])
            brow2_s = cpool.tile([1, 128], f32, tag="brow2")
            nc.sync.dma_start(out=brow2_s[:], in_=wp_d[193:194, :])
            arow_s = cpool.tile([1, 128], f32, tag="arow")
            nc.sync.dma_start(out=arow_s[:], in_=wp_d[194:195, :])
            bn1_s = cpool.tile([HID, 8], f32, tag="bn1")
            nc.sync.dma_start(
                out=bn1_s[:],
                in_=wp_d[195:199, :].rearrange("a (b c) -> (a b) c", c=8))
            bn2_s = cpool.tile([OUT_DIM, 8], f32, tag="bn2")
            nc.sync.dma_start(
                out=bn2_s[:],
                in_=wp_d[199:203, :].rearrange("a (b c) -> (a b) c", c=8))
            r1_s = cpool.tile([H1, HID], f32, tag="r1")
            nc.sync.dma_start(
                out=r1_s[:],
                in_=wp_d[203:207, :].rearrange("a (b c) -> (a b) c", c=64))
            r2_s = cpool.tile([H2, OUT_DIM], f32, tag="r2")
            nc.sync.dma_start(
                out=r2_s[:],
                in_=wp_d[207:209, :].rearrange("a (b c) -> (a b) c", c=64))

            # gather idx tiles: replicate [16, C] across the 8 Q7 core groups
            fsidx_s = cpool.tile([128, NB * BLK_COLS], i16, tag="fsidx")
            fdidx_s = cpool.tile([128, NB * BLK_COLS], i16, tag="fdidx")
            for g in range(8):
                nc.sync.dma_start(out=fsidx_s[16 * g:16 * g + 16, :],
                                  in_=fsc_d[:, :])
                nc.sync.dma_start(out=fdidx_s[16 * g:16 * g + 16, :],
                                  in_=fdc_d[:, :])

            dli_s = cpool.tile([128, NB * T_BLK], i16, tag="dli")
            nc.sync.dma_start(out=dli_s[:], in_=dl_d[:, :])
            dloc_s = cpool.tile([128, NB * T_BLK], f32, tag="dloc")
            nc.vector.tensor_copy(out=dloc_s[:], in_=dli_s[:])

            iota_s = cpool.tile([128, BLK], f32, tag="iota")
            nc.gpsimd.iota(iota_s[:], [[1, BLK]], channel_multiplier=0,
                           allow_small_or_imprecise_dtypes=True)

            ones_s = cpool.tile([1, 128], f32, tag="ones")
            nc.vector.memset(ones_s[:], 1.0)

            def bcast128(row_ap, w, tag):
                ps = psB.tile([128, w], f32, tag="bc_ps")
                nc.tensor.matmul(out=ps[:], lhsT=ones_s[:], rhs=row_ap,
                                 start=True, stop=True)
                t = cpool.tile([128, w], f32, tag=tag)
                nc.vector.tensor_copy(out=t[:], in_=ps[:])
                return t

            b1f_s = bcast128(brow1_s[0:1, :], 128, "b1f")
            b2f_s = bcast128(brow2_s[0:1, :], 128, "b2f")
            a1_s = bcast128(arow_s[0:1, 0:64], HID, "a1f")
            a2_s = bcast128(arow_s[0:1, 64:128], OUT_DIM, "a2f")

            h1_s = npool.tile([HID, CORE_NODES], f32, tag="h1")
            h_s = npool.tile([HID, CORE_NODES], f32, tag="h")

            NT = CORE_NODES // 128  # node tiles per core for GEMMs

            def gemm_layer(get_lhs, K, w_s, bfull_s, loc_dram, full_dram):
                for t in range(NT):
                    ps = psG.tile([128, 128], f32)
                    nc.tensor.matmul(
                        out=ps[:], lhsT=get_lhs(t),
                        rhs=w_s[:K, :], start=True, stop=True)
                    gs = spool.tile([128, 128], f32, tag="gemm")
                    nc.vector.tensor_tensor(
                        out=gs[:], in0=ps[:], in1=bfull_s[:], op=Alu.add)
                    nc.sync.dma_start(
                        out=loc_dram[t * 128:(t + 1) * 128, :], in_=gs[:])
                nc.gpsimd.collective_compute(
                    "AllGather", Alu.bypass, replica_groups=RG,
                    ins=[loc_dram.ap().opt()], outs=[full_dram.ap().opt()])

            def gemm1_lhs(t):
                xg = spool.tile([IN_DIM, 128], f16, tag="xg")
                nc.sync.dma_start(out=xg[:], in_=xh_d[:, t * 128:(t + 1) * 128])
                return xg[:]

            def edge_layer(Hh, Ff, full_dram, loc_dram, attn_s, rX_s, bn_s,
                           hout_s, do_elu):
                HF = Hh * Ff
                VW = Hh + HF  # vals width per tile
                for b in range(NB):
                    ps_s = psA.tile([VW, BLK], f32, tag="scat")
                    for sec in range(2):
                        cA = b * BLK_COLS + sec * SEC_COLS
                        fs_e = gpool.tile([128, SEC_T * 64], f32, tag="fs")
                        fd_e = gpool.tile([128, SEC_T * 64], f32, tag="fd")
                        tab = full_dram[:, 0:64] if sec == 0 else \
                            full_dram[HALF:, 0:64]
                        nc.gpsimd.dma_gather(
                            out_ap=fs_e[:].rearrange("p (t e) -> p t e", e=64),
                            in_ap=tab,
                            idxs_ap=fsidx_s[:, cA:cA + SEC_COLS],
                            num_idxs=CAP, num_idxs_reg=CAP,
                            elem_size=64, elem_step=128, single_packet=False)
                        nc.gpsimd.dma_gather(
                            out_ap=fd_e[:].rearrange("p (t e) -> p t e", e=64),
                            in_ap=loc_dram[:, 64:128],
                            idxs_ap=fdidx_s[:, cA:cA + SEC_COLS],
                            num_idxs=CAP, num_idxs_reg=CAP,
                            elem_size=64, elem_step=128, single_packet=False)

                        # one-hot (bf16): O[p, t, n] = (dloc[p, t] == n)
                        dcol = b * T_BLK + sec * SEC_T
                        O_t = wpool.tile([128, SEC_T * BLK], bf16, tag="O")
                        nc.vector.tensor_tensor(
                            out=O_t[:].rearrange("p (t n) -> p t n", n=BLK),
                            in0=dloc_s[:, dcol:dcol + SEC_T, None]
                                .to_broadcast([128, SEC_T, BLK]),
                            in1=iota_s[:, None, :]
                                .to_broadcast([128, SEC_T, BLK]),
                            op=Alu.is_equal)

                        # z = fs + fd ; lrelu(z)*attn = (z + c|z|) * attn06
                        z_t = wpool.tile([128, SEC_T * 64], f32, tag="z")
                        wz_t = wpool.tile([128, SEC_T * 64], f32, tag="wz")
                        nc.vector.tensor_tensor(
                            out=z_t[:], in0=fs_e[:], in1=fd_e[:], op=Alu.add)
                        nc.scalar.activation(
                            out=wz_t[:], in_=z_t[:], func=Act.Abs,
                            scale=(1.0 - SLOPE) / (1.0 + SLOPE))
                        nc.vector.tensor_tensor(
                            out=wz_t[:], in0=z_t[:], in1=wz_t[:], op=Alu.add)
                        nc.vector.tensor_tensor(
                            out=z_t[:].rearrange("p (t e) -> p t e", e=64),
                            in0=wz_t[:].rearrange("p (t e) -> p t e", e=64),
                            in1=attn_s[:, None, :]
                                .to_broadcast([128, SEC_T, 64]),
                            op=Alu.mult)
                        # l = sum_f wz ; p = exp(l) into vals
                        l_t = spool.tile([128, SEC_T * Hh], f32, tag="l")
                        nc.vector.tensor_reduce(
                            out=l_t[:],
                            in_=z_t[:].rearrange("p (t h f) -> p t h f",
                                                 h=Hh, f=Ff),
                            axis=mybir.AxisListType.X, op=Alu.add)
                        # vals layout: [p*fs (HF cols) | p (Hh cols)] so that
                        # psum num rows start at partition 0, den at HF (=64)
                        vals = wpool.tile([128, SEC_T * VW], bf16, tag="vals")
                        vals3 = vals[:].rearrange("p (t v) -> p t v", v=VW)
                        nc.scalar.activation(
                            out=vals3[:, :, HF:VW],
                            in_=l_t[:].rearrange("p (t h) -> p t h", h=Hh),
                            func=Act.Exp)
                        nc.vector.tensor_tensor(
                            out=vals3[:, :, 0:HF].rearrange(
                                "p t (h f) -> p t h f", f=Ff),
                            in0=fs_e[:].rearrange("p (t h f) -> p t h f",
                                                  h=Hh, f=Ff),
                            in1=vals3[:, :, HF:VW][:, :, :, None]
                                .to_broadcast([128, SEC_T, Hh, Ff]),
                            op=Alu.mult)

                        # scatter: psum[v, n] += sum_e vals[e, v] * O[e, n]
                        for t in range(SEC_T):
                            nc.tensor.matmul(
                                out=ps_s[:],
                                lhsT=vals[:, t * VW:(t + 1) * VW],
                                rhs=O_t[:, t * BLK:(t + 1) * BLK],
                                start=(sec == 0 and t == 0),
                                stop=(sec == 1 and t == SEC_T - 1))

                    # normalize: out = num * (1/den) + bias
                    den = spool.tile([Hh, BLK], f32, tag="den")
                    nc.vector.tensor_scalar(
                        out=den[:], in0=ps_s[HF:VW, :], scalar1=DEN_EPS,
                        scalar2=None, op0=Alu.add)
                    rcp = spool.tile([Hh, BLK], f32, tag="rcp")
                    nc.vector.reciprocal(out=rcp[:], in_=den[:])
                    ps_r = psB.tile([HF, BLK], f32, tag="rrep")
                    nc.tensor.matmul(out=ps_r[:], lhsT=rX_s[:], rhs=rcp[:],
                                     start=True, stop=True)
                    rr = spool.tile([HF, BLK], f32, tag="rr")
                    nc.vector.tensor_copy(out=rr[:], in_=ps_r[:])
                    o1 = spool.tile([HF, BLK], f32, tag="o1")
                    nc.vector.tensor_tensor(
                        out=o1[:], in0=ps_s[0:HF, :], in1=rr[:], op=Alu.mult)
                    nsl = slice(b * BLK, (b + 1) * BLK)
                    if do_elu:
                        ob = spool.tile([HF, BLK], f32, tag="ob")
                        nc.vector.tensor_scalar(
                            out=ob[:], in0=o1[:], scalar1=bn_s[:, 0:1],
                            scalar2=None, op0=Alu.add)
                        m_t = spool.tile([HF, BLK], f32, tag="elum")
                        nc.vector.tensor_scalar(
                            out=m_t[:], in0=ob[:], scalar1=0.0,
                            scalar2=None, op0=Alu.min)
                        e_t = spool.tile([HF, BLK], f32, tag="elue")
                        nc.scalar.activation(out=e_t[:], in_=m_t[:],
                                             func=Act.Exp)
                        nc.vector.tensor_scalar(
                            out=m_t[:], in0=ob[:], scalar1=0.0,
                            scalar2=None, op0=Alu.max)
                        t_t = spool.tile([HF, BLK], f32, tag="elut")
                        nc.vector.tensor_tensor(
                            out=t_t[:], in0=e_t[:], in1=m_t[:], op=Alu.add)
                        nc.vector.tensor_scalar(
                            out=hout_s[:, nsl], in0=t_t[:], scalar1=-1.0,
                            scalar2=None, op0=Alu.add)
                    else:
                        nc.vector.tensor_scalar(
                            out=hout_s[:, nsl], in0=o1[:], scalar1=bn_s[:, 0:1],
                            scalar2=None, op0=Alu.add)

            def bn_norm(hin_s, bn_s, bnin_d, bnout_d, D):
                """BN stats (blockwise) + AllReduce; returns (scale, shift)."""
                s_cols = spool.tile([D, NB], f32, tag="bnscols")
                q_cols = spool.tile([D, NB], f32, tag="bnqcols")
                for b in range(NB):
                    nsl = slice(b * BLK, (b + 1) * BLK)
                    nc.vector.tensor_reduce(
                        out=s_cols[:, b:b + 1], in_=hin_s[:, nsl],
                        axis=mybir.AxisListType.X, op=Alu.add)
                    scr = spool.tile([D, BLK], f32, tag="bnscr")
                    nc.scalar.activation(
                        out=scr[:], in_=hin_s[:, nsl], func=Act.Square,
                        accum_out=q_cols[:, b:b + 1])
                st = spool.tile([D, 2], f32, tag="bnst")
                nc.vector.tensor_reduce(out=st[:, 0:1], in_=s_cols[:],
                                        axis=mybir.AxisListType.X, op=Alu.add)
                nc.vector.tensor_reduce(out=st[:, 1:2], in_=q_cols[:],
                                        axis=mybir.AxisListType.X, op=Alu.add)
                nc.sync.dma_start(out=bnin_d[:, :], in_=st[:])
                nc.gpsimd.collective_compute(
                    "AllReduce", Alu.add, replica_groups=RG,
                    ins=[bnin_d.ap().opt()], outs=[bnout_d.ap().opt()])
                g = spool.tile([D, 2], f32, tag="bng")
                nc.sync.dma_start(out=g[:], in_=bnout_d[:, :])
                # mu = (S - corr)/N ; var = (SQ - corrsq)/N - mu^2
                t_a = spool.tile([D, 1], f32, tag="bnta")
                nc.vector.tensor_tensor(out=t_a[:], in0=g[:, 0:1],
                                        in1=bn_s[:, 3:4], op=Alu.subtract)
                mu = spool.tile([D, 1], f32, tag="bnmu")
                nc.vector.tensor_scalar(out=mu[:], in0=t_a[:],
                                        scalar1=1.0 / N_REAL, scalar2=None,
                                        op0=Alu.mult)
                t_b = spool.tile([D, 1], f32, tag="bntb")
                nc.vector.tensor_tensor(out=t_b[:], in0=g[:, 1:2],
                                        in1=bn_s[:, 4:5], op=Alu.subtract)
                msq = spool.tile([D, 1], f32, tag="bnmsq")
                nc.vector.tensor_scalar(out=msq[:], in0=t_b[:],
                                        scalar1=1.0 / N_REAL, scalar2=None,
                                        op0=Alu.mult)
                mu2 = spool.tile([D, 1], f32, tag="bnmu2")
                nc.vector.tensor_tensor(out=mu2[:], in0=mu[:], in1=mu[:],
                                        op=Alu.mult)
                var = spool.tile([D, 1], f32, tag="bnvar")
                nc.vector.tensor_tensor(out=var[:], in0=msq[:], in1=mu2[:],
                                        op=Alu.subtract)
                sd = spool.tile([D, 1], f32, tag="bnsd")
                nc.scalar.activation(out=sd[:], in_=var[:], func=Act.Sqrt,
                                     bias=bn_s[:, 5:6])
                rs = spool.tile([D, 1], f32, tag="bnrs")
                nc.vector.reciprocal(out=rs[:], in_=sd[:])
                scl = spool.tile([D, 1], f32, tag="bnscl")
                nc.vector.tensor_tensor(out=scl[:], in0=bn_s[:, 1:2],
                                        in1=rs[:], op=Alu.mult)
                t_c = spool.tile([D, 1], f32, tag="bntc")
                nc.vector.tensor_tensor(out=t_c[:], in0=mu[:], in1=scl[:],
                                        op=Alu.mult)
                shf = spool.tile([D, 1], f32, tag="bnshf")
                nc.vector.tensor_tensor(out=shf[:], in0=bn_s[:, 2:3],
                                        in1=t_c[:], op=Alu.subtract)
                return scl, shf

            def norm_elu_blockwise(dst_s, src_s, scl, shf, D, do_elu):
                for b in range(NB):
                    nsl = slice(b * BLK, (b + 1) * BLK)
                    if not do_elu:
                        nc.vector.tensor_scalar(
                            out=dst_s[:, nsl], in0=src_s[:, nsl],
                            scalar1=scl[:], scalar2=shf[:],
                            op0=Alu.mult, op1=Alu.add)
                        continue
                    hb = spool.tile([D, BLK], f32, tag="nrmh")
                    nc.vector.tensor_scalar(
                        out=hb[:], in0=src_s[:, nsl], scalar1=scl[:],
                        scalar2=shf[:], op0=Alu.mult, op1=Alu.add)
                    m_t = spool.tile([D, BLK], f32, tag="nrmm")
                    nc.vector.tensor_scalar(out=m_t[:], in0=hb[:],
                                            scalar1=0.0, scalar2=None,
                                            op0=Alu.min)
                    e_t = spool.tile([D, BLK], f32, tag="nrme")
                    nc.scalar.activation(out=e_t[:], in_=m_t[:], func=Act.Exp)
                    nc.vector.tensor_scalar(out=m_t[:], in0=hb[:],
                                            scalar1=0.0, scalar2=None,
                                            op0=Alu.max)
                    t_t = spool.tile([D, BLK], f32, tag="nrmt")
                    nc.vector.tensor_tensor(out=t_t[:], in0=e_t[:],
                                            in1=m_t[:], op=Alu.add)
                    nc.vector.tensor_scalar(out=dst_s[:, nsl], in0=t_t[:],
                                            scalar1=-1.0, scalar2=None,
                                            op0=Alu.add)

            # ================= layer 1 =================
            gemm_layer(gemm1_lhs, IN_DIM, w1h_s, b1f_s, fsfd1_loc, fsfd1_full)
            edge_layer(H1, F1, fsfd1_full, fsfd1_loc, a1_s, r1_s, bn1_s,
                       h1_s, do_elu=True)
            scl1, shf1 = bn_norm(h1_s, bn1_s, bnin[0], bnout[0], HID)
            norm_elu_blockwise(h_s, h1_s, scl1, shf1, HID, do_elu=True)

            # ================= layer 2 =================
            gemm_layer(lambda t: h_s[:, t * 128:(t + 1) * 128], HID, w2_s,
                       b2f_s, fsfd2_loc, fsfd2_full)
            edge_layer(H2, F2, fsfd2_full, fsfd2_loc, a2_s, r2_s, bn2_s,
                       h1_s, do_elu=False)  # reuse h1_s as h2 buffer
            scl2, shf2 = bn_norm(h1_s, bn2_s, bnin[1], bnout[1], OUT_DIM)
            outb = npool.tile([OUT_DIM, CORE_NODES], f16, tag="outb")
            norm_elu_blockwise(outb, h1_s, scl2, shf2, OUT_DIM, do_elu=False)
            nc.sync.dma_start(out=out_d[:, :], in_=outb[:])

    return nc


# ---------------------------------------------------------------- pjrt runner
_CACHE = {}


def _get_runtime():
    if "rt" in _CACHE:
        return _CACHE["rt"]
    for p in ("/opt/trn_rl_repo",):
        if os.path.isdir(p) and p not in sys.path:
            sys.path.insert(0, p)
    import jax
    import jax.numpy as jnp
    from jax.sharding import Mesh, PartitionSpec
    from jax.experimental.shard_map import shard_map
    from concourse import mybir
    from concourse.bass2jax import (_bass_exec_p, partition_id_tensor,
                                    install_neuronx_cc_hook)

    install_neuronx_cc_hook()
    nc = build_program()
    nc.finalize()

    partition_name = (nc.partition_id_tensor.name
                      if nc.partition_id_tensor else None)
    dbg_name = nc.dbg_addr.name if nc.dbg_addr is not None else None
    in_names, out_names, out_info = [], [], []
    for alloc in nc.m.functions[0].allocations:
        if not isinstance(alloc, mybir.MemoryLocationSet):
            continue
        name = alloc.memorylocations[0].name
        if alloc.kind == "ExternalInput":
            if name != partition_name:
                in_names.append(name)
        elif alloc.kind == "ExternalOutput":
            out_names.append(name)
            out_info.append((tuple(alloc.tensor_shape),
                             mybir.dt.np(alloc.dtype)))
    in_names_all = tuple(in_names + out_names
                         + ([partition_name] if partition_name else []))
    out_avals = tuple(jax.core.ShapedArray(s, d) for s, d in out_info)

    assert dbg_name is None and in_names == OPERAND_NAMES, (dbg_name, in_names)
    n_params = len(in_names)

    def _body(*args):
        operands = list(args)
        if partition_name is not None:
            operands.append(partition_id_tensor())
        return tuple(_bass_exec_p.bind(
            *operands, out_avals=out_avals, in_names=in_names_all,
            out_names=tuple(out_names), lowering_input_output_aliases=(),
            sim_require_finite=True, sim_require_nnan=True, nc=nc))

    devices = jax.devices()[:NCORES]
    assert len(devices) == NCORES
    mesh = Mesh(np.asarray(devices), ("core",))
    from jax.sharding import NamedSharding
    spec = NamedSharding(mesh, PartitionSpec("core"))
    fn = jax.jit(shard_map(
        _body, mesh=mesh,
        in_specs=(PartitionSpec("core"),) * (n_params + len(out_names)),
        out_specs=(PartitionSpec("core"),) * len(out_names),
        check_rep=False),
        donate_argnums=tuple(range(n_params, n_params + len(out_names))))
    # ExternalOutput staging buffers, zero-filled on device (never uploaded).
    # Donated into the bass_exec results; the kernel writes every element.
    zfn = jax.jit(
        lambda: tuple(jnp.zeros((NCORES * s[0], *s[1:]), d)
                      for s, d in out_info),
        out_shardings=(spec,) * len(out_info))
    _CACHE["rt"] = (fn, zfn, mesh)
    return _CACHE["rt"]


def kernel(**inputs) -> np.ndarray:
    fn, zfn, _ = _get_runtime()
    ops = make_operands(inputs)
    (out,) = fn(*ops, *zfn())
    arr = np.asarray(out)  # [NCORES*OUT_DIM, CORE_NODES] f16
    res = arr.reshape(NCORES, OUT_DIM, CORE_NODES).transpose(0, 2, 1)
    return np.ascontiguousarray(
        res.reshape(NPAD, OUT_DIM)[:N_REAL]).astype(np.float32)


if __name__ == "__main__":
    import jax
    with jax.default_device(jax.devices("cpu")[0]):
        import reference
        inputs = {k: np.asarray(v) for k, v in reference.setup_inputs().items()}
        expected = np.asarray(reference.reference(**inputs))
    actual = kernel(**inputs)
    rel = np.linalg.norm(actual - expected) / np.linalg.norm(expected)
    print("Relative error:", rel)


# revision 7
# speedup vs baseline: 32.8266x; 1.0026x over previous
"""Trainium2 Bass kernel for a 2-layer GATv2 encoder (nn_GATv2Encoder).

Strategy (8 NeuronCores, SPMD):
- Host sorts edges by dst; dst nodes are partitioned into contiguous 256-node
  blocks, 25 blocks per core -> each core owns a disjoint contiguous node
  range, so segment softmax/scatter are core-local (no cross-core segment
  reductions needed).
- Each block's edges are split into two sections by src < HALF (int16 index
  range for dma_gather), each section padded to a fixed tile capacity.
- Per layer: node-parallel GEMMs produce a combined [fs|fd] table (512B rows);
  fs tables are AllGather'd; fs[src] and fd[dst] are fetched per edge with
  dma_gather (256B elements). Logits l = sum_f attn*leakyrelu(fs+fd) computed
  without segment-max (logits are tiny); softmax normalization is folded into
  a final num/den division. Scatter-add is a one-hot matmul accumulated in
  PSUM per block (bf16 one-hot + bf16 vals, fp32 accumulation).
- BatchNorm stats are computed per-core over the feature-partition layout and
  AllReduce'd; padded rows are corrected with host-precomputed constants.

Host<->device traffic is minimized (the axon tunnel is ~45 MB/s): x ships as
fp16, gather indices ship un-replicated (int16) and are replicated across the
8 gpsimd partition groups on device, all replicated constant tiles are built
on device from a small packed table, output returns as fp16, and the
ExternalOutput staging zeros are materialized on device instead of uploaded.
"""

import os
import sys
import numpy as np

# ---------------------------------------------------------------- constants
N_REAL = 50000
E_REAL = 800000
IN_DIM, HID, OUT_DIM = 128, 64, 64
H1, F1 = 8, 8
H2, F2 = 4, 16
SLOPE = 0.2
EPS = 1e-5
DEN_EPS = 1e-30

NCORES = 8
BLK = 256                      # nodes per block (one-hot / psum free width)
NB = 25                        # blocks per core
CORE_NODES = BLK * NB          # 6400
NPAD = NCORES * CORE_NODES     # 51200
HALF = NPAD // 2               # 25600 (int16-safe gather split)
NBLK_TOT = NCORES * NB         # 200
TILE = 128                     # edges per matmul tile
SEC_T = 18                     # tiles per section (A and B)
CAP = SEC_T * TILE             # 2304 edge slots per section
T_BLK = 2 * SEC_T              # 36 tiles per block
SLOTS = 2 * CAP                # 4608 edge slots per block
SEC_COLS = CAP // 16           # idx cols per section (144)
BLK_COLS = SLOTS // 16         # idx cols per block (288)
WROWS = 209                    # rows in the packed f32 const table


# ---------------------------------------------------------------- host prep
def host_prep(src, dst):
    """Vectorized edge bucketing. Returns global (concat-over-core) arrays:
    fsc/fdc [NCORES*16, NB*BLK_COLS] i16 (un-replicated dma_gather indices),
    dl [NCORES*128, NB*T_BLK] i16 (dst-local slot->node map, -1 = padding)."""
    src = np.asarray(src).astype(np.int32, copy=False)
    dst = np.asarray(dst).astype(np.int32, copy=False)
    # only the dst block (0..199, fits uint8) matters for bucketing — a
    # stable argsort on uint8 keys is a single radix pass
    blk_of = (dst >> 8).astype(np.uint8)
    order = np.argsort(blk_of, kind="stable")
    s_src, s_dst = src[order], dst[order]

    blk = (s_dst >> 8).astype(np.int32)             # block id per edge
    isB = s_src >= HALF
    blk_counts = np.bincount(blk, minlength=NBLK_TOT)
    blk_start = np.zeros(NBLK_TOT, np.int64)
    np.cumsum(blk_counts[:-1], out=blk_start[1:])
    start_e = blk_start[blk]
    nB_bef = np.cumsum(isB, dtype=np.int32) - isB    # B-count before edge e
    posB = nB_bef - nB_bef[start_e]
    pos_in_blk = (np.arange(len(s_src), dtype=np.int64) - start_e).astype(
        np.int32)
    posA = pos_in_blk - posB

    nB_per = np.bincount(blk[isB], minlength=NBLK_TOT)
    nA_per = blk_counts - nB_per
    assert nA_per.max() <= CAP and nB_per.max() <= CAP, (
        nA_per.max(), nB_per.max())

    slot = np.where(isB, CAP + posB, posA)
    gslot = blk * np.int32(SLOTS) + slot
    fsv = np.where(isB, s_src - HALF, s_src).astype(np.int16)
    fdv = (s_dst - (blk // NB) * CORE_NODES).astype(np.int16)
    dlv = (s_dst & 255).astype(np.int16)

    fs_flat = np.zeros(NBLK_TOT * SLOTS, np.int16)
    fd_flat = np.zeros(NBLK_TOT * SLOTS, np.int16)
    dl_flat = np.full(NBLK_TOT * SLOTS, -1, np.int16)
    fs_flat[gslot] = fsv
    fd_flat[gslot] = fdv
    dl_flat[gslot] = dlv

    # dma_gather layout: idx j of a section at (partition j%16, col j//16)
    def wrap16(flat):
        a = flat.reshape(NCORES, NB * 2, SEC_COLS, 16)
        return np.ascontiguousarray(a.transpose(0, 3, 1, 2)).reshape(
            NCORES * 16, NB * BLK_COLS)

    # one-hot layout: slot s of block b at (partition s%128, col b*T_BLK+s//128)
    d4 = dl_flat.reshape(NCORES, NB, T_BLK, 128)
    dl_g = np.ascontiguousarray(d4.transpose(0, 3, 1, 2)).reshape(
        NCORES * 128, NB * T_BLK)
    return wrap16(fs_flat), wrap16(fd_flat), dl_g


def _elu_np(x):
    return np.where(x > 0, x, np.exp(np.minimum(x, 0)) - 1).astype(np.float32)


def make_operands(inputs):
    """Build the 5 global operand arrays (axis 0 = concat over cores)."""
    f32 = np.float32
    x = np.asarray(inputs["x"], f32)

    xh_g = np.zeros((NCORES * IN_DIM, CORE_NODES), np.float16)
    xv = x.T  # [128, 50000] view
    for c in range(NCORES):
        lo = c * CORE_NODES
        hi = min(lo + CORE_NODES, N_REAL)
        if hi > lo:
            xh_g[c * IN_DIM:(c + 1) * IN_DIM, :hi - lo] = xv[:, lo:hi]

    fsc_g, fdc_g, dl_g = host_prep(inputs["src"], inputs["dst"])

    # packed f32 const table, replicated per core
    lr_a = (1.0 + SLOPE) / 2.0
    wp = np.zeros((WROWS, 128), f32)
    wp[0:128] = np.concatenate([np.asarray(inputs["Wsrc1"], f32),
                                np.asarray(inputs["Wdst1"], f32)], 1)
    wp[128:192] = np.concatenate([np.asarray(inputs["Wsrc2"], f32),
                                  np.asarray(inputs["Wdst2"], f32)], 1)
    wp[192] = np.concatenate([np.asarray(inputs["bsrc1"], f32),
                              np.asarray(inputs["bdst1"], f32)])
    wp[193] = np.concatenate([np.asarray(inputs["bsrc2"], f32),
                              np.asarray(inputs["bdst2"], f32)])
    wp[194, 0:64] = np.asarray(inputs["attn1"], f32).reshape(-1) * lr_a
    wp[194, 64:128] = np.asarray(inputs["attn2"], f32).reshape(-1) * lr_a

    npad_rows = NPAD - N_REAL
    bias1 = np.asarray(inputs["bias1"], f32)
    bias2 = np.asarray(inputs["bias2"], f32)
    cpad1 = _elu_np(bias1)
    bn1 = np.zeros((HID, 8), f32)
    bn1[:, 0] = bias1
    bn1[:, 1] = np.asarray(inputs["gamma1"], f32)
    bn1[:, 2] = np.asarray(inputs["beta1"], f32)
    bn1[:, 3] = npad_rows * cpad1
    bn1[:, 4] = npad_rows * cpad1 ** 2
    bn1[:, 5] = EPS
    bn2 = np.zeros((OUT_DIM, 8), f32)
    bn2[:, 0] = bias2
    bn2[:, 1] = np.asarray(inputs["gamma2"], f32)
    bn2[:, 2] = np.asarray(inputs["beta2"], f32)
    bn2[:, 3] = npad_rows * bias2
    bn2[:, 4] = npad_rows * bias2 ** 2
    bn2[:, 5] = EPS
    wp[195:199] = bn1.reshape(4, 128)
    wp[199:203] = bn2.reshape(4, 128)

    r1 = np.zeros((H1, HID), f32)
    for h in range(H1):
        r1[h, h * F1:(h + 1) * F1] = 1.0
    r2 = np.zeros((H2, OUT_DIM), f32)
    for h in range(H2):
        r2[h, h * F2:(h + 1) * F2] = 1.0
    wp[203:207] = r1.reshape(4, 128)
    wp[207:209] = r2.reshape(2, 128)
    wp_g = np.tile(wp, (NCORES, 1))

    return [xh_g, fsc_g, fdc_g, dl_g, wp_g]


OPERAND_NAMES = ["xh", "fsc", "fdc", "dl", "wp"]


# ---------------------------------------------------------------- bass program
def build_program():
    import concourse.bacc as bacc
    import concourse.tile as tile
    from concourse import mybir

    f32 = mybir.dt.float32
    f16 = mybir.dt.float16
    bf16 = mybir.dt.bfloat16
    i16 = mybir.dt.int16
    Alu = mybir.AluOpType
    Act = mybir.ActivationFunctionType

    nc = bacc.Bacc(None, target_bir_lowering=False, num_devices=NCORES)
    RG = [list(range(NCORES))]

    # ---- I/O ----
    xh_d = nc.dram_tensor("xh", [IN_DIM, CORE_NODES], f16, kind="ExternalInput")
    fsc_d = nc.dram_tensor("fsc", [16, NB * BLK_COLS], i16, kind="ExternalInput")
    fdc_d = nc.dram_tensor("fdc", [16, NB * BLK_COLS], i16, kind="ExternalInput")
    dl_d = nc.dram_tensor("dl", [128, NB * T_BLK], i16, kind="ExternalInput")
    wp_d = nc.dram_tensor("wp", [WROWS, 128], f32, kind="ExternalInput")
    out_d = nc.dram_tensor("outT", [OUT_DIM, CORE_NODES], f16,
                           kind="ExternalOutput")

    # ---- internal DRAM ----
    fsfd1_loc = nc.dram_tensor("fsfd1_loc", [CORE_NODES, 128], f32)
    fsfd1_full = nc.dram_tensor("fsfd1_full", [NPAD, 128], f32,
                                addr_space="Shared")
    fsfd2_loc = nc.dram_tensor("fsfd2_loc", [CORE_NODES, 128], f32)
    fsfd2_full = nc.dram_tensor("fsfd2_full", [NPAD, 128], f32,
                                addr_space="Shared")
    bnin = [nc.dram_tensor(f"bnin{i}", [64, 2], f32) for i in (1, 2)]
    bnout = [nc.dram_tensor(f"bnout{i}", [64, 2], f32, addr_space="Shared")
             for i in (1, 2)]

    with tile.TileContext(nc) as tc:
        with (
            tc.tile_pool(name="const", bufs=1) as cpool,
            tc.tile_pool(name="gath", bufs=2) as gpool,
            tc.tile_pool(name="work", bufs=2) as wpool,
            tc.tile_pool(name="small", bufs=2) as spool,
            tc.tile_pool(name="node", bufs=1) as npool,
            tc.tile_pool(name="psA", bufs=2, space="PSUM") as psA,
            tc.tile_pool(name="psB", bufs=2, space="PSUM") as psB,
            tc.tile_pool(name="psG", bufs=2, space="PSUM") as psG,
        ):
            # ---- load + derive constants ----
            w1_s = cpool.tile([IN_DIM, 128], f32, tag="w1")
            nc.sync.dma_start(out=w1_s[:], in_=wp_d[0:128, :])
            w1h_s = cpool.tile([IN_DIM, 128], f16, tag="w1h")
            nc.vector.tensor_copy(out=w1h_s[:], in_=w1_s[:])
            w2_s = cpool.tile([HID, 128], f32, tag="w2")
            nc.sync.dma_start(out=w2_s[:], in_=wp_d[128:192, :])
            brow1_s = cpool.tile([1, 128], f32, tag="brow1")
            nc.sync.dma_start(out=brow1_s[:], in_=wp_d[192:193, :])
            brow2_s = cpool.tile([1, 128], f32, tag="brow2")
            nc.sync.dma_start(out=brow2_s[:], in_=wp_d[193:194, :])
            arow_s = cpool.tile([1, 128], f32, tag="arow")
            nc.sync.dma_start(out=arow_s[:], in_=wp_d[194:195, :])
            bn1_s = cpool.tile([HID, 8], f32, tag="bn1")
            nc.sync.dma_start(
                out=bn1_s[:],
                in_=wp_d[195:199, :].rearrange("a (b c) -> (a b) c", c=8))
            bn2_s = cpool.tile([OUT_DIM, 8], f32, tag="bn2")
            nc.sync.dma_start(
                out=bn2_s[:],
                in_=wp_d[199:203, :].rearrange("a (b c) -> (a b) c", c=8))
            r1_s = cpool.tile([H1, HID], f32, tag="r1")
            nc.sync.dma_start(
                out=r1_s[:],
                in_=wp_d[203:207, :].rearrange("a (b c) -> (a b) c", c=64))
            r2_s = cpool.tile([H2, OUT_DIM], f32, tag="r2")
            nc.sync.dma_start(
                out=r2_s[:],
                in_=wp_d[207:209, :].rearrange("a (b c) -> (a b) c", c=64))

            # gather idx tiles: replicate [16, C] across the 8 Q7 core groups
            fsidx_s = cpool.tile([128, NB * BLK_COLS], i16, tag="fsidx")
            fdidx_s = cpool.tile([128, NB * BLK_COLS], i16, tag="fdidx")
            for g in range(8):
                nc.sync.dma_start(out=fsidx_s[16 * g:16 * g + 16, :],
                                  in_=fsc_d[:, :])
                nc.sync.dma_start(out=fdidx_s[16 * g:16 * g + 16, :],
                                  in_=fdc_d[:, :])

            dli_s = cpool.tile([128, NB * T_BLK], i16, tag="dli")
            nc.sync.dma_start(out=dli_s[:], in_=dl_d[:, :])
            dloc_s = cpool.tile([128, NB * T_BLK], f32, tag="dloc")
            nc.vector.tensor_copy(out=dloc_s[:], in_=dli_s[:])

            iota_s = cpool.tile([128, BLK], f32, tag="iota")
            nc.gpsimd.iota(iota_s[:], [[1, BLK]], channel_multiplier=0,
                           allow_small_or_imprecise_dtypes=True)

            ones_s = cpool.tile([1, 128], f32, tag="ones")
            nc.vector.memset(ones_s[:], 1.0)

            def bcast128(row_ap, w, tag):
                ps = psB.tile([128, w], f32, tag="bc_ps")
                nc.tensor.matmul(out=ps[:], lhsT=ones_s[:], rhs=row_ap,
                                 start=True, stop=True)
                t = cpool.tile([128, w], f32, tag=tag)
                nc.vector.tensor_copy(out=t[:], in_=ps[:])
                return t

            b1f_s = bcast128(brow1_s[0:1, :], 128, "b1f")
            b2f_s = bcast128(brow2_s[0:1, :], 128, "b2f")
            a1_s = bcast128(arow_s[0:1, 0:64], HID, "a1f")
            a2_s = bcast128(arow_s[0:1, 64:128], OUT_DIM, "a2f")

            h1_s = npool.tile([HID, CORE_NODES], f32, tag="h1")
            h_s = npool.tile([HID, CORE_NODES], f32, tag="h")

            NT = CORE_NODES // 128  # node tiles per core for GEMMs

            def gemm_layer(get_lhs, K, w_s, bfull_s, loc_dram, full_dram):
                for t in range(NT):
                    ps = psG.tile([128, 128], f32)
                    nc.tensor.matmul(
                        out=ps[:], lhsT=get_lhs(t),
                        rhs=w_s[:K, :], start=True, stop=True)
                    gs = spool.tile([128, 128], f32, tag="gemm")
                    nc.vector.tensor_tensor(
                        out=gs[:], in0=ps[:], in1=bfull_s[:], op=Alu.add)
                    nc.sync.dma_start(
                        out=loc_dram[t * 128:(t + 1) * 128, :], in_=gs[:])
                nc.gpsimd.collective_compute(
                    "AllGather", Alu.bypass, replica_groups=RG,
                    ins=[loc_dram.ap().opt()], outs=[full_dram.ap().opt()])

            def gemm1_lhs(t):
                xg = spool.tile([IN_DIM, 128], f16, tag="xg")
                nc.sync.dma_start(out=xg[:], in_=xh_d[:, t * 128:(t + 1) * 128])
                return xg[:]

            def edge_layer(Hh, Ff, full_dram, loc_dram, attn_s, rX_s, bn_s,
                           hout_s, do_elu):
                HF = Hh * Ff
                VW = Hh + HF  # vals width per tile
                for b in range(NB):
                    ps_s = psA.tile([VW, BLK], f32, tag="scat")
                    for sec in range(2):
                        cA = b * BLK_COLS + sec * SEC_COLS
                        fs_e = gpool.tile([128, SEC_T * 64], f32, tag="fs")
                        fd_e = gpool.tile([128, SEC_T * 64], f32, tag="fd")
                        tab = full_dram[:, 0:64] if sec == 0 else \
                            full_dram[HALF:, 0:64]
                        nc.gpsimd.dma_gather(
                            out_ap=fs_e[:].rearrange("p (t e) -> p t e", e=64),
                            in_ap=tab,
                            idxs_ap=fsidx_s[:, cA:cA + SEC_COLS],
                            num_idxs=CAP, num_idxs_reg=CAP,
                            elem_size=64, elem_step=128, single_packet=False)
                        nc.gpsimd.dma_gather(
                            out_ap=fd_e[:].rearrange("p (t e) -> p t e", e=64),
                            in_ap=loc_dram[:, 64:128],
                            idxs_ap=fdidx_s[:, cA:cA + SEC_COLS],
                            num_idxs=CAP, num_idxs_reg=CAP,
                            elem_size=64, elem_step=128, single_packet=False)

                        # one-hot (bf16): O[p, t, n] = (dloc[p, t] == n)
                        dcol = b * T_BLK + sec * SEC_T
                        O_t = wpool.tile([128, SEC_T * BLK], bf16, tag="O")
                        nc.vector.tensor_tensor(
                            out=O_t[:].rearrange("p (t n) -> p t n", n=BLK),
                            in0=dloc_s[:, dcol:dcol + SEC_T, None]
                                .to_broadcast([128, SEC_T, BLK]),
                            in1=iota_s[:, None, :]
                                .to_broadcast([128, SEC_T, BLK]),
                            op=Alu.is_equal)

                        # z = fs + fd ; lrelu(z)*attn = (z + c|z|) * attn06
                        z_t = wpool.tile([128, SEC_T * 64], f32, tag="z")
                        wz_t = wpool.tile([128, SEC_T * 64], f32, tag="wz")
                        nc.vector.tensor_tensor(
                            out=z_t[:], in0=fs_e[:], in1=fd_e[:], op=Alu.add)
                        nc.scalar.activation(
                            out=wz_t[:], in_=z_t[:], func=Act.Abs,
                            scale=(1.0 - SLOPE) / (1.0 + SLOPE))
                        nc.vector.tensor_tensor(
                            out=wz_t[:], in0=z_t[:], in1=wz_t[:], op=Alu.add)
                        nc.vector.tensor_tensor(
                            out=z_t[:].rearrange("p (t e) -> p t e", e=64),
                            in0=wz_t[:].rearrange("p (t e) -> p t e", e=64),
                            in1=attn_s[:, None, :]
                                .to_broadcast([128, SEC_T, 64]),
                            op=Alu.mult)
                        # l = sum_f wz ; p = exp(l) into vals
                        l_t = spool.tile([128, SEC_T * Hh], f32, tag="l")
                        nc.vector.tensor_reduce(
                            out=l_t[:],
                            in_=z_t[:].rearrange("p (t h f) -> p t h f",
                                                 h=Hh, f=Ff),
                            axis=mybir.AxisListType.X, op=Alu.add)
                        # vals layout: [p*fs (HF cols) | p (Hh cols)] so that
                        # psum num rows start at partition 0, den at HF (=64)
                        vals = wpool.tile([128, SEC_T * VW], bf16, tag="vals")
                        vals3 = vals[:].rearrange("p (t v) -> p t v", v=VW)
                        nc.scalar.activation(
                            out=vals3[:, :, HF:VW],
                            in_=l_t[:].rearrange("p (t h) -> p t h", h=Hh),
                            func=Act.Exp)
                        nc.vector.tensor_tensor(
                            out=vals3[:, :, 0:HF].rearrange(
                                "p t (h f) -> p t h f", f=Ff),
                            in0=fs_e[:].rearrange("p (t h f) -> p t h f",
                                                  h=Hh, f=Ff),
                            in1=vals3[:, :, HF:VW][:, :, :, None]
                                .to_broadcast([128, SEC_T, Hh, Ff]),
                            op=Alu.mult)

                        # scatter: psum[v, n] += sum_e vals[e, v] * O[e, n]
                        for t in range(SEC_T):
                            nc.tensor.matmul(
                                out=ps_s[:],
                                lhsT=vals[:, t * VW:(t + 1) * VW],
                                rhs=O_t[:, t * BLK:(t + 1) * BLK],
                                start=(sec == 0 and t == 0),
                                stop=(sec == 1 and t == SEC_T - 1))

                    # normalize: out = num * (1/den) + bias
                    den = spool.tile([Hh, BLK], f32, tag="den")
                    nc.vector.tensor_scalar(
                        out=den[:], in0=ps_s[HF:VW, :], scalar1=DEN_EPS,
                        scalar2=None, op0=Alu.add)
                    rcp = spool.tile([Hh, BLK], f32, tag="rcp")
                    nc.vector.reciprocal(out=rcp[:], in_=den[:])
                    ps_r = psB.tile([HF, BLK], f32, tag="rrep")
                    nc.tensor.matmul(out=ps_r[:], lhsT=rX_s[:], rhs=rcp[:],
                                     start=True, stop=True)
                    rr = spool.tile([HF, BLK], f32, tag="rr")
                    nc.vector.tensor_copy(out=rr[:], in_=ps_r[:])
                    o1 = spool.tile([HF, BLK], f32, tag="o1")
                    nc.vector.tensor_tensor(
                        out=o1[:], in0=ps_s[0:HF, :], in1=rr[:], op=Alu.mult)
                    nsl = slice(b * BLK, (b + 1) * BLK)
                    if do_elu:
                        ob = spool.tile([HF, BLK], f32, tag="ob")
                        nc.vector.tensor_scalar(
                            out=ob[:], in0=o1[:], scalar1=bn_s[:, 0:1],
                            scalar2=None, op0=Alu.add)
                        m_t = spool.tile([HF, BLK], f32, tag="elum")
                        nc.vector.tensor_scalar(
                            out=m_t[:], in0=ob[:], scalar1=0.0,
                            scalar2=None, op0=Alu.min)
                        e_t = spool.tile([HF, BLK], f32, tag="elue")
                        nc.scalar.activation(out=e_t[:], in_=m_t[:],
                                             func=Act.Exp)
                        nc.vector.tensor_scalar(
                            out=m_t[:], in0=ob[:], scalar1=0.0,
                            scalar2=None, op0=Alu.max)
                        t_t = spool.tile([HF, BLK], f32, tag="elut")
                        nc.vector.tensor_tensor(
                            out=t_t[:], in0=e_t[:], in1=m_t[:], op=Alu.add)
                        nc.vector.tensor_scalar(
                            out=hout_s[:, nsl], in0=t_t[:], scalar1=-1.0,
                            scalar2=None, op0=Alu.add)
                    else:
                        nc.vector.tensor_scalar(
                            out=hout_s[:, nsl], in0=o1[:], scalar1=bn_s[:, 0:1],
                            scalar2=None, op0=Alu.add)

            def bn_norm(hin_s, bn_s, bnin_d, bnout_d, D):
                """BN stats (blockwise) + AllReduce; returns (scale, shift)."""
                s_cols = spool.tile([D, NB], f32, tag="bnscols")
                q_cols = spool.tile([D, NB], f32, tag="bnqcols")
                for b in range(NB):
                    nsl = slice(b * BLK, (b + 1) * BLK)
                    nc.vector.tensor_reduce(
                        out=s_cols[:, b:b + 1], in_=hin_s[:, nsl],
                        axis=mybir.AxisListType.X, op=Alu.add)
                    scr = spool.tile([D, BLK], f32, tag="bnscr")
                    nc.scalar.activation(
                        out=scr[:], in_=hin_s[:, nsl], func=Act.Square,
                        accum_out=q_cols[:, b:b + 1])
                st = spool.tile([D, 2], f32, tag="bnst")
                nc.vector.tensor_reduce(out=st[:, 0:1], in_=s_cols[:],
                                        axis=mybir.AxisListType.X, op=Alu.add)
                nc.vector.tensor_reduce(out=st[:, 1:2], in_=q_cols[:],
                                        axis=mybir.AxisListType.X, op=Alu.add)
                nc.sync.dma_start(out=bnin_d[:, :], in_=st[:])
                nc.gpsimd.collective_compute(
                    "AllReduce", Alu.add, replica_groups=RG,
                    ins=[bnin_d.ap().opt()], outs=[bnout_d.ap().opt()])
                g = spool.tile([D, 2], f32, tag="bng")
                nc.sync.dma_start(out=g[:], in_=bnout_d[:, :])
                # mu = (S - corr)/N ; var = (SQ - corrsq)/N - mu^2
                t_a = spool.tile([D, 1], f32, tag="bnta")
                nc.vector.tensor_tensor(out=t_a[:], in0=g[:, 0:1],
                                        in1=bn_s[:, 3:4], op=Alu.subtract)
                mu = spool.tile([D, 1], f32, tag="bnmu")
                nc.vector.tensor_scalar(out=mu[:], in0=t_a[:],
                                        scalar1=1.0 / N_REAL, scalar2=None,
                                        op0=Alu.mult)
                t_b = spool.tile([D, 1], f32, tag="bntb")
                nc.vector.tensor_tensor(out=t_b[:], in0=g[:, 1:2],
                                        in1=bn_s[:, 4:5], op=Alu.subtract)
                msq = spool.tile([D, 1], f32, tag="bnmsq")
                nc.vector.tensor_scalar(out=msq[:], in0=t_b[:],
                                        scalar1=1.0 / N_REAL, scalar2=None,
                                        op0=Alu.mult)
                mu2 = spool.tile([D, 1], f32, tag="bnmu2")
                nc.vector.tensor_tensor(out=mu2[:], in0=mu[:], in1=mu[:],
                                        op=Alu.mult)
                var = spool.tile([D, 1], f32, tag="bnvar")
                nc.vector.tensor_tensor(out=var[:], in0=msq[:], in1=mu2[:],
                                        op=Alu.subtract)
                sd = spool.tile([D, 1], f32, tag="bnsd")
                nc.scalar.activation(out=sd[:], in_=var[:], func=Act.Sqrt,
                                     bias=bn_s[:, 5:6])
                rs = spool.tile([D, 1], f32, tag="bnrs")
                nc.vector.reciprocal(out=rs[:], in_=sd[:])
                scl = spool.tile([D, 1], f32, tag="bnscl")
                nc.vector.tensor_tensor(out=scl[:], in0=bn_s[:, 1:2],
                                        in1=rs[:], op=Alu.mult)
                t_c = spool.tile([D, 1], f32, tag="bntc")
                nc.vector.tensor_tensor(out=t_c[:], in0=mu[:], in1=scl[:],
                                        op=Alu.mult)
                shf = spool.tile([D, 1], f32, tag="bnshf")
                nc.vector.tensor_tensor(out=shf[:], in0=bn_s[:, 2:3],
                                        in1=t_c[:], op=Alu.subtract)
                return scl, shf

            def norm_elu_blockwise(dst_s, src_s, scl, shf, D, do_elu):
                for b in range(NB):
                    nsl = slice(b * BLK, (b + 1) * BLK)
                    if not do_elu:
                        nc.vector.tensor_scalar(
                            out=dst_s[:, nsl], in0=src_s[:, nsl],
                            scalar1=scl[:], scalar2=shf[:],
                            op0=Alu.mult, op1=Alu.add)
                        continue
                    hb = spool.tile([D, BLK], f32, tag="nrmh")
                    nc.vector.tensor_scalar(
                        out=hb[:], in0=src_s[:, nsl], scalar1=scl[:],
                        scalar2=shf[:], op0=Alu.mult, op1=Alu.add)
                    m_t = spool.tile([D, BLK], f32, tag="nrmm")
                    nc.vector.tensor_scalar(out=m_t[:], in0=hb[:],
                                            scalar1=0.0, scalar2=None,
                                            op0=Alu.min)
                    e_t = spool.tile([D, BLK], f32, tag="nrme")
                    nc.scalar.activation(out=e_t[:], in_=m_t[:], func=Act.Exp)
                    nc.vector.tensor_scalar(out=m_t[:], in0=hb[:],
                                            scalar1=0.0, scalar2=None,
                                            op0=Alu.max)
                    t_t = spool.tile([D, BLK], f32, tag="nrmt")
                    nc.vector.tensor_tensor(out=t_t[:], in0=e_t[:],
                                            in1=m_t[:], op=Alu.add)
                    nc.vector.tensor_scalar(out=dst_s[:, nsl], in0=t_t[:],
                                            scalar1=-1.0, scalar2=None,
                                            op0=Alu.add)

            # ================= layer 1 =================
            gemm_layer(gemm1_lhs, IN_DIM, w1h_s, b1f_s, fsfd1_loc, fsfd1_full)
            edge_layer(H1, F1, fsfd1_full, fsfd1_loc, a1_s, r1_s, bn1_s,
                       h1_s, do_elu=True)
            scl1, shf1 = bn_norm(h1_s, bn1_s, bnin[0], bnout[0], HID)
            norm_elu_blockwise(h_s, h1_s, scl1, shf1, HID, do_elu=True)

            # ================= layer 2 =================
            gemm_layer(lambda t: h_s[:, t * 128:(t + 1) * 128], HID, w2_s,
                       b2f_s, fsfd2_loc, fsfd2_full)
            edge_layer(H2, F2, fsfd2_full, fsfd2_loc, a2_s, r2_s, bn2_s,
                       h1_s, do_elu=False)  # reuse h1_s as h2 buffer
            scl2, shf2 = bn_norm(h1_s, bn2_s, bnin[1], bnout[1], OUT_DIM)
            outb = npool.tile([OUT_DIM, CORE_NODES], f16, tag="outb")
            norm_elu_blockwise(outb, h1_s, scl2, shf2, OUT_DIM, do_elu=False)
            nc.sync.dma_start(out=out_d[:, :], in_=outb[:])

    return nc


# ---------------------------------------------------------------- pjrt runner
_CACHE = {}


def _get_runtime():
    if "rt" in _CACHE:
        return _CACHE["rt"]
    for p in ("/opt/trn_rl_repo",):
        if os.path.isdir(p) and p not in sys.path:
            sys.path.insert(0, p)
    import jax
    import jax.numpy as jnp
    from jax.sharding import Mesh, PartitionSpec
    from jax.experimental.shard_map import shard_map
    from concourse import mybir
    from concourse.bass2jax import (_bass_exec_p, partition_id_tensor,
                                    install_neuronx_cc_hook)

    install_neuronx_cc_hook()
    nc = build_program()
    nc.finalize()

    partition_name = (nc.partition_id_tensor.name
                      if nc.partition_id_tensor else None)
    dbg_name = nc.dbg_addr.name if nc.dbg_addr is not None else None
    in_names, out_names, out_info = [], [], []
    for alloc in nc.m.functions[0].allocations:
        if not isinstance(alloc, mybir.MemoryLocationSet):
            continue
        name = alloc.memorylocations[0].name
        if alloc.kind == "ExternalInput":
            if name != partition_name:
                in_names.append(name)
        elif alloc.kind == "ExternalOutput":
            out_names.append(name)
            out_info.append((tuple(alloc.tensor_shape),
                             mybir.dt.np(alloc.dtype)))
    in_names_all = tuple(in_names + out_names
                         + ([partition_name] if partition_name else []))
    out_avals = tuple(jax.core.ShapedArray(s, d) for s, d in out_info)

    assert dbg_name is None and in_names == OPERAND_NAMES, (dbg_name, in_names)
    n_params = len(in_names)

    def _body(*args):
        operands = list(args)
        if partition_name is not None:
            operands.append(partition_id_tensor())
        return tuple(_bass_exec_p.bind(
            *operands, out_avals=out_avals, in_names=in_names_all,
            out_names=tuple(out_names), lowering_input_output_aliases=(),
            sim_require_finite=True, sim_require_nnan=True, nc=nc))

    devices = jax.devices()[:NCORES]
    assert len(devices) == NCORES
    mesh = Mesh(np.asarray(devices), ("core",))
    from jax.sharding import NamedSharding
    spec = NamedSharding(mesh, PartitionSpec("core"))
    fn = jax.jit(shard_map(
        _body, mesh=mesh,
        in_specs=(PartitionSpec("core"),) * (n_params + len(out_names)),
        out_specs=(PartitionSpec("core"),) * len(out_names),
        check_rep=False),
        donate_argnums=tuple(range(n_params, n_params + len(out_names))))
    # ExternalOutput staging buffers, zero-filled on device (never uploaded).
    # Donated into the bass_exec results; the kernel writes every element.
    zfn = jax.jit(
        lambda: tuple(jnp.zeros((NCORES * s[0], *s[1:]), d)
                      for s, d in out_info),
        out_shardings=(spec,) * len(out_info))
    _CACHE["rt"] = (fn, zfn, mesh)
    return _CACHE["rt"]


def kernel(**inputs) -> np.ndarray:
    fn, zfn, _ = _get_runtime()
    ops = make_operands(inputs)
    (out,) = fn(*ops, *zfn())
    arr = np.asarray(out)  # [NCORES*OUT_DIM, CORE_NODES] f16
    res = arr.reshape(NCORES, OUT_DIM, CORE_NODES).transpose(0, 2, 1)
    return np.ascontiguousarray(
        res.reshape(NPAD, OUT_DIM)[:N_REAL]).astype(np.float32)


if __name__ == "__main__":
    import jax
    with jax.default_device(jax.devices("cpu")[0]):
        import reference
        inputs = {k: np.asarray(v) for k, v in reference.setup_inputs().items()}
        expected = np.asarray(reference.reference(**inputs))
    actual = kernel(**inputs)
    rel = np.linalg.norm(actual - expected) / np.linalg.norm(expected)
    print("Relative error:", rel)


# revision 10
# speedup vs baseline: 32.9466x; 1.0037x over previous
"""Trainium2 Bass kernel for a 2-layer GATv2 encoder (nn_GATv2Encoder).

Strategy (8 NeuronCores, SPMD):
- Host sorts edges by dst; dst nodes are partitioned into contiguous 256-node
  blocks, 25 blocks per core -> each core owns a disjoint contiguous node
  range, so segment softmax/scatter are core-local (no cross-core segment
  reductions needed).
- Each block's edges are split into two sections by src < HALF (int16 index
  range for dma_gather), each section padded to a fixed tile capacity.
- Per layer: node-parallel GEMMs produce a combined [fs|fd] table (512B rows);
  fs tables are AllGather'd; fs[src] and fd[dst] are fetched per edge with
  dma_gather (256B elements). Logits l = sum_f attn*leakyrelu(fs+fd) computed
  without segment-max (logits are tiny); softmax normalization is folded into
  a final num/den division. Scatter-add is a one-hot matmul accumulated in
  PSUM per block (bf16 one-hot + bf16 vals, fp32 accumulation).
- BatchNorm stats are computed per-core over the feature-partition layout and
  AllReduce'd; padded rows are corrected with host-precomputed constants.

Host<->device traffic is minimized (the axon tunnel is ~45 MB/s): x ships as
fp16, gather indices ship un-replicated (int16) and are replicated across the
8 gpsimd partition groups on device, all replicated constant tiles are built
on device from a small packed table, output returns as fp16, and the
ExternalOutput staging zeros are materialized on device instead of uploaded.
"""

import os
import sys
import numpy as np

# ---------------------------------------------------------------- constants
N_REAL = 50000
E_REAL = 800000
IN_DIM, HID, OUT_DIM = 128, 64, 64
H1, F1 = 8, 8
H2, F2 = 4, 16
SLOPE = 0.2
EPS = 1e-5
DEN_EPS = 1e-30

NCORES = 8
BLK = 256                      # nodes per block (one-hot / psum free width)
NB = 25                        # blocks per core
CORE_NODES = BLK * NB          # 6400
NPAD = NCORES * CORE_NODES     # 51200
HALF = NPAD // 2               # 25600 (int16-safe gather split)
NBLK_TOT = NCORES * NB         # 200
TILE = 128                     # edges per matmul tile
SEC_T = 18                     # tiles per section (A and B)
CAP = SEC_T * TILE             # 2304 edge slots per section
T_BLK = 2 * SEC_T              # 36 tiles per block
SLOTS = 2 * CAP                # 4608 edge slots per block
SEC_COLS = CAP // 16           # idx cols per section (144)
BLK_COLS = SLOTS // 16         # idx cols per block (288)
WROWS = 209                    # rows in the packed f32 const table


# ---------------------------------------------------------------- host prep
def host_prep(src, dst):
    """Vectorized edge bucketing. Returns global (concat-over-core) arrays:
    fsc/fdc [NCORES*16, NB*BLK_COLS] i16 (un-replicated dma_gather indices),
    dl [NCORES*128, NB*T_BLK] i16 (dst-local slot->node map, -1 = padding)."""
    src = np.asarray(src).astype(np.int32, copy=False)
    dst = np.asarray(dst).astype(np.int32, copy=False)
    # only the dst block (0..199, fits uint8) matters for bucketing — a
    # stable argsort on uint8 keys is a single radix pass
    blk_of = (dst >> 8).astype(np.uint8)
    order = np.argsort(blk_of, kind="stable")
    s_src, s_dst = src[order], dst[order]

    blk = (s_dst >> 8).astype(np.int32)             # block id per edge
    isB = s_src >= HALF
    blk_counts = np.bincount(blk, minlength=NBLK_TOT)
    blk_start = np.zeros(NBLK_TOT, np.int64)
    np.cumsum(blk_counts[:-1], out=blk_start[1:])
    start_e = blk_start[blk]
    nB_bef = np.cumsum(isB, dtype=np.int32) - isB    # B-count before edge e
    posB = nB_bef - nB_bef[start_e]
    pos_in_blk = (np.arange(len(s_src), dtype=np.int64) - start_e).astype(
        np.int32)
    posA = pos_in_blk - posB

    nB_per = np.bincount(blk[isB], minlength=NBLK_TOT)
    nA_per = blk_counts - nB_per
    assert nA_per.max() <= CAP and nB_per.max() <= CAP, (
        nA_per.max(), nB_per.max())

    slot = np.where(isB, CAP + posB, posA)
    gslot = blk * np.int32(SLOTS) + slot
    fsv = np.where(isB, s_src - HALF, s_src).astype(np.int16)
    fdv = (s_dst - (blk // NB) * CORE_NODES).astype(np.int16)
    dlv = (s_dst & 255).astype(np.int16)

    fs_flat = np.zeros(NBLK_TOT * SLOTS, np.int16)
    fd_flat = np.zeros(NBLK_TOT * SLOTS, np.int16)
    dl_flat = np.full(NBLK_TOT * SLOTS, -1, np.int16)
    fs_flat[gslot] = fsv
    fd_flat[gslot] = fdv
    dl_flat[gslot] = dlv

    # dma_gather layout: idx j of a section at (partition j%16, col j//16)
    def wrap16(flat):
        a = flat.reshape(NCORES, NB * 2, SEC_COLS, 16)
        return np.ascontiguousarray(a.transpose(0, 3, 1, 2)).reshape(
            NCORES * 16, NB * BLK_COLS)

    # one-hot layout: slot s of block b at (partition s%128, col b*T_BLK+s//128)
    d4 = dl_flat.reshape(NCORES, NB, T_BLK, 128)
    dl_g = np.ascontiguousarray(d4.transpose(0, 3, 1, 2)).reshape(
        NCORES * 128, NB * T_BLK)
    return wrap16(fs_flat), wrap16(fd_flat), dl_g


def _elu_np(x):
    return np.where(x > 0, x, np.exp(np.minimum(x, 0)) - 1).astype(np.float32)


def make_xh(inputs):
    """x.T as fp16, padded to NPAD nodes, core-major rows."""
    x = np.asarray(inputs["x"], np.float32)
    xh_g = np.zeros((NCORES * IN_DIM, CORE_NODES), np.float16)
    xv = x.T  # [128, 50000] view
    for c in range(NCORES):
        lo = c * CORE_NODES
        hi = min(lo + CORE_NODES, N_REAL)
        if hi > lo:
            xh_g[c * IN_DIM:(c + 1) * IN_DIM, :hi - lo] = xv[:, lo:hi]
    return xh_g


def make_rest(inputs):
    """The 4 non-x global operand arrays (axis 0 = concat over cores)."""
    f32 = np.float32
    fsc_g, fdc_g, dl_g = host_prep(inputs["src"], inputs["dst"])

    # packed f32 const table, replicated per core
    lr_a = (1.0 + SLOPE) / 2.0
    wp = np.zeros((WROWS, 128), f32)
    wp[0:128] = np.concatenate([np.asarray(inputs["Wsrc1"], f32),
                                np.asarray(inputs["Wdst1"], f32)], 1)
    wp[128:192] = np.concatenate([np.asarray(inputs["Wsrc2"], f32),
                                  np.asarray(inputs["Wdst2"], f32)], 1)
    wp[192] = np.concatenate([np.asarray(inputs["bsrc1"], f32),
                              np.asarray(inputs["bdst1"], f32)])
    wp[193] = np.concatenate([np.asarray(inputs["bsrc2"], f32),
                              np.asarray(inputs["bdst2"], f32)])
    wp[194, 0:64] = np.asarray(inputs["attn1"], f32).reshape(-1) * lr_a
    wp[194, 64:128] = np.asarray(inputs["attn2"], f32).reshape(-1) * lr_a

    npad_rows = NPAD - N_REAL
    bias1 = np.asarray(inputs["bias1"], f32)
    bias2 = np.asarray(inputs["bias2"], f32)
    cpad1 = _elu_np(bias1)
    bn1 = np.zeros((HID, 8), f32)
    bn1[:, 0] = bias1
    bn1[:, 1] = np.asarray(inputs["gamma1"], f32)
    bn1[:, 2] = np.asarray(inputs["beta1"], f32)
    bn1[:, 3] = npad_rows * cpad1
    bn1[:, 4] = npad_rows * cpad1 ** 2
    bn1[:, 5] = EPS
    bn2 = np.zeros((OUT_DIM, 8), f32)
    bn2[:, 0] = bias2
    bn2[:, 1] = np.asarray(inputs["gamma2"], f32)
    bn2[:, 2] = np.asarray(inputs["beta2"], f32)
    bn2[:, 3] = npad_rows * bias2
    bn2[:, 4] = npad_rows * bias2 ** 2
    bn2[:, 5] = EPS
    wp[195:199] = bn1.reshape(4, 128)
    wp[199:203] = bn2.reshape(4, 128)

    r1 = np.zeros((H1, HID), f32)
    for h in range(H1):
        r1[h, h * F1:(h + 1) * F1] = 1.0
    r2 = np.zeros((H2, OUT_DIM), f32)
    for h in range(H2):
        r2[h, h * F2:(h + 1) * F2] = 1.0
    wp[203:207] = r1.reshape(4, 128)
    wp[207:209] = r2.reshape(2, 128)
    wp_g = np.tile(wp, (NCORES, 1))

    return [fsc_g, fdc_g, dl_g, wp_g]


def make_operands(inputs):
    """All 5 global operand arrays, in program input order."""
    return [make_xh(inputs)] + make_rest(inputs)


OPERAND_NAMES = ["xh", "fsc", "fdc", "dl", "wp"]


# ---------------------------------------------------------------- bass program
def build_program():
    import concourse.bacc as bacc
    import concourse.tile as tile
    from concourse import mybir

    f32 = mybir.dt.float32
    f16 = mybir.dt.float16
    bf16 = mybir.dt.bfloat16
    i16 = mybir.dt.int16
    Alu = mybir.AluOpType
    Act = mybir.ActivationFunctionType

    nc = bacc.Bacc(None, target_bir_lowering=False, num_devices=NCORES)
    RG = [list(range(NCORES))]

    # ---- I/O ----
    xh_d = nc.dram_tensor("xh", [IN_DIM, CORE_NODES], f16, kind="ExternalInput")
    fsc_d = nc.dram_tensor("fsc", [16, NB * BLK_COLS], i16, kind="ExternalInput")
    fdc_d = nc.dram_tensor("fdc", [16, NB * BLK_COLS], i16, kind="ExternalInput")
    dl_d = nc.dram_tensor("dl", [128, NB * T_BLK], i16, kind="ExternalInput")
    wp_d = nc.dram_tensor("wp", [WROWS, 128], f32, kind="ExternalInput")
    out_d = nc.dram_tensor("outT", [OUT_DIM, CORE_NODES], f16,
                           kind="ExternalOutput")

    # ---- internal DRAM ----
    fsfd1_loc = nc.dram_tensor("fsfd1_loc", [CORE_NODES, 128], f32)
    fsfd1_full = nc.dram_tensor("fsfd1_full", [NPAD, 128], f32,
                                addr_space="Shared")
    fsfd2_loc = nc.dram_tensor("fsfd2_loc", [CORE_NODES, 128], f32)
    fsfd2_full = nc.dram_tensor("fsfd2_full", [NPAD, 128], f32,
                                addr_space="Shared")
    bnin = [nc.dram_tensor(f"bnin{i}", [64, 2], f32) for i in (1, 2)]
    bnout = [nc.dram_tensor(f"bnout{i}", [64, 2], f32, addr_space="Shared")
             for i in (1, 2)]

    with tile.TileContext(nc) as tc:
        with (
            tc.tile_pool(name="const", bufs=1) as cpool,
            tc.tile_pool(name="gath", bufs=2) as gpool,
            tc.tile_pool(name="work", bufs=2) as wpool,
            tc.tile_pool(name="small", bufs=2) as spool,
            tc.tile_pool(name="node", bufs=1) as npool,
            tc.tile_pool(name="psA", bufs=2, space="PSUM") as psA,
            tc.tile_pool(name="psB", bufs=2, space="PSUM") as psB,
            tc.tile_pool(name="psG", bufs=2, space="PSUM") as psG,
        ):
            # ---- load + derive constants ----
            w1_s = cpool.tile([IN_DIM, 128], f32, tag="w1")
            nc.sync.dma_start(out=w1_s[:], in_=wp_d[0:128, :])
            w1h_s = cpool.tile([IN_DIM, 128], f16, tag="w1h")
            nc.vector.tensor_copy(out=w1h_s[:], in_=w1_s[:])
            w2_s = cpool.tile([HID, 128], f32, tag="w2")
            nc.sync.dma_start(out=w2_s[:], in_=wp_d[128:192, :])
            brow1_s = cpool.tile([1, 128], f32, tag="brow1")
            nc.sync.dma_start(out=brow1_s[:], in_=wp_d[192:193, :])
            brow2_s = cpool.tile([1, 128], f32, tag="brow2")
            nc.sync.dma_start(out=brow2_s[:], in_=wp_d[193:194, :])
            arow_s = cpool.tile([1, 128], f32, tag="arow")
            nc.sync.dma_start(out=arow_s[:], in_=wp_d[194:195, :])
            bn1_s = cpool.tile([HID, 8], f32, tag="bn1")
            nc.sync.dma_start(
                out=bn1_s[:],
                in_=wp_d[195:199, :].rearrange("a (b c) -> (a b) c", c=8))
            bn2_s = cpool.tile([OUT_DIM, 8], f32, tag="bn2")
            nc.sync.dma_start(
                out=bn2_s[:],
                in_=wp_d[199:203, :].rearrange("a (b c) -> (a b) c", c=8))
            r1_s = cpool.tile([H1, HID], f32, tag="r1")
            nc.sync.dma_start(
                out=r1_s[:],
                in_=wp_d[203:207, :].rearrange("a (b c) -> (a b) c", c=64))
            r2_s = cpool.tile([H2, OUT_DIM], f32, tag="r2")
            nc.sync.dma_start(
                out=r2_s[:],
                in_=wp_d[207:209, :].rearrange("a (b c) -> (a b) c", c=64))

            # gather idx tiles: replicate [16, C] across the 8 Q7 core groups
            fsidx_s = cpool.tile([128, NB * BLK_COLS], i16, tag="fsidx")
            fdidx_s = cpool.tile([128, NB * BLK_COLS], i16, tag="fdidx")
            for g in range(8):
                nc.sync.dma_start(out=fsidx_s[16 * g:16 * g + 16, :],
                                  in_=fsc_d[:, :])
                nc.sync.dma_start(out=fdidx_s[16 * g:16 * g + 16, :],
                                  in_=fdc_d[:, :])

            dli_s = cpool.tile([128, NB * T_BLK], i16, tag="dli")
            nc.sync.dma_start(out=dli_s[:], in_=dl_d[:, :])
            dloc_s = cpool.tile([128, NB * T_BLK], f32, tag="dloc")
            nc.vector.tensor_copy(out=dloc_s[:], in_=dli_s[:])

            iota_s = cpool.tile([128, BLK], f32, tag="iota")
            nc.gpsimd.iota(iota_s[:], [[1, BLK]], channel_multiplier=0,
                           allow_small_or_imprecise_dtypes=True)

            ones_s = cpool.tile([1, 128], f32, tag="ones")
            nc.vector.memset(ones_s[:], 1.0)

            def bcast128(row_ap, w, tag):
                ps = psB.tile([128, w], f32, tag="bc_ps")
                nc.tensor.matmul(out=ps[:], lhsT=ones_s[:], rhs=row_ap,
                                 start=True, stop=True)
                t = cpool.tile([128, w], f32, tag=tag)
                nc.vector.tensor_copy(out=t[:], in_=ps[:])
                return t

            b1f_s = bcast128(brow1_s[0:1, :], 128, "b1f")
            b2f_s = bcast128(brow2_s[0:1, :], 128, "b2f")
            a1_s = bcast128(arow_s[0:1, 0:64], HID, "a1f")
            a2_s = bcast128(arow_s[0:1, 64:128], OUT_DIM, "a2f")

            h1_s = npool.tile([HID, CORE_NODES], f32, tag="h1")
            h_s = npool.tile([HID, CORE_NODES], f32, tag="h")

            NT = CORE_NODES // 128  # node tiles per core for GEMMs

            def gemm_layer(get_lhs, K, w_s, bfull_s, loc_dram, full_dram):
                for t in range(NT):
                    ps = psG.tile([128, 128], f32)
                    nc.tensor.matmul(
                        out=ps[:], lhsT=get_lhs(t),
                        rhs=w_s[:K, :], start=True, stop=True)
                    gs = spool.tile([128, 128], f32, tag="gemm")
                    nc.vector.tensor_tensor(
                        out=gs[:], in0=ps[:], in1=bfull_s[:], op=Alu.add)
                    nc.sync.dma_start(
                        out=loc_dram[t * 128:(t + 1) * 128, :], in_=gs[:])
                nc.gpsimd.collective_compute(
                    "AllGather", Alu.bypass, replica_groups=RG,
                    ins=[loc_dram.ap().opt()], outs=[full_dram.ap().opt()])

            def gemm1_lhs(t):
                xg = spool.tile([IN_DIM, 128], f16, tag="xg")
                nc.sync.dma_start(out=xg[:], in_=xh_d[:, t * 128:(t + 1) * 128])
                return xg[:]

            def edge_layer(Hh, Ff, full_dram, loc_dram, attn_s, rX_s, bn_s,
                           hout_s, do_elu):
                HF = Hh * Ff
                VW = Hh + HF  # vals width per tile
                for b in range(NB):
                    ps_s = psA.tile([VW, BLK], f32, tag="scat")
                    for sec in range(2):
                        cA = b * BLK_COLS + sec * SEC_COLS
                        fs_e = gpool.tile([128, SEC_T * 64], f32, tag="fs")
                        fd_e = gpool.tile([128, SEC_T * 64], f32, tag="fd")
                        tab = full_dram[:, 0:64] if sec == 0 else \
                            full_dram[HALF:, 0:64]
                        nc.gpsimd.dma_gather(
                            out_ap=fs_e[:].rearrange("p (t e) -> p t e", e=64),
                            in_ap=tab,
                            idxs_ap=fsidx_s[:, cA:cA + SEC_COLS],
                            num_idxs=CAP, num_idxs_reg=CAP,
                            elem_size=64, elem_step=128, single_packet=False)
                        nc.gpsimd.dma_gather(
                            out_ap=fd_e[:].rearrange("p (t e) -> p t e", e=64),
                            in_ap=loc_dram[:, 64:128],
                            idxs_ap=fdidx_s[:, cA:cA + SEC_COLS],
                            num_idxs=CAP, num_idxs_reg=CAP,
                            elem_size=64, elem_step=128, single_packet=False)

                        # one-hot (bf16): O[p, t, n] = (dloc[p, t] == n)
                        dcol = b * T_BLK + sec * SEC_T
                        O_t = wpool.tile([128, SEC_T * BLK], bf16, tag="O")
                        nc.vector.tensor_tensor(
                            out=O_t[:].rearrange("p (t n) -> p t n", n=BLK),
                            in0=dloc_s[:, dcol:dcol + SEC_T, None]
                                .to_broadcast([128, SEC_T, BLK]),
                            in1=iota_s[:, None, :]
                                .to_broadcast([128, SEC_T, BLK]),
                            op=Alu.is_equal)

                        # z = fs + fd ; lrelu(z)*attn = (z + c|z|) * attn06
                        z_t = wpool.tile([128, SEC_T * 64], f32, tag="z")
                        wz_t = wpool.tile([128, SEC_T * 64], f32, tag="wz")
                        nc.vector.tensor_tensor(
                            out=z_t[:], in0=fs_e[:], in1=fd_e[:], op=Alu.add)
                        nc.scalar.activation(
                            out=wz_t[:], in_=z_t[:], func=Act.Abs,
                            scale=(1.0 - SLOPE) / (1.0 + SLOPE))
                        nc.vector.tensor_tensor(
                            out=wz_t[:], in0=z_t[:], in1=wz_t[:], op=Alu.add)
                        nc.vector.tensor_tensor(
                            out=z_t[:].rearrange("p (t e) -> p t e", e=64),
                            in0=wz_t[:].rearrange("p (t e) -> p t e", e=64),
                            in1=attn_s[:, None, :]
                                .to_broadcast([128, SEC_T, 64]),
                            op=Alu.mult)
                        # l = sum_f wz ; p = exp(l) into vals
                        l_t = spool.tile([128, SEC_T * Hh], f32, tag="l")
                        nc.vector.tensor_reduce(
                            out=l_t[:],
                            in_=z_t[:].rearrange("p (t h f) -> p t h f",
                                                 h=Hh, f=Ff),
                            axis=mybir.AxisListType.X, op=Alu.add)
                        # vals layout: [p*fs (HF cols) | p (Hh cols)] so that
                        # psum num rows start at partition 0, den at HF (=64)
                        vals = wpool.tile([128, SEC_T * VW], bf16, tag="vals")
                        vals3 = vals[:].rearrange("p (t v) -> p t v", v=VW)
                        nc.scalar.activation(
                            out=vals3[:, :, HF:VW],
                            in_=l_t[:].rearrange("p (t h) -> p t h", h=Hh),
                            func=Act.Exp)
                        nc.vector.tensor_tensor(
                            out=vals3[:, :, 0:HF].rearrange(
                                "p t (h f) -> p t h f", f=Ff),
                            in0=fs_e[:].rearrange("p (t h f) -> p t h f",
                                                  h=Hh, f=Ff),
                            in1=vals3[:, :, HF:VW][:, :, :, None]
                                .to_broadcast([128, SEC_T, Hh, Ff]),
                            op=Alu.mult)

                        # scatter: psum[v, n] += sum_e vals[e, v] * O[e, n]
                        for t in range(SEC_T):
                            nc.tensor.matmul(
                                out=ps_s[:],
                                lhsT=vals[:, t * VW:(t + 1) * VW],
                                rhs=O_t[:, t * BLK:(t + 1) * BLK],
                                start=(sec == 0 and t == 0),
                                stop=(sec == 1 and t == SEC_T - 1))

                    # normalize: out = num * (1/den) + bias
                    den = spool.tile([Hh, BLK], f32, tag="den")
                    nc.vector.tensor_scalar(
                        out=den[:], in0=ps_s[HF:VW, :], scalar1=DEN_EPS,
                        scalar2=None, op0=Alu.add)
                    rcp = spool.tile([Hh, BLK], f32, tag="rcp")
                    nc.vector.reciprocal(out=rcp[:], in_=den[:])
                    ps_r = psB.tile([HF, BLK], f32, tag="rrep")
                    nc.tensor.matmul(out=ps_r[:], lhsT=rX_s[:], rhs=rcp[:],
                                     start=True, stop=True)
                    rr = spool.tile([HF, BLK], f32, tag="rr")
                    nc.vector.tensor_copy(out=rr[:], in_=ps_r[:])
                    o1 = spool.tile([HF, BLK], f32, tag="o1")
                    nc.vector.tensor_tensor(
                        out=o1[:], in0=ps_s[0:HF, :], in1=rr[:], op=Alu.mult)
                    nsl = slice(b * BLK, (b + 1) * BLK)
                    if do_elu:
                        ob = spool.tile([HF, BLK], f32, tag="ob")
                        nc.vector.tensor_scalar(
                            out=ob[:], in0=o1[:], scalar1=bn_s[:, 0:1],
                            scalar2=None, op0=Alu.add)
                        m_t = spool.tile([HF, BLK], f32, tag="elum")
                        nc.vector.tensor_scalar(
                            out=m_t[:], in0=ob[:], scalar1=0.0,
                            scalar2=None, op0=Alu.min)
                        e_t = spool.tile([HF, BLK], f32, tag="elue")
                        nc.scalar.activation(out=e_t[:], in_=m_t[:],
                                             func=Act.Exp)
                        nc.vector.tensor_scalar(
                            out=m_t[:], in0=ob[:], scalar1=0.0,
                            scalar2=None, op0=Alu.max)
                        t_t = spool.tile([HF, BLK], f32, tag="elut")
                        nc.vector.tensor_tensor(
                            out=t_t[:], in0=e_t[:], in1=m_t[:], op=Alu.add)
                        nc.vector.tensor_scalar(
                            out=hout_s[:, nsl], in0=t_t[:], scalar1=-1.0,
                            scalar2=None, op0=Alu.add)
                    else:
                        nc.vector.tensor_scalar(
                            out=hout_s[:, nsl], in0=o1[:], scalar1=bn_s[:, 0:1],
                            scalar2=None, op0=Alu.add)

            def bn_norm(hin_s, bn_s, bnin_d, bnout_d, D):
                """BN stats (blockwise) + AllReduce; returns (scale, shift)."""
                s_cols = spool.tile([D, NB], f32, tag="bnscols")
                q_cols = spool.tile([D, NB], f32, tag="bnqcols")
                for b in range(NB):
                    nsl = slice(b * BLK, (b + 1) * BLK)
                    nc.vector.tensor_reduce(
                        out=s_cols[:, b:b + 1], in_=hin_s[:, nsl],
                        axis=mybir.AxisListType.X, op=Alu.add)
                    scr = spool.tile([D, BLK], f32, tag="bnscr")
                    nc.scalar.activation(
                        out=scr[:], in_=hin_s[:, nsl], func=Act.Square,
                        accum_out=q_cols[:, b:b + 1])
                st = spool.tile([D, 2], f32, tag="bnst")
                nc.vector.tensor_reduce(out=st[:, 0:1], in_=s_cols[:],
                                        axis=mybir.AxisListType.X, op=Alu.add)
                nc.vector.tensor_reduce(out=st[:, 1:2], in_=q_cols[:],
                                        axis=mybir.AxisListType.X, op=Alu.add)
                nc.sync.dma_start(out=bnin_d[:, :], in_=st[:])
                nc.gpsimd.collective_compute(
                    "AllReduce", Alu.add, replica_groups=RG,
                    ins=[bnin_d.ap().opt()], outs=[bnout_d.ap().opt()])
                g = spool.tile([D, 2], f32, tag="bng")
                nc.sync.dma_start(out=g[:], in_=bnout_d[:, :])
                # mu = (S - corr)/N ; var = (SQ - corrsq)/N - mu^2
                t_a = spool.tile([D, 1], f32, tag="bnta")
                nc.vector.tensor_tensor(out=t_a[:], in0=g[:, 0:1],
                                        in1=bn_s[:, 3:4], op=Alu.subtract)
                mu = spool.tile([D, 1], f32, tag="bnmu")
                nc.vector.tensor_scalar(out=mu[:], in0=t_a[:],
                                        scalar1=1.0 / N_REAL, scalar2=None,
                                        op0=Alu.mult)
                t_b = spool.tile([D, 1], f32, tag="bntb")
                nc.vector.tensor_tensor(out=t_b[:], in0=g[:, 1:2],
                                        in1=bn_s[:, 4:5], op=Alu.subtract)
                msq = spool.tile([D, 1], f32, tag="bnmsq")
                nc.vector.tensor_scalar(out=msq[:], in0=t_b[:],
                                        scalar1=1.0 / N_REAL, scalar2=None,
                                        op0=Alu.mult)
                mu2 = spool.tile([D, 1], f32, tag="bnmu2")
                nc.vector.tensor_tensor(out=mu2[:], in0=mu[:], in1=mu[:],
                                        op=Alu.mult)
                var = spool.tile([D, 1], f32, tag="bnvar")
                nc.vector.tensor_tensor(out=var[:], in0=msq[:], in1=mu2[:],
                                        op=Alu.subtract)
                sd = spool.tile([D, 1], f32, tag="bnsd")
                nc.scalar.activation(out=sd[:], in_=var[:], func=Act.Sqrt,
                                     bias=bn_s[:, 5:6])
                rs = spool.tile([D, 1], f32, tag="bnrs")
                nc.vector.reciprocal(out=rs[:], in_=sd[:])
                scl = spool.tile([D, 1], f32, tag="bnscl")
                nc.vector.tensor_tensor(out=scl[:], in0=bn_s[:, 1:2],
                                        in1=rs[:], op=Alu.mult)
                t_c = spool.tile([D, 1], f32, tag="bntc")
                nc.vector.tensor_tensor(out=t_c[:], in0=mu[:], in1=scl[:],
                                        op=Alu.mult)
                shf = spool.tile([D, 1], f32, tag="bnshf")
                nc.vector.tensor_tensor(out=shf[:], in0=bn_s[:, 2:3],
                                        in1=t_c[:], op=Alu.subtract)
                return scl, shf

            def norm_elu_blockwise(dst_s, src_s, scl, shf, D, do_elu):
                for b in range(NB):
                    nsl = slice(b * BLK, (b + 1) * BLK)
                    if not do_elu:
                        nc.vector.tensor_scalar(
                            out=dst_s[:, nsl], in0=src_s[:, nsl],
                            scalar1=scl[:], scalar2=shf[:],
                            op0=Alu.mult, op1=Alu.add)
                        continue
                    hb = spool.tile([D, BLK], f32, tag="nrmh")
                    nc.vector.tensor_scalar(
                        out=hb[:], in0=src_s[:, nsl], scalar1=scl[:],
                        scalar2=shf[:], op0=Alu.mult, op1=Alu.add)
                    m_t = spool.tile([D, BLK], f32, tag="nrmm")
                    nc.vector.tensor_scalar(out=m_t[:], in0=hb[:],
                                            scalar1=0.0, scalar2=None,
                                            op0=Alu.min)
                    e_t = spool.tile([D, BLK], f32, tag="nrme")
                    nc.scalar.activation(out=e_t[:], in_=m_t[:], func=Act.Exp)
                    nc.vector.tensor_scalar(out=m_t[:], in0=hb[:],
                                            scalar1=0.0, scalar2=None,
                                            op0=Alu.max)
                    t_t = spool.tile([D, BLK], f32, tag="nrmt")
                    nc.vector.tensor_tensor(out=t_t[:], in0=e_t[:],
                                            in1=m_t[:], op=Alu.add)
                    nc.vector.tensor_scalar(out=dst_s[:, nsl], in0=t_t[:],
                                            scalar1=-1.0, scalar2=None,
                                            op0=Alu.add)

            # ================= layer 1 =================
            gemm_layer(gemm1_lhs, IN_DIM, w1h_s, b1f_s, fsfd1_loc, fsfd1_full)
            edge_layer(H1, F1, fsfd1_full, fsfd1_loc, a1_s, r1_s, bn1_s,
                       h1_s, do_elu=True)
            scl1, shf1 = bn_norm(h1_s, bn1_s, bnin[0], bnout[0], HID)
            norm_elu_blockwise(h_s, h1_s, scl1, shf1, HID, do_elu=True)

            # ================= layer 2 =================
            gemm_layer(lambda t: h_s[:, t * 128:(t + 1) * 128], HID, w2_s,
                       b2f_s, fsfd2_loc, fsfd2_full)
            edge_layer(H2, F2, fsfd2_full, fsfd2_loc, a2_s, r2_s, bn2_s,
                       h1_s, do_elu=False)  # reuse h1_s as h2 buffer
            scl2, shf2 = bn_norm(h1_s, bn2_s, bnin[1], bnout[1], OUT_DIM)
            outb = npool.tile([OUT_DIM, CORE_NODES], f16, tag="outb")
            norm_elu_blockwise(outb, h1_s, scl2, shf2, OUT_DIM, do_elu=False)
            nc.sync.dma_start(out=out_d[:, :], in_=outb[:])

    return nc


# ---------------------------------------------------------------- pjrt runner
_CACHE = {}


def _get_runtime():
    if "rt" in _CACHE:
        return _CACHE["rt"]
    for p in ("/opt/trn_rl_repo",):
        if os.path.isdir(p) and p not in sys.path:
            sys.path.insert(0, p)
    import jax
    import jax.numpy as jnp
    from jax.sharding import Mesh, PartitionSpec
    from jax.experimental.shard_map import shard_map
    from concourse import mybir
    from concourse.bass2jax import (_bass_exec_p, partition_id_tensor,
                                    install_neuronx_cc_hook)

    install_neuronx_cc_hook()
    nc = build_program()
    nc.finalize()

    partition_name = (nc.partition_id_tensor.name
                      if nc.partition_id_tensor else None)
    dbg_name = nc.dbg_addr.name if nc.dbg_addr is not None else None
    in_names, out_names, out_info = [], [], []
    for alloc in nc.m.functions[0].allocations:
        if not isinstance(alloc, mybir.MemoryLocationSet):
            continue
        name = alloc.memorylocations[0].name
        if alloc.kind == "ExternalInput":
            if name != partition_name:
                in_names.append(name)
        elif alloc.kind == "ExternalOutput":
            out_names.append(name)
            out_info.append((tuple(alloc.tensor_shape),
                             mybir.dt.np(alloc.dtype)))
    in_names_all = tuple(in_names + out_names
                         + ([partition_name] if partition_name else []))
    out_avals = tuple(jax.core.ShapedArray(s, d) for s, d in out_info)

    assert dbg_name is None and in_names == OPERAND_NAMES, (dbg_name, in_names)
    n_params = len(in_names)

    def _body(*args):
        operands = list(args)
        if partition_name is not None:
            operands.append(partition_id_tensor())
        return tuple(_bass_exec_p.bind(
            *operands, out_avals=out_avals, in_names=in_names_all,
            out_names=tuple(out_names), lowering_input_output_aliases=(),
            sim_require_finite=True, sim_require_nnan=True, nc=nc))

    devices = jax.devices()[:NCORES]
    assert len(devices) == NCORES
    mesh = Mesh(np.asarray(devices), ("core",))
    from jax.sharding import NamedSharding
    spec = NamedSharding(mesh, PartitionSpec("core"))
    fn = jax.jit(shard_map(
        _body, mesh=mesh,
        in_specs=(PartitionSpec("core"),) * (n_params + len(out_names)),
        out_specs=(PartitionSpec("core"),) * len(out_names),
        check_rep=False),
        donate_argnums=tuple(range(n_params, n_params + len(out_names))))
    # ExternalOutput staging buffers, zero-filled on device (never uploaded).
    # Donated into the bass_exec results; the kernel writes every element.
    zfn = jax.jit(
        lambda: tuple(jnp.zeros((NCORES * s[0], *s[1:]), d)
                      for s, d in out_info),
        out_shardings=(spec,) * len(out_info))
    _CACHE["rt"] = (fn, zfn, mesh)
    return _CACHE["rt"]


def kernel(**inputs) -> np.ndarray:
    import threading
    import jax
    from jax.sharding import NamedSharding, PartitionSpec

    fn, zfn, mesh = _get_runtime()
    spec = NamedSharding(mesh, PartitionSpec("core"))
    # overlap the (largest) x upload with the remaining host-side prep
    xh_g = make_xh(inputs)
    holder = {}

    def _put():
        holder["xh"] = jax.device_put(xh_g, spec)

    th = threading.Thread(target=_put)
    th.start()
    rest = make_rest(inputs)
    th.join()
    (out,) = fn(holder["xh"], *rest, *zfn())
    arr = np.asarray(out)  # [NCORES*OUT_DIM, CORE_NODES] f16
    res = arr.reshape(NCORES, OUT_DIM, CORE_NODES).transpose(0, 2, 1)
    return np.ascontiguousarray(
        res.reshape(NPAD, OUT_DIM)[:N_REAL]).astype(np.float32)


if __name__ == "__main__":
    import jax
    with jax.default_device(jax.devices("cpu")[0]):
        import reference
        inputs = {k: np.asarray(v) for k, v in reference.setup_inputs().items()}
        expected = np.asarray(reference.reference(**inputs))
    actual = kernel(**inputs)
    rel = np.linalg.norm(actual - expected) / np.linalg.norm(expected)
    print("Relative error:", rel)


# revision 23
# speedup vs baseline: 33.5287x; 1.0177x over previous
"""Trainium2 Bass kernel for a 2-layer GATv2 encoder (nn_GATv2Encoder).

Strategy (8 NeuronCores, SPMD):
- Host sorts edges by dst; dst nodes are partitioned into contiguous 256-node
  blocks, 25 blocks per core -> each core owns a disjoint contiguous node
  range, so segment softmax/scatter are core-local (no cross-core segment
  reductions needed).
- Each block's edges are split into two sections by src < HALF (int16 index
  range for dma_gather), each section padded to a fixed tile capacity.
- Per layer: node-parallel GEMMs produce a combined [fs|fd] table (512B rows);
  fs tables are AllGather'd; fs[src] and fd[dst] are fetched per edge with
  dma_gather (256B elements). Logits l = sum_f attn*leakyrelu(fs+fd) computed
  without segment-max (logits are tiny); softmax normalization is folded into
  a final num/den division. Scatter-add is a one-hot matmul accumulated in
  PSUM per block (bf16 one-hot + bf16 vals, fp32 accumulation).
- BatchNorm stats are computed per-core over the feature-partition layout and
  AllReduce'd; padded rows are corrected with host-precomputed constants.

Host<->device traffic is minimized (the axon tunnel is ~45 MB/s): x ships as
fp16, gather indices ship un-replicated (int16) and are replicated across the
8 gpsimd partition groups on device, all replicated constant tiles are built
on device from a small packed table, output returns as fp16, and the
ExternalOutput staging zeros are materialized on device instead of uploaded.
"""

import os
import sys
import numpy as np

# ---------------------------------------------------------------- constants
N_REAL = 50000
E_REAL = 800000
IN_DIM, HID, OUT_DIM = 128, 64, 64
H1, F1 = 8, 8
H2, F2 = 4, 16
SLOPE = 0.2
EPS = 1e-5
DEN_EPS = 1e-30

NCORES = 8
BLK = 256                      # nodes per block (one-hot / psum free width)
NB = 25                        # blocks per core
CORE_NODES = BLK * NB          # 6400
NPAD = NCORES * CORE_NODES     # 51200
HALF = NPAD // 2               # 25600 (int16-safe gather split)
NBLK_TOT = NCORES * NB         # 200
TILE = 128                     # edges per matmul tile
SEC_T = 18                     # tiles per section (A and B)
CAP = SEC_T * TILE             # 2304 edge slots per section
T_BLK = 2 * SEC_T              # 36 tiles per block
SLOTS = 2 * CAP                # 4608 edge slots per block
SEC_COLS = CAP // 16           # idx cols per section (144)
BLK_COLS = SLOTS // 16         # idx cols per block (288)
WROWS = 209                    # rows in the packed f32 const table


# ---------------------------------------------------------------- host prep
def host_prep(src, dst):
    """Vectorized edge bucketing. Returns global (concat-over-core) arrays:
    fsc/fdc [NCORES*16, NB*BLK_COLS] i16 (un-replicated dma_gather indices),
    dl [NCORES*128, NB*T_BLK] i16 (dst-local slot->node map, -1 = padding)."""
    src = np.asarray(src).astype(np.int32, copy=False)
    dst = np.asarray(dst).astype(np.int32, copy=False)
    # only the dst block (0..199, fits uint8) matters for bucketing — a
    # stable argsort on uint8 keys is a single radix pass
    blk_of = (dst >> 8).astype(np.uint8)
    order = np.argsort(blk_of, kind="stable")
    s_src, s_dst = src[order], dst[order]

    blk = (s_dst >> 8).astype(np.int32)             # block id per edge
    isB = s_src >= HALF
    blk_counts = np.bincount(blk, minlength=NBLK_TOT)
    blk_start = np.zeros(NBLK_TOT, np.int64)
    np.cumsum(blk_counts[:-1], out=blk_start[1:])
    start_e = blk_start[blk]
    nB_bef = np.cumsum(isB, dtype=np.int32) - isB    # B-count before edge e
    posB = nB_bef - nB_bef[start_e]
    pos_in_blk = (np.arange(len(s_src), dtype=np.int64) - start_e).astype(
        np.int32)
    posA = pos_in_blk - posB

    nB_per = np.bincount(blk[isB], minlength=NBLK_TOT)
    nA_per = blk_counts - nB_per
    assert nA_per.max() <= CAP and nB_per.max() <= CAP, (
        nA_per.max(), nB_per.max())

    slot = np.where(isB, CAP + posB, posA)
    gslot = blk * np.int32(SLOTS) + slot
    fsv = np.where(isB, s_src - HALF, s_src).astype(np.int16)
    fdv = (s_dst - (blk // NB) * CORE_NODES).astype(np.int16)
    dlv = (s_dst & 255).astype(np.int16)

    fs_flat = np.zeros(NBLK_TOT * SLOTS, np.int16)
    fd_flat = np.zeros(NBLK_TOT * SLOTS, np.int16)
    dl_flat = np.full(NBLK_TOT * SLOTS, -1, np.int16)
    fs_flat[gslot] = fsv
    fd_flat[gslot] = fdv
    dl_flat[gslot] = dlv

    # dma_gather layout: idx j of a section at (partition j%16, col j//16)
    def wrap16(flat):
        a = flat.reshape(NCORES, NB * 2, SEC_COLS, 16)
        return np.ascontiguousarray(a.transpose(0, 3, 1, 2)).reshape(
            NCORES * 16, NB * BLK_COLS)

    # one-hot layout: slot s of block b at (partition s%128, col b*T_BLK+s//128)
    d4 = dl_flat.reshape(NCORES, NB, T_BLK, 128)
    dl_g = np.ascontiguousarray(d4.transpose(0, 3, 1, 2)).reshape(
        NCORES * 128, NB * T_BLK)
    return wrap16(fs_flat), wrap16(fd_flat), dl_g


def _elu_np(x):
    return np.where(x > 0, x, np.exp(np.minimum(x, 0)) - 1).astype(np.float32)


def make_xh(inputs):
    """x.T as fp16, padded to NPAD nodes, core-major rows."""
    x = np.asarray(inputs["x"], np.float32)
    xh_g = np.zeros((NCORES * IN_DIM, CORE_NODES), np.float16)
    xv = x.T  # [128, 50000] view
    for c in range(NCORES):
        lo = c * CORE_NODES
        hi = min(lo + CORE_NODES, N_REAL)
        if hi > lo:
            xh_g[c * IN_DIM:(c + 1) * IN_DIM, :hi - lo] = xv[:, lo:hi]
    return xh_g


def make_rest(inputs):
    """The 4 non-x global operand arrays (axis 0 = concat over cores)."""
    f32 = np.float32
    fsc_g, fdc_g, dl_g = host_prep(inputs["src"], inputs["dst"])

    # packed f32 const table, replicated per core
    lr_a = (1.0 + SLOPE) / 2.0
    wp = np.zeros((WROWS, 128), f32)
    wp[0:128] = np.concatenate([np.asarray(inputs["Wsrc1"], f32),
                                np.asarray(inputs["Wdst1"], f32)], 1)
    wp[128:192] = np.concatenate([np.asarray(inputs["Wsrc2"], f32),
                                  np.asarray(inputs["Wdst2"], f32)], 1)
    wp[192] = np.concatenate([np.asarray(inputs["bsrc1"], f32),
                              np.asarray(inputs["bdst1"], f32)])
    wp[193] = np.concatenate([np.asarray(inputs["bsrc2"], f32),
                              np.asarray(inputs["bdst2"], f32)])
    wp[194, 0:64] = np.asarray(inputs["attn1"], f32).reshape(-1) * lr_a
    wp[194, 64:128] = np.asarray(inputs["attn2"], f32).reshape(-1) * lr_a

    npad_rows = NPAD - N_REAL
    bias1 = np.asarray(inputs["bias1"], f32)
    bias2 = np.asarray(inputs["bias2"], f32)
    cpad1 = _elu_np(bias1)
    bn1 = np.zeros((HID, 8), f32)
    bn1[:, 0] = bias1
    bn1[:, 1] = np.asarray(inputs["gamma1"], f32)
    bn1[:, 2] = np.asarray(inputs["beta1"], f32)
    bn1[:, 3] = npad_rows * cpad1
    bn1[:, 4] = npad_rows * cpad1 ** 2
    bn1[:, 5] = EPS
    bn2 = np.zeros((OUT_DIM, 8), f32)
    bn2[:, 0] = bias2
    bn2[:, 1] = np.asarray(inputs["gamma2"], f32)
    bn2[:, 2] = np.asarray(inputs["beta2"], f32)
    bn2[:, 3] = npad_rows * bias2
    bn2[:, 4] = npad_rows * bias2 ** 2
    bn2[:, 5] = EPS
    wp[195:199] = bn1.reshape(4, 128)
    wp[199:203] = bn2.reshape(4, 128)

    r1 = np.zeros((H1, HID), f32)
    for h in range(H1):
        r1[h, h * F1:(h + 1) * F1] = 1.0
    r2 = np.zeros((H2, OUT_DIM), f32)
    for h in range(H2):
        r2[h, h * F2:(h + 1) * F2] = 1.0
    wp[203:207] = r1.reshape(4, 128)
    wp[207:209] = r2.reshape(2, 128)
    wp_g = np.tile(wp, (NCORES, 1))

    return [fsc_g, fdc_g, dl_g, wp_g]


def make_operands(inputs):
    """All 5 global operand arrays, in program input order."""
    return [make_xh(inputs)] + make_rest(inputs)


OPERAND_NAMES = ["xh", "fsc", "fdc", "dl", "wp"]


# ---------------------------------------------------------------- bass program
def build_program():
    import concourse.bacc as bacc
    import concourse.tile as tile
    from concourse import mybir

    f32 = mybir.dt.float32
    f16 = mybir.dt.float16
    bf16 = mybir.dt.bfloat16
    i16 = mybir.dt.int16
    Alu = mybir.AluOpType
    Act = mybir.ActivationFunctionType

    nc = bacc.Bacc(None, target_bir_lowering=False, num_devices=NCORES)
    RG = [list(range(NCORES))]

    # ---- I/O ----
    xh_d = nc.dram_tensor("xh", [IN_DIM, CORE_NODES], f16, kind="ExternalInput")
    fsc_d = nc.dram_tensor("fsc", [16, NB * BLK_COLS], i16, kind="ExternalInput")
    fdc_d = nc.dram_tensor("fdc", [16, NB * BLK_COLS], i16, kind="ExternalInput")
    dl_d = nc.dram_tensor("dl", [128, NB * T_BLK], i16, kind="ExternalInput")
    wp_d = nc.dram_tensor("wp", [WROWS, 128], f32, kind="ExternalInput")
    out_d = nc.dram_tensor("outT", [OUT_DIM, CORE_NODES], f16,
                           kind="ExternalOutput")

    # ---- internal DRAM ----
    # [fs|fd] per-node table in bf16: 256B rows satisfy dma_gather's elem
    # granularity, the AllGather moves half the f32 bytes, and bf16 gather
    # outputs unlock the DVE 2x 16-bit path for the edge element-wise chain
    fsfd1_loc = nc.dram_tensor("fsfd1_loc", [CORE_NODES, 128], bf16)
    fsfd1_full = nc.dram_tensor("fsfd1_full", [NPAD, 128], bf16,
                                addr_space="Shared")
    fsfd2_loc = nc.dram_tensor("fsfd2_loc", [CORE_NODES, 128], bf16)
    fsfd2_full = nc.dram_tensor("fsfd2_full", [NPAD, 128], bf16,
                                addr_space="Shared")
    bnin = [nc.dram_tensor(f"bnin{i}", [64, 2], f32) for i in (1, 2)]
    bnout = [nc.dram_tensor(f"bnout{i}", [64, 2], f32, addr_space="Shared")
             for i in (1, 2)]

    with tile.TileContext(nc) as tc:
        with (
            tc.tile_pool(name="const", bufs=1) as cpool,
            tc.tile_pool(name="gath", bufs=2) as gpool,
            tc.tile_pool(name="work", bufs=2) as wpool,
            tc.tile_pool(name="small", bufs=2) as spool,
            tc.tile_pool(name="node", bufs=1) as npool,
            tc.tile_pool(name="psA", bufs=2, space="PSUM") as psA,
            tc.tile_pool(name="psB", bufs=2, space="PSUM") as psB,
            tc.tile_pool(name="psG", bufs=2, space="PSUM") as psG,
        ):
            # ---- load + derive constants ----
            w1_s = cpool.tile([IN_DIM, 128], f32, tag="w1")
            nc.sync.dma_start(out=w1_s[:], in_=wp_d[0:128, :])
            w1h_s = cpool.tile([IN_DIM, 128], f16, tag="w1h")
            nc.vector.tensor_copy(out=w1h_s[:], in_=w1_s[:])
            w2_s = cpool.tile([HID, 128], f32, tag="w2")
            nc.sync.dma_start(out=w2_s[:], in_=wp_d[128:192, :])
            brow1_s = cpool.tile([1, 128], f32, tag="brow1")
            nc.sync.dma_start(out=brow1_s[:], in_=wp_d[192:193, :])
            brow2_s = cpool.tile([1, 128], f32, tag="brow2")
            nc.sync.dma_start(out=brow2_s[:], in_=wp_d[193:194, :])
            arow_s = cpool.tile([1, 128], f32, tag="arow")
            nc.sync.dma_start(out=arow_s[:], in_=wp_d[194:195, :])
            bn1_s = cpool.tile([HID, 8], f32, tag="bn1")
            nc.sync.dma_start(
                out=bn1_s[:],
                in_=wp_d[195:199, :].rearrange("a (b c) -> (a b) c", c=8))
            bn2_s = cpool.tile([OUT_DIM, 8], f32, tag="bn2")
            nc.sync.dma_start(
                out=bn2_s[:],
                in_=wp_d[199:203, :].rearrange("a (b c) -> (a b) c", c=8))
            r1_s = cpool.tile([H1, HID], f32, tag="r1")
            nc.sync.dma_start(
                out=r1_s[:],
                in_=wp_d[203:207, :].rearrange("a (b c) -> (a b) c", c=64))
            r2_s = cpool.tile([H2, OUT_DIM], f32, tag="r2")
            nc.sync.dma_start(
                out=r2_s[:],
                in_=wp_d[207:209, :].rearrange("a (b c) -> (a b) c", c=64))

            # gather idx tiles: replicate [16, C] across the 8 Q7 core groups
            fsidx_s = cpool.tile([128, NB * BLK_COLS], i16, tag="fsidx")
            fdidx_s = cpool.tile([128, NB * BLK_COLS], i16, tag="fdidx")
            for g in range(8):
                nc.sync.dma_start(out=fsidx_s[16 * g:16 * g + 16, :],
                                  in_=fsc_d[:, :])
                nc.sync.dma_start(out=fdidx_s[16 * g:16 * g + 16, :],
                                  in_=fdc_d[:, :])

            # bf16 one-hot operands (0..255 and -1 are exact in bf16):
            # 16-bit in/out doubles DVE throughput on the dominant is_equal
            dli_s = cpool.tile([128, NB * T_BLK], i16, tag="dli")
            nc.sync.dma_start(out=dli_s[:], in_=dl_d[:, :])
            dloc_s = cpool.tile([128, NB * T_BLK], bf16, tag="dloc")
            nc.vector.tensor_copy(out=dloc_s[:], in_=dli_s[:])

            iota_s = cpool.tile([128, BLK], bf16, tag="iota")
            nc.gpsimd.iota(iota_s[:], [[1, BLK]], channel_multiplier=0,
                           allow_small_or_imprecise_dtypes=True)

            ones_s = cpool.tile([1, 128], f32, tag="ones")
            nc.vector.memset(ones_s[:], 1.0)

            def bcast128(row_ap, w, tag, dtype=f32):
                ps = psB.tile([128, w], f32, tag="bc_ps")
                nc.tensor.matmul(out=ps[:], lhsT=ones_s[:], rhs=row_ap,
                                 start=True, stop=True)
                t = cpool.tile([128, w], dtype, tag=tag)
                nc.vector.tensor_copy(out=t[:], in_=ps[:])
                return t

            b1f_s = bcast128(brow1_s[0:1, :], 128, "b1f")
            b2f_s = bcast128(brow2_s[0:1, :], 128, "b2f")
            a1_s = bcast128(arow_s[0:1, 0:64], HID, "a1f", bf16)
            a2_s = bcast128(arow_s[0:1, 64:128], OUT_DIM, "a2f", bf16)

            # iota pre-expanded over the tile axis: value n at (n, t).  With
            # the one-hot stored [p, n, t], every is_equal operand has a
            # packed 2-byte last dim -> 2x DVE throughput.
            iota_exp = cpool.tile([128, BLK * SEC_T], bf16, tag="iotax")
            nc.vector.tensor_copy(
                out=iota_exp[:].rearrange("p (n t) -> p n t", t=SEC_T),
                in_=iota_s[:, :, None].to_broadcast([128, BLK, SEC_T]))

            h1_s = npool.tile([HID, CORE_NODES], f32, tag="h1")
            h_s = npool.tile([HID, CORE_NODES], f32, tag="h")

            NT = CORE_NODES // 128  # node tiles per core for GEMMs

            def gemm_layer(get_lhs, K, w_s, bfull_s, loc_dram, full_dram):
                for t in range(NT):
                    ps = psG.tile([128, 128], f32)
                    nc.tensor.matmul(
                        out=ps[:], lhsT=get_lhs(t),
                        rhs=w_s[:K, :], start=True, stop=True)
                    gs = spool.tile([128, 128], bf16, tag="gemm")
                    nc.vector.tensor_tensor(
                        out=gs[:], in0=ps[:], in1=bfull_s[:], op=Alu.add)
                    nc.sync.dma_start(
                        out=loc_dram[t * 128:(t + 1) * 128, :], in_=gs[:])
                nc.gpsimd.collective_compute(
                    "AllGather", Alu.bypass, replica_groups=RG,
                    ins=[loc_dram.ap().opt()], outs=[full_dram.ap().opt()])

            def gemm1_lhs(t):
                xg = spool.tile([IN_DIM, 128], f16, tag="xg")
                nc.sync.dma_start(out=xg[:], in_=xh_d[:, t * 128:(t + 1) * 128])
                return xg[:]

            def edge_layer(Hh, Ff, full_dram, loc_dram, attn_s, rX_s, bn_s,
                           hout_s, do_elu):
                HF = Hh * Ff
                VW = Hh + HF  # vals width per tile
                for b in range(NB):
                    ps_s = psA.tile([VW, BLK], f32, tag="scat")
                    for sec in range(2):
                        cA = b * BLK_COLS + sec * SEC_COLS
                        fs_e = gpool.tile([128, SEC_T * 128], bf16, tag="fs")
                        fd_e = gpool.tile([128, SEC_T * 128], bf16, tag="fd")
                        tab = full_dram[:, :] if sec == 0 else \
                            full_dram[HALF:, :]
                        nc.gpsimd.dma_gather(
                            out_ap=fs_e[:].rearrange("p (t e) -> p t e", e=128),
                            in_ap=tab,
                            idxs_ap=fsidx_s[:, cA:cA + SEC_COLS],
                            num_idxs=CAP, num_idxs_reg=CAP,
                            elem_size=128, elem_step=128, single_packet=False)
                        nc.gpsimd.dma_gather(
                            out_ap=fd_e[:].rearrange("p (t e) -> p t e", e=128),
                            in_ap=loc_dram[:, :],
                            idxs_ap=fdidx_s[:, cA:cA + SEC_COLS],
                            num_idxs=CAP, num_idxs_reg=CAP,
                            elem_size=128, elem_step=128, single_packet=False)
                        fs_v = fs_e[:].rearrange("p (t e) -> p t e", e=128)
                        fd_v = fd_e[:].rearrange("p (t e) -> p t e", e=128)

                        # one-hot (bf16), [p, n, t]: O[p, n, t] =
                        # (dloc[p, t] == n); all operands packed 2-byte in
                        # the last dim -> 2x DVE
                        dcol = b * T_BLK + sec * SEC_T
                        O_t = wpool.tile([128, BLK * SEC_T], bf16, tag="O")
                        O_v = O_t[:].rearrange("p (n t) -> p n t", t=SEC_T)
                        nc.vector.tensor_tensor(
                            out=O_v,
                            in0=dloc_s[:, None, dcol:dcol + SEC_T]
                                .to_broadcast([128, BLK, SEC_T]),
                            in1=iota_exp[:].rearrange(
                                "p (n t) -> p n t", t=SEC_T),
                            op=Alu.is_equal)

                        # z = fs + fd ; lrelu(z)*attn = (z + c|z|) * attn06
                        z_t = wpool.tile([128, SEC_T * 64], bf16, tag="z")
                        wz_t = wpool.tile([128, SEC_T * 64], bf16, tag="wz")
                        nc.vector.tensor_tensor(
                            out=z_t[:].rearrange("p (t e) -> p t e", e=64),
                            in0=fs_v[:, :, 0:64], in1=fd_v[:, :, 64:128],
                            op=Alu.add)
                        nc.scalar.activation(
                            out=wz_t[:], in_=z_t[:], func=Act.Abs,
                            scale=(1.0 - SLOPE) / (1.0 + SLOPE))
                        nc.vector.tensor_tensor(
                            out=wz_t[:], in0=z_t[:], in1=wz_t[:], op=Alu.add)
                        nc.vector.tensor_tensor(
                            out=z_t[:].rearrange("p (t e) -> p t e", e=64),
                            in0=wz_t[:].rearrange("p (t e) -> p t e", e=64),
                            in1=attn_s[:, None, :]
                                .to_broadcast([128, SEC_T, 64]),
                            op=Alu.mult)
                        # l = sum_f wz ; p = exp(l) into vals
                        l_t = spool.tile([128, SEC_T * Hh], f32, tag="l")
                        nc.vector.tensor_reduce(
                            out=l_t[:],
                            in_=z_t[:].rearrange("p (t h f) -> p t h f",
                                                 h=Hh, f=Ff),
                            axis=mybir.AxisListType.X, op=Alu.add)
                        # vals layout: [p*fs (HF cols) | p (Hh cols)] so that
                        # psum num rows start at partition 0, den at HF (=64)
                        vals = wpool.tile([128, SEC_T * VW], bf16, tag="vals")
                        vals3 = vals[:].rearrange("p (t v) -> p t v", v=VW)
                        nc.scalar.activation(
                            out=vals3[:, :, HF:VW],
                            in_=l_t[:].rearrange("p (t h) -> p t h", h=Hh),
                            func=Act.Exp)
                        nc.vector.tensor_tensor(
                            out=vals3[:, :, 0:HF].rearrange(
                                "p t (h f) -> p t h f", f=Ff),
                            in0=fs_v[:, :, 0:64].rearrange(
                                "p t (h f) -> p t h f", f=Ff),
                            in1=vals3[:, :, HF:VW][:, :, :, None]
                                .to_broadcast([128, SEC_T, Hh, Ff]),
                            op=Alu.mult)

                        # scatter: psum[v, n] += sum_e vals[e, v] * O[e, n]
                        O_m = O_t[:].rearrange("p (n t) -> p n t",
                                                       t=SEC_T)
                        for t in range(SEC_T):
                            nc.tensor.matmul(
                                out=ps_s[:],
                                lhsT=vals[:, t * VW:(t + 1) * VW],
                                rhs=O_m[:, :, t],
                                start=(sec == 0 and t == 0),
                                stop=(sec == 1 and t == SEC_T - 1))

                    # normalize: out = num * (1/den) + bias
                    den = spool.tile([Hh, BLK], f32, tag="den")
                    nc.vector.tensor_scalar(
                        out=den[:], in0=ps_s[HF:VW, :], scalar1=DEN_EPS,
                        scalar2=None, op0=Alu.add)
                    rcp = spool.tile([Hh, BLK], f32, tag="rcp")
                    nc.vector.reciprocal(out=rcp[:], in_=den[:])
                    ps_r = psB.tile([HF, BLK], f32, tag="rrep")
                    nc.tensor.matmul(out=ps_r[:], lhsT=rX_s[:], rhs=rcp[:],
                                     start=True, stop=True)
                    rr = spool.tile([HF, BLK], f32, tag="rr")
                    nc.vector.tensor_copy(out=rr[:], in_=ps_r[:])
                    o1 = spool.tile([HF, BLK], f32, tag="o1")
                    nc.vector.tensor_tensor(
                        out=o1[:], in0=ps_s[0:HF, :], in1=rr[:], op=Alu.mult)
                    nsl = slice(b * BLK, (b + 1) * BLK)
                    if do_elu:
                        ob = spool.tile([HF, BLK], f32, tag="ob")
                        nc.vector.tensor_scalar(
                            out=ob[:], in0=o1[:], scalar1=bn_s[:, 0:1],
                            scalar2=None, op0=Alu.add)
                        m_t = spool.tile([HF, BLK], f32, tag="elum")
                        nc.vector.tensor_scalar(
                            out=m_t[:], in0=ob[:], scalar1=0.0,
                            scalar2=None, op0=Alu.min)
                        e_t = spool.tile([HF, BLK], f32, tag="elue")
                        nc.scalar.activation(out=e_t[:], in_=m_t[:],
                                             func=Act.Exp)
                        nc.vector.tensor_scalar(
                            out=m_t[:], in0=ob[:], scalar1=0.0,
                            scalar2=None, op0=Alu.max)
                        t_t = spool.tile([HF, BLK], f32, tag="elut")
                        nc.vector.tensor_tensor(
                            out=t_t[:], in0=e_t[:], in1=m_t[:], op=Alu.add)
                        nc.vector.tensor_scalar(
                            out=hout_s[:, nsl], in0=t_t[:], scalar1=-1.0,
                            scalar2=None, op0=Alu.add)
                    else:
                        nc.vector.tensor_scalar(
                            out=hout_s[:, nsl], in0=o1[:], scalar1=bn_s[:, 0:1],
                            scalar2=None, op0=Alu.add)

            def bn_norm(hin_s, bn_s, bnin_d, bnout_d, D):
                """BN stats (blockwise) + AllReduce; returns (scale, shift)."""
                s_cols = spool.tile([D, NB], f32, tag="bnscols")
                q_cols = spool.tile([D, NB], f32, tag="bnqcols")
                for b in range(NB):
                    nsl = slice(b * BLK, (b + 1) * BLK)
                    nc.vector.tensor_reduce(
                        out=s_cols[:, b:b + 1], in_=hin_s[:, nsl],
                        axis=mybir.AxisListType.X, op=Alu.add)
                    scr = spool.tile([D, BLK], f32, tag="bnscr")
                    nc.scalar.activation(
                        out=scr[:], in_=hin_s[:, nsl], func=Act.Square,
                        accum_out=q_cols[:, b:b + 1])
                st = spool.tile([D, 2], f32, tag="bnst")
                nc.vector.tensor_reduce(out=st[:, 0:1], in_=s_cols[:],
                                        axis=mybir.AxisListType.X, op=Alu.add)
                nc.vector.tensor_reduce(out=st[:, 1:2], in_=q_cols[:],
                                        axis=mybir.AxisListType.X, op=Alu.add)
                nc.sync.dma_start(out=bnin_d[:, :], in_=st[:])
                nc.gpsimd.collective_compute(
                    "AllReduce", Alu.add, replica_groups=RG,
                    ins=[bnin_d.ap().opt()], outs=[bnout_d.ap().opt()])
                g = spool.tile([D, 2], f32, tag="bng")
                nc.sync.dma_start(out=g[:], in_=bnout_d[:, :])
                # mu = (S - corr)/N ; var = (SQ - corrsq)/N - mu^2
                t_a = spool.tile([D, 1], f32, tag="bnta")
                nc.vector.tensor_tensor(out=t_a[:], in0=g[:, 0:1],
                                        in1=bn_s[:, 3:4], op=Alu.subtract)
                mu = spool.tile([D, 1], f32, tag="bnmu")
                nc.vector.tensor_scalar(out=mu[:], in0=t_a[:],
                                        scalar1=1.0 / N_REAL, scalar2=None,
                                        op0=Alu.mult)
                t_b = spool.tile([D, 1], f32, tag="bntb")
                nc.vector.tensor_tensor(out=t_b[:], in0=g[:, 1:2],
                                        in1=bn_s[:, 4:5], op=Alu.subtract)
                msq = spool.tile([D, 1], f32, tag="bnmsq")
                nc.vector.tensor_scalar(out=msq[:], in0=t_b[:],
                                        scalar1=1.0 / N_REAL, scalar2=None,
                                        op0=Alu.mult)
                mu2 = spool.tile([D, 1], f32, tag="bnmu2")
                nc.vector.tensor_tensor(out=mu2[:], in0=mu[:], in1=mu[:],
                                        op=Alu.mult)
                var = spool.tile([D, 1], f32, tag="bnvar")
                nc.vector.tensor_tensor(out=var[:], in0=msq[:], in1=mu2[:],
                                        op=Alu.subtract)
                sd = spool.tile([D, 1], f32, tag="bnsd")
                nc.scalar.activation(out=sd[:], in_=var[:], func=Act.Sqrt,
                                     bias=bn_s[:, 5:6])
                rs = spool.tile([D, 1], f32, tag="bnrs")
                nc.vector.reciprocal(out=rs[:], in_=sd[:])
                scl = spool.tile([D, 1], f32, tag="bnscl")
                nc.vector.tensor_tensor(out=scl[:], in0=bn_s[:, 1:2],
                                        in1=rs[:], op=Alu.mult)
                t_c = spool.tile([D, 1], f32, tag="bntc")
                nc.vector.tensor_tensor(out=t_c[:], in0=mu[:], in1=scl[:],
                                        op=Alu.mult)
                shf = spool.tile([D, 1], f32, tag="bnshf")
                nc.vector.tensor_tensor(out=shf[:], in0=bn_s[:, 2:3],
                                        in1=t_c[:], op=Alu.subtract)
                return scl, shf

            def norm_elu_blockwise(dst_s, src_s, scl, shf, D, do_elu):
                for b in range(NB):
                    nsl = slice(b * BLK, (b + 1) * BLK)
                    if not do_elu:
                        nc.vector.tensor_scalar(
                            out=dst_s[:, nsl], in0=src_s[:, nsl],
                            scalar1=scl[:], scalar2=shf[:],
                            op0=Alu.mult, op1=Alu.add)
                        continue
                    hb = spool.tile([D, BLK], f32, tag="nrmh")
                    nc.vector.tensor_scalar(
                        out=hb[:], in0=src_s[:, nsl], scalar1=scl[:],
                        scalar2=shf[:], op0=Alu.mult, op1=Alu.add)
                    m_t = spool.tile([D, BLK], f32, tag="nrmm")
                    nc.vector.tensor_scalar(out=m_t[:], in0=hb[:],
                                            scalar1=0.0, scalar2=None,
                                            op0=Alu.min)
                    e_t = spool.tile([D, BLK], f32, tag="nrme")
                    nc.scalar.activation(out=e_t[:], in_=m_t[:], func=Act.Exp)
                    nc.vector.tensor_scalar(out=m_t[:], in0=hb[:],
                                            scalar1=0.0, scalar2=None,
                                            op0=Alu.max)
                    t_t = spool.tile([D, BLK], f32, tag="nrmt")
                    nc.vector.tensor_tensor(out=t_t[:], in0=e_t[:],
                                            in1=m_t[:], op=Alu.add)
                    nc.vector.tensor_scalar(out=dst_s[:, nsl], in0=t_t[:],
                                            scalar1=-1.0, scalar2=None,
                                            op0=Alu.add)

            # ================= layer 1 =================
            gemm_layer(gemm1_lhs, IN_DIM, w1h_s, b1f_s, fsfd1_loc, fsfd1_full)
            edge_layer(H1, F1, fsfd1_full, fsfd1_loc, a1_s, r1_s, bn1_s,
                       h1_s, do_elu=True)
            scl1, shf1 = bn_norm(h1_s, bn1_s, bnin[0], bnout[0], HID)
            norm_elu_blockwise(h_s, h1_s, scl1, shf1, HID, do_elu=True)

            # ================= layer 2 =================
            gemm_layer(lambda t: h_s[:, t * 128:(t + 1) * 128], HID, w2_s,
                       b2f_s, fsfd2_loc, fsfd2_full)
            edge_layer(H2, F2, fsfd2_full, fsfd2_loc, a2_s, r2_s, bn2_s,
                       h1_s, do_elu=False)  # reuse h1_s as h2 buffer
            scl2, shf2 = bn_norm(h1_s, bn2_s, bnin[1], bnout[1], OUT_DIM)
            outb = npool.tile([OUT_DIM, CORE_NODES], f16, tag="outb")
            norm_elu_blockwise(outb, h1_s, scl2, shf2, OUT_DIM, do_elu=False)
            nc.sync.dma_start(out=out_d[:, :], in_=outb[:])

    return nc


# ---------------------------------------------------------------- pjrt runner
_CACHE = {}


def _get_runtime():
    if "rt" in _CACHE:
        return _CACHE["rt"]
    for p in ("/opt/trn_rl_repo",):
        if os.path.isdir(p) and p not in sys.path:
            sys.path.insert(0, p)
    import jax
    import jax.numpy as jnp
    from jax.sharding import Mesh, PartitionSpec
    from jax.experimental.shard_map import shard_map
    from concourse import mybir
    from concourse.bass2jax import (_bass_exec_p, partition_id_tensor,
                                    install_neuronx_cc_hook)

    install_neuronx_cc_hook()
    nc = build_program()
    nc.finalize()

    partition_name = (nc.partition_id_tensor.name
                      if nc.partition_id_tensor else None)
    dbg_name = nc.dbg_addr.name if nc.dbg_addr is not None else None
    in_names, out_names, out_info = [], [], []
    for alloc in nc.m.functions[0].allocations:
        if not isinstance(alloc, mybir.MemoryLocationSet):
            continue
        name = alloc.memorylocations[0].name
        if alloc.kind == "ExternalInput":
            if name != partition_name:
                in_names.append(name)
        elif alloc.kind == "ExternalOutput":
            out_names.append(name)
            out_info.append((tuple(alloc.tensor_shape),
                             mybir.dt.np(alloc.dtype)))
    in_names_all = tuple(in_names + out_names
                         + ([partition_name] if partition_name else []))
    out_avals = tuple(jax.core.ShapedArray(s, d) for s, d in out_info)

    assert dbg_name is None and in_names == OPERAND_NAMES, (dbg_name, in_names)
    n_params = len(in_names)

    def _body(*args):
        operands = list(args)
        if partition_name is not None:
            operands.append(partition_id_tensor())
        return tuple(_bass_exec_p.bind(
            *operands, out_avals=out_avals, in_names=in_names_all,
            out_names=tuple(out_names), lowering_input_output_aliases=(),
            sim_require_finite=True, sim_require_nnan=True, nc=nc))

    devices = jax.devices()[:NCORES]
    assert len(devices) == NCORES
    mesh = Mesh(np.asarray(devices), ("core",))
    from jax.sharding import NamedSharding
    spec = NamedSharding(mesh, PartitionSpec("core"))
    fn = jax.jit(shard_map(
        _body, mesh=mesh,
        in_specs=(PartitionSpec("core"),) * (n_params + len(out_names)),
        out_specs=(PartitionSpec("core"),) * len(out_names),
        check_rep=False),
        donate_argnums=tuple(range(n_params, n_params + len(out_names))))
    # ExternalOutput staging buffers, zero-filled on device (never uploaded).
    # Donated into the bass_exec results; the kernel writes every element.
    zfn = jax.jit(
        lambda: tuple(jnp.zeros((NCORES * s[0], *s[1:]), d)
                      for s, d in out_info),
        out_shardings=(spec,) * len(out_info))
    _CACHE["rt"] = (fn, zfn, mesh)
    return _CACHE["rt"]


def kernel(**inputs) -> np.ndarray:
    import threading
    import jax
    from jax.sharding import NamedSharding, PartitionSpec

    fn, zfn, mesh = _get_runtime()
    spec = NamedSharding(mesh, PartitionSpec("core"))
    # overlap the (largest) x upload with the remaining host-side prep
    xh_g = make_xh(inputs)
    holder = {}

    def _put():
        holder["xh"] = jax.device_put(xh_g, spec)

    th = threading.Thread(target=_put)
    th.start()
    rest = make_rest(inputs)
    th.join()
    (out,) = fn(holder["xh"], *rest, *zfn())
    arr = np.asarray(out)  # [NCORES*OUT_DIM, CORE_NODES] f16
    res = arr.reshape(NCORES, OUT_DIM, CORE_NODES).transpose(0, 2, 1)
    return np.ascontiguousarray(
        res.reshape(NPAD, OUT_DIM)[:N_REAL]).astype(np.float32)


if __name__ == "__main__":
    import jax
    with jax.default_device(jax.devices("cpu")[0]):
        import reference
        inputs = {k: np.asarray(v) for k, v in reference.setup_inputs().items()}
        expected = np.asarray(reference.reference(**inputs))
    actual = kernel(**inputs)
    rel = np.linalg.norm(actual - expected) / np.linalg.norm(expected)
    print("Relative error:", rel)


# revision 26
# speedup vs baseline: 38.7259x; 1.1550x over previous
"""Trainium2 Bass kernel for a 2-layer GATv2 encoder (nn_GATv2Encoder).

Strategy (8 NeuronCores, SPMD):
- Host sorts edges by dst; dst nodes are partitioned into contiguous 256-node
  blocks, 25 blocks per core -> each core owns a disjoint contiguous node
  range, so segment softmax/scatter are core-local (no cross-core segment
  reductions needed).
- Each block's edges are split into two sections by src < HALF (int16 index
  range for dma_gather), each section padded to a fixed tile capacity.
- Per layer: node-parallel GEMMs produce a combined [fs|fd] table (512B rows);
  fs tables are AllGather'd; fs[src] and fd[dst] are fetched per edge with
  dma_gather (256B elements). Logits l = sum_f attn*leakyrelu(fs+fd) computed
  without segment-max (logits are tiny); softmax normalization is folded into
  a final num/den division. Scatter-add is a one-hot matmul accumulated in
  PSUM per block (bf16 one-hot + bf16 vals, fp32 accumulation).
- BatchNorm stats are computed per-core over the feature-partition layout and
  AllReduce'd; padded rows are corrected with host-precomputed constants.

Host<->device traffic is minimized (the axon tunnel is ~45 MB/s): x ships as
fp16, gather indices ship un-replicated (int16) and are replicated across the
8 gpsimd partition groups on device, all replicated constant tiles are built
on device from a small packed table, output returns as fp16, and the
ExternalOutput staging zeros are materialized on device instead of uploaded.
"""

import os
import sys
import numpy as np

# ---------------------------------------------------------------- constants
N_REAL = 50000
E_REAL = 800000
IN_DIM, HID, OUT_DIM = 128, 64, 64
H1, F1 = 8, 8
H2, F2 = 4, 16
SLOPE = 0.2
EPS = 1e-5
DEN_EPS = 1e-30

NCORES = 8
BLK = 256                      # nodes per block (one-hot / psum free width)
NB = 25                        # blocks per core
CORE_NODES = BLK * NB          # 6400
NPAD = NCORES * CORE_NODES     # 51200
HALF = NPAD // 2               # 25600 (int16-safe gather split)
NBLK_TOT = NCORES * NB         # 200
TILE = 128                     # edges per matmul tile
SEC_T = 18                     # tiles per section (A and B)
CAP = SEC_T * TILE             # 2304 edge slots per section
T_BLK = 2 * SEC_T              # 36 tiles per block
SLOTS = 2 * CAP                # 4608 edge slots per block
SEC_COLS = CAP // 16           # idx cols per section (144)
BLK_COLS = SLOTS // 16         # idx cols per block (288)
WROWS = 209                    # rows in the packed f32 const table


# ---------------------------------------------------------------- host prep
def host_prep(src, dst):
    """Vectorized edge bucketing. Returns global (concat-over-core) arrays:
    fsc/fdc [NCORES*16, NB*BLK_COLS] i16 (un-replicated dma_gather indices),
    dl [NCORES*128, NB*T_BLK] i16 (dst-local slot->node map, -1 = padding)."""
    src = np.asarray(src).astype(np.int32, copy=False)
    dst = np.asarray(dst).astype(np.int32, copy=False)
    # only the dst block (0..199, fits uint8) matters for bucketing — a
    # stable argsort on uint8 keys is a single radix pass
    blk_of = (dst >> 8).astype(np.uint8)
    order = np.argsort(blk_of, kind="stable")
    s_src, s_dst = src[order], dst[order]

    blk = (s_dst >> 8).astype(np.int32)             # block id per edge
    isB = s_src >= HALF
    blk_counts = np.bincount(blk, minlength=NBLK_TOT)
    blk_start = np.zeros(NBLK_TOT, np.int64)
    np.cumsum(blk_counts[:-1], out=blk_start[1:])
    start_e = blk_start[blk]
    nB_bef = np.cumsum(isB, dtype=np.int32) - isB    # B-count before edge e
    posB = nB_bef - nB_bef[start_e]
    pos_in_blk = (np.arange(len(s_src), dtype=np.int64) - start_e).astype(
        np.int32)
    posA = pos_in_blk - posB

    nB_per = np.bincount(blk[isB], minlength=NBLK_TOT)
    nA_per = blk_counts - nB_per
    assert nA_per.max() <= CAP and nB_per.max() <= CAP, (
        nA_per.max(), nB_per.max())

    slot = np.where(isB, CAP + posB, posA)
    gslot = blk * np.int32(SLOTS) + slot
    fsv = np.where(isB, s_src - HALF, s_src).astype(np.int16)
    fdv = (s_dst - (blk // NB) * CORE_NODES).astype(np.int16)
    dlv = (s_dst & 255).astype(np.int16)

    fs_flat = np.zeros(NBLK_TOT * SLOTS, np.int16)
    fd_flat = np.zeros(NBLK_TOT * SLOTS, np.int16)
    dl_flat = np.full(NBLK_TOT * SLOTS, -1, np.int16)
    fs_flat[gslot] = fsv
    fd_flat[gslot] = fdv
    dl_flat[gslot] = dlv

    # dma_gather layout: idx j of a section at (partition j%16, col j//16)
    def wrap16(flat):
        a = flat.reshape(NCORES, NB * 2, SEC_COLS, 16)
        return np.ascontiguousarray(a.transpose(0, 3, 1, 2)).reshape(
            NCORES * 16, NB * BLK_COLS)

    # one-hot layout: slot s of block b at (partition s%128, col b*T_BLK+s//128)
    d4 = dl_flat.reshape(NCORES, NB, T_BLK, 128)
    dl_g = np.ascontiguousarray(d4.transpose(0, 3, 1, 2)).reshape(
        NCORES * 128, NB * T_BLK)
    return wrap16(fs_flat), wrap16(fd_flat), dl_g


def _elu_np(x):
    return np.where(x > 0, x, np.exp(np.minimum(x, 0)) - 1).astype(np.float32)


def make_xh(inputs):
    """x.T as fp16, padded to NPAD nodes, core-major rows."""
    x = np.asarray(inputs["x"], np.float32)
    xh_g = np.zeros((NCORES * IN_DIM, CORE_NODES), np.float16)
    xv = x.T  # [128, 50000] view
    for c in range(NCORES):
        lo = c * CORE_NODES
        hi = min(lo + CORE_NODES, N_REAL)
        if hi > lo:
            xh_g[c * IN_DIM:(c + 1) * IN_DIM, :hi - lo] = xv[:, lo:hi]
    return xh_g


def make_rest(inputs):
    """The 4 non-x global operand arrays (axis 0 = concat over cores)."""
    f32 = np.float32
    fsc_g, fdc_g, dl_g = host_prep(inputs["src"], inputs["dst"])

    # packed f32 const table, replicated per core
    lr_a = (1.0 + SLOPE) / 2.0
    wp = np.zeros((WROWS, 128), f32)
    wp[0:128] = np.concatenate([np.asarray(inputs["Wsrc1"], f32),
                                np.asarray(inputs["Wdst1"], f32)], 1)
    wp[128:192] = np.concatenate([np.asarray(inputs["Wsrc2"], f32),
                                  np.asarray(inputs["Wdst2"], f32)], 1)
    wp[192] = np.concatenate([np.asarray(inputs["bsrc1"], f32),
                              np.asarray(inputs["bdst1"], f32)])
    wp[193] = np.concatenate([np.asarray(inputs["bsrc2"], f32),
                              np.asarray(inputs["bdst2"], f32)])
    wp[194, 0:64] = np.asarray(inputs["attn1"], f32).reshape(-1) * lr_a
    wp[194, 64:128] = np.asarray(inputs["attn2"], f32).reshape(-1) * lr_a

    npad_rows = NPAD - N_REAL
    bias1 = np.asarray(inputs["bias1"], f32)
    bias2 = np.asarray(inputs["bias2"], f32)
    cpad1 = _elu_np(bias1)
    bn1 = np.zeros((HID, 8), f32)
    bn1[:, 0] = bias1
    bn1[:, 1] = np.asarray(inputs["gamma1"], f32)
    bn1[:, 2] = np.asarray(inputs["beta1"], f32)
    bn1[:, 3] = npad_rows * cpad1
    bn1[:, 4] = npad_rows * cpad1 ** 2
    bn1[:, 5] = EPS
    bn2 = np.zeros((OUT_DIM, 8), f32)
    bn2[:, 0] = bias2
    bn2[:, 1] = np.asarray(inputs["gamma2"], f32)
    bn2[:, 2] = np.asarray(inputs["beta2"], f32)
    bn2[:, 3] = npad_rows * bias2
    bn2[:, 4] = npad_rows * bias2 ** 2
    bn2[:, 5] = EPS
    wp[195:199] = bn1.reshape(4, 128)
    wp[199:203] = bn2.reshape(4, 128)

    r1 = np.zeros((H1, HID), f32)
    for h in range(H1):
        r1[h, h * F1:(h + 1) * F1] = 1.0
    r2 = np.zeros((H2, OUT_DIM), f32)
    for h in range(H2):
        r2[h, h * F2:(h + 1) * F2] = 1.0
    wp[203:207] = r1.reshape(4, 128)
    wp[207:209] = r2.reshape(2, 128)
    wp_g = np.tile(wp, (NCORES, 1))

    return [fsc_g, fdc_g, dl_g, wp_g]


def make_operands(inputs):
    """All 5 global operand arrays, in program input order."""
    return [make_xh(inputs)] + make_rest(inputs)


OPERAND_NAMES = ["xh", "fsc", "fdc", "dl", "wp"]


# ---------------------------------------------------------------- bass program
def build_program():
    import concourse.bacc as bacc
    import concourse.tile as tile
    from concourse import mybir

    f32 = mybir.dt.float32
    f16 = mybir.dt.float16
    bf16 = mybir.dt.bfloat16
    i16 = mybir.dt.int16
    Alu = mybir.AluOpType
    Act = mybir.ActivationFunctionType

    nc = bacc.Bacc(None, target_bir_lowering=False, num_devices=NCORES)
    RG = [list(range(NCORES))]

    # ---- I/O ----
    xh_d = nc.dram_tensor("xh", [IN_DIM, CORE_NODES], f16, kind="ExternalInput")
    fsc_d = nc.dram_tensor("fsc", [16, NB * BLK_COLS], i16, kind="ExternalInput")
    fdc_d = nc.dram_tensor("fdc", [16, NB * BLK_COLS], i16, kind="ExternalInput")
    dl_d = nc.dram_tensor("dl", [128, NB * T_BLK], i16, kind="ExternalInput")
    wp_d = nc.dram_tensor("wp", [WROWS, 128], f32, kind="ExternalInput")
    out_d = nc.dram_tensor("outT", [OUT_DIM, CORE_NODES], f16,
                           kind="ExternalOutput")

    # ---- internal DRAM ----
    # [fs|fd] per-node table in bf16: 256B rows satisfy dma_gather's elem
    # granularity, the AllGather moves half the f32 bytes, and bf16 gather
    # outputs unlock the DVE 2x 16-bit path for the edge element-wise chain
    fsfd1_loc = nc.dram_tensor("fsfd1_loc", [CORE_NODES, 128], bf16)
    fsfd1_full = nc.dram_tensor("fsfd1_full", [NPAD, 128], bf16,
                                addr_space="Shared")
    fsfd2_loc = nc.dram_tensor("fsfd2_loc", [CORE_NODES, 128], bf16)
    fsfd2_full = nc.dram_tensor("fsfd2_full", [NPAD, 128], bf16,
                                addr_space="Shared")
    bnin = [nc.dram_tensor(f"bnin{i}", [64, 2], f32) for i in (1, 2)]
    bnout = [nc.dram_tensor(f"bnout{i}", [64, 2], f32, addr_space="Shared")
             for i in (1, 2)]

    with tile.TileContext(nc) as tc:
        with (
            tc.tile_pool(name="const", bufs=1) as cpool,
            tc.tile_pool(name="gath", bufs=2) as gpool,
            tc.tile_pool(name="work", bufs=2) as wpool,
            tc.tile_pool(name="small", bufs=2) as spool,
            tc.tile_pool(name="node", bufs=1) as npool,
            tc.tile_pool(name="psA", bufs=2, space="PSUM") as psA,
            tc.tile_pool(name="psB", bufs=2, space="PSUM") as psB,
            tc.tile_pool(name="psG", bufs=2, space="PSUM") as psG,
        ):
            # ---- load + derive constants ----
            w1_s = cpool.tile([IN_DIM, 128], f32, tag="w1")
            nc.sync.dma_start(out=w1_s[:], in_=wp_d[0:128, :])
            w1h_s = cpool.tile([IN_DIM, 128], f16, tag="w1h")
            nc.vector.tensor_copy(out=w1h_s[:], in_=w1_s[:])
            w2_s = cpool.tile([HID, 128], f32, tag="w2")
            nc.sync.dma_start(out=w2_s[:], in_=wp_d[128:192, :])
            brow1_s = cpool.tile([1, 128], f32, tag="brow1")
            nc.sync.dma_start(out=brow1_s[:], in_=wp_d[192:193, :])
            brow2_s = cpool.tile([1, 128], f32, tag="brow2")
            nc.sync.dma_start(out=brow2_s[:], in_=wp_d[193:194, :])
            arow_s = cpool.tile([1, 128], f32, tag="arow")
            nc.sync.dma_start(out=arow_s[:], in_=wp_d[194:195, :])
            bn1_s = cpool.tile([HID, 8], f32, tag="bn1")
            nc.sync.dma_start(
                out=bn1_s[:],
                in_=wp_d[195:199, :].rearrange("a (b c) -> (a b) c", c=8))
            bn2_s = cpool.tile([OUT_DIM, 8], f32, tag="bn2")
            nc.sync.dma_start(
                out=bn2_s[:],
                in_=wp_d[199:203, :].rearrange("a (b c) -> (a b) c", c=8))
            r1_s = cpool.tile([H1, HID], f32, tag="r1")
            nc.sync.dma_start(
                out=r1_s[:],
                in_=wp_d[203:207, :].rearrange("a (b c) -> (a b) c", c=64))
            r2_s = cpool.tile([H2, OUT_DIM], f32, tag="r2")
            nc.sync.dma_start(
                out=r2_s[:],
                in_=wp_d[207:209, :].rearrange("a (b c) -> (a b) c", c=64))

            # gather idx tiles: replicate [16, C] across the 8 Q7 core groups
            fsidx_s = cpool.tile([128, NB * BLK_COLS], i16, tag="fsidx")
            fdidx_s = cpool.tile([128, NB * BLK_COLS], i16, tag="fdidx")
            for g in range(8):
                nc.sync.dma_start(out=fsidx_s[16 * g:16 * g + 16, :],
                                  in_=fsc_d[:, :])
                nc.sync.dma_start(out=fdidx_s[16 * g:16 * g + 16, :],
                                  in_=fdc_d[:, :])

            # bf16 one-hot operands (0..255 and -1 are exact in bf16):
            # 16-bit in/out doubles DVE throughput on the dominant is_equal
            dli_s = cpool.tile([128, NB * T_BLK], i16, tag="dli")
            nc.sync.dma_start(out=dli_s[:], in_=dl_d[:, :])
            dloc_s = cpool.tile([128, NB * T_BLK], bf16, tag="dloc")
            nc.vector.tensor_copy(out=dloc_s[:], in_=dli_s[:])

            iota_s = cpool.tile([128, BLK], bf16, tag="iota")
            nc.gpsimd.iota(iota_s[:], [[1, BLK]], channel_multiplier=0,
                           allow_small_or_imprecise_dtypes=True)

            ones_s = cpool.tile([1, 128], f32, tag="ones")
            nc.vector.memset(ones_s[:], 1.0)

            def bcast128(row_ap, w, tag, dtype=f32):
                ps = psB.tile([128, w], f32, tag="bc_ps")
                nc.tensor.matmul(out=ps[:], lhsT=ones_s[:], rhs=row_ap,
                                 start=True, stop=True)
                t = cpool.tile([128, w], dtype, tag=tag)
                nc.vector.tensor_copy(out=t[:], in_=ps[:])
                return t

            b1f_s = bcast128(brow1_s[0:1, :], 128, "b1f")
            b2f_s = bcast128(brow2_s[0:1, :], 128, "b2f")
            a1_s = bcast128(arow_s[0:1, 0:64], HID, "a1f", bf16)
            a2_s = bcast128(arow_s[0:1, 64:128], OUT_DIM, "a2f", bf16)

            # iota pre-expanded over the tile axis: value n at (n, t).  With
            # the one-hot stored [p, n, t], every is_equal operand has a
            # packed 2-byte last dim -> 2x DVE throughput.
            iota_exp = cpool.tile([128, BLK * SEC_T], bf16, tag="iotax")
            nc.vector.tensor_copy(
                out=iota_exp[:].rearrange("p (n t) -> p n t", t=SEC_T),
                in_=iota_s[:, :, None].to_broadcast([128, BLK, SEC_T]))

            h1_s = npool.tile([HID, CORE_NODES], f32, tag="h1")
            h_s = npool.tile([HID, CORE_NODES], f32, tag="h")

            NT = CORE_NODES // 128  # node tiles per core for GEMMs

            def gemm_layer(get_lhs, K, w_s, bfull_s, loc_dram, full_dram):
                for t in range(NT):
                    ps = psG.tile([128, 128], f32)
                    nc.tensor.matmul(
                        out=ps[:], lhsT=get_lhs(t),
                        rhs=w_s[:K, :], start=True, stop=True)
                    gs = spool.tile([128, 128], bf16, tag="gemm")
                    nc.vector.tensor_tensor(
                        out=gs[:], in0=ps[:], in1=bfull_s[:], op=Alu.add)
                    nc.sync.dma_start(
                        out=loc_dram[t * 128:(t + 1) * 128, :], in_=gs[:])
                nc.gpsimd.collective_compute(
                    "AllGather", Alu.bypass, replica_groups=RG,
                    ins=[loc_dram.ap().opt()], outs=[full_dram.ap().opt()])

            def gemm1_lhs(t):
                xg = spool.tile([IN_DIM, 128], f16, tag="xg")
                nc.sync.dma_start(out=xg[:], in_=xh_d[:, t * 128:(t + 1) * 128])
                return xg[:]

            def edge_layer(Hh, Ff, full_dram, loc_dram, attn_s, rX_s, bn_s,
                           hout_s, do_elu):
                HF = Hh * Ff
                VW = Hh + HF  # vals width per tile
                for b in range(NB):
                    ps_s = psA.tile([VW, BLK], f32, tag="scat")
                    for sec in range(2):
                        cA = b * BLK_COLS + sec * SEC_COLS
                        fs_e = gpool.tile([128, SEC_T * 128], bf16, tag="fs")
                        fd_e = gpool.tile([128, SEC_T * 128], bf16, tag="fd")
                        tab = full_dram[:, :] if sec == 0 else \
                            full_dram[HALF:, :]
                        nc.gpsimd.dma_gather(
                            out_ap=fs_e[:].rearrange("p (t e) -> p t e", e=128),
                            in_ap=tab,
                            idxs_ap=fsidx_s[:, cA:cA + SEC_COLS],
                            num_idxs=CAP, num_idxs_reg=CAP,
                            elem_size=128, elem_step=128, single_packet=False)
                        nc.gpsimd.dma_gather(
                            out_ap=fd_e[:].rearrange("p (t e) -> p t e", e=128),
                            in_ap=loc_dram[:, :],
                            idxs_ap=fdidx_s[:, cA:cA + SEC_COLS],
                            num_idxs=CAP, num_idxs_reg=CAP,
                            elem_size=128, elem_step=128, single_packet=False)
                        fs_v = fs_e[:].rearrange("p (t e) -> p t e", e=128)
                        fd_v = fd_e[:].rearrange("p (t e) -> p t e", e=128)

                        # one-hot (bf16), [p, n, t]: O[p, n, t] =
                        # (dloc[p, t] == n); all operands packed 2-byte in
                        # the last dim -> 2x DVE
                        dcol = b * T_BLK + sec * SEC_T
                        O_t = wpool.tile([128, BLK * SEC_T], bf16, tag="O")
                        O_v = O_t[:].rearrange("p (n t) -> p n t", t=SEC_T)
                        nc.vector.tensor_tensor(
                            out=O_v,
                            in0=dloc_s[:, None, dcol:dcol + SEC_T]
                                .to_broadcast([128, BLK, SEC_T]),
                            in1=iota_exp[:].rearrange(
                                "p (n t) -> p n t", t=SEC_T),
                            op=Alu.is_equal)

                        # z = fs + fd ; lrelu(z)*attn = (z + c|z|) * attn06
                        z_t = wpool.tile([128, SEC_T * 64], bf16, tag="z")
                        wz_t = wpool.tile([128, SEC_T * 64], bf16, tag="wz")
                        nc.vector.tensor_tensor(
                            out=z_t[:].rearrange("p (t e) -> p t e", e=64),
                            in0=fs_v[:, :, 0:64], in1=fd_v[:, :, 64:128],
                            op=Alu.add)
                        nc.scalar.activation(
                            out=wz_t[:], in_=z_t[:], func=Act.Abs,
                            scale=(1.0 - SLOPE) / (1.0 + SLOPE))
                        nc.vector.tensor_tensor(
                            out=wz_t[:], in0=z_t[:], in1=wz_t[:], op=Alu.add)
                        nc.vector.tensor_tensor(
                            out=z_t[:].rearrange("p (t e) -> p t e", e=64),
                            in0=wz_t[:].rearrange("p (t e) -> p t e", e=64),
                            in1=attn_s[:, None, :]
                                .to_broadcast([128, SEC_T, 64]),
                            op=Alu.mult)
                        # l = sum_f wz ; p = exp(l) into vals
                        l_t = spool.tile([128, SEC_T * Hh], f32, tag="l")
                        nc.vector.tensor_reduce(
                            out=l_t[:],
                            in_=z_t[:].rearrange("p (t h f) -> p t h f",
                                                 h=Hh, f=Ff),
                            axis=mybir.AxisListType.X, op=Alu.add)
                        # vals layout: [p*fs (HF cols) | p (Hh cols)] so that
                        # psum num rows start at partition 0, den at HF (=64)
                        vals = wpool.tile([128, SEC_T * VW], bf16, tag="vals")
                        vals3 = vals[:].rearrange("p (t v) -> p t v", v=VW)
                        nc.scalar.activation(
                            out=vals3[:, :, HF:VW],
                            in_=l_t[:].rearrange("p (t h) -> p t h", h=Hh),
                            func=Act.Exp)
                        nc.vector.tensor_tensor(
                            out=vals3[:, :, 0:HF].rearrange(
                                "p t (h f) -> p t h f", f=Ff),
                            in0=fs_v[:, :, 0:64].rearrange(
                                "p t (h f) -> p t h f", f=Ff),
                            in1=vals3[:, :, HF:VW][:, :, :, None]
                                .to_broadcast([128, SEC_T, Hh, Ff]),
                            op=Alu.mult)

                        # scatter: psum[v, n] += sum_e vals[e, v] * O[e, n]
                        O_m = O_t[:].rearrange("p (n t) -> p n t",
                                                       t=SEC_T)
                        for t in range(SEC_T):
                            nc.tensor.matmul(
                                out=ps_s[:],
                                lhsT=vals[:, t * VW:(t + 1) * VW],
                                rhs=O_m[:, :, t],
                                start=(sec == 0 and t == 0),
                                stop=(sec == 1 and t == SEC_T - 1))

                    # normalize: out = num * (1/den) + bias
                    den = spool.tile([Hh, BLK], f32, tag="den")
                    nc.vector.tensor_scalar(
                        out=den[:], in0=ps_s[HF:VW, :], scalar1=DEN_EPS,
                        scalar2=None, op0=Alu.add)
                    rcp = spool.tile([Hh, BLK], f32, tag="rcp")
                    nc.vector.reciprocal(out=rcp[:], in_=den[:])
                    ps_r = psB.tile([HF, BLK], f32, tag="rrep")
                    nc.tensor.matmul(out=ps_r[:], lhsT=rX_s[:], rhs=rcp[:],
                                     start=True, stop=True)
                    rr = spool.tile([HF, BLK], f32, tag="rr")
                    nc.vector.tensor_copy(out=rr[:], in_=ps_r[:])
                    o1 = spool.tile([HF, BLK], f32, tag="o1")
                    nc.vector.tensor_tensor(
                        out=o1[:], in0=ps_s[0:HF, :], in1=rr[:], op=Alu.mult)
                    nsl = slice(b * BLK, (b + 1) * BLK)
                    if do_elu:
                        ob = spool.tile([HF, BLK], f32, tag="ob")
                        nc.vector.tensor_scalar(
                            out=ob[:], in0=o1[:], scalar1=bn_s[:, 0:1],
                            scalar2=None, op0=Alu.add)
                        m_t = spool.tile([HF, BLK], f32, tag="elum")
                        nc.vector.tensor_scalar(
                            out=m_t[:], in0=ob[:], scalar1=0.0,
                            scalar2=None, op0=Alu.min)
                        e_t = spool.tile([HF, BLK], f32, tag="elue")
                        nc.scalar.activation(out=e_t[:], in_=m_t[:],
                                             func=Act.Exp)
                        nc.vector.tensor_scalar(
                            out=m_t[:], in0=ob[:], scalar1=0.0,
                            scalar2=None, op0=Alu.max)
                        t_t = spool.tile([HF, BLK], f32, tag="elut")
                        nc.vector.tensor_tensor(
                            out=t_t[:], in0=e_t[:], in1=m_t[:], op=Alu.add)
                        nc.vector.tensor_scalar(
                            out=hout_s[:, nsl], in0=t_t[:], scalar1=-1.0,
                            scalar2=None, op0=Alu.add)
                    else:
                        nc.vector.tensor_scalar(
                            out=hout_s[:, nsl], in0=o1[:], scalar1=bn_s[:, 0:1],
                            scalar2=None, op0=Alu.add)

            def bn_norm(hin_s, bn_s, bnin_d, bnout_d, D):
                """BN stats (blockwise) + AllReduce; returns (scale, shift)."""
                s_cols = spool.tile([D, NB], f32, tag="bnscols")
                q_cols = spool.tile([D, NB], f32, tag="bnqcols")
                for b in range(NB):
                    nsl = slice(b * BLK, (b + 1) * BLK)
                    nc.vector.tensor_reduce(
                        out=s_cols[:, b:b + 1], in_=hin_s[:, nsl],
                        axis=mybir.AxisListType.X, op=Alu.add)
                    scr = spool.tile([D, BLK], f32, tag="bnscr")
                    nc.scalar.activation(
                        out=scr[:], in_=hin_s[:, nsl], func=Act.Square,
                        accum_out=q_cols[:, b:b + 1])
                st = spool.tile([D, 2], f32, tag="bnst")
                nc.vector.tensor_reduce(out=st[:, 0:1], in_=s_cols[:],
                                        axis=mybir.AxisListType.X, op=Alu.add)
                nc.vector.tensor_reduce(out=st[:, 1:2], in_=q_cols[:],
                                        axis=mybir.AxisListType.X, op=Alu.add)
                nc.sync.dma_start(out=bnin_d[:, :], in_=st[:])
                nc.gpsimd.collective_compute(
                    "AllReduce", Alu.add, replica_groups=RG,
                    ins=[bnin_d.ap().opt()], outs=[bnout_d.ap().opt()])
                g = spool.tile([D, 2], f32, tag="bng")
                nc.sync.dma_start(out=g[:], in_=bnout_d[:, :])
                # mu = (S - corr)/N ; var = (SQ - corrsq)/N - mu^2
                t_a = spool.tile([D, 1], f32, tag="bnta")
                nc.vector.tensor_tensor(out=t_a[:], in0=g[:, 0:1],
                                        in1=bn_s[:, 3:4], op=Alu.subtract)
                mu = spool.tile([D, 1], f32, tag="bnmu")
                nc.vector.tensor_scalar(out=mu[:], in0=t_a[:],
                                        scalar1=1.0 / N_REAL, scalar2=None,
                                        op0=Alu.mult)
                t_b = spool.tile([D, 1], f32, tag="bntb")
                nc.vector.tensor_tensor(out=t_b[:], in0=g[:, 1:2],
                                        in1=bn_s[:, 4:5], op=Alu.subtract)
                msq = spool.tile([D, 1], f32, tag="bnmsq")
                nc.vector.tensor_scalar(out=msq[:], in0=t_b[:],
                                        scalar1=1.0 / N_REAL, scalar2=None,
                                        op0=Alu.mult)
                mu2 = spool.tile([D, 1], f32, tag="bnmu2")
                nc.vector.tensor_tensor(out=mu2[:], in0=mu[:], in1=mu[:],
                                        op=Alu.mult)
                var = spool.tile([D, 1], f32, tag="bnvar")
                nc.vector.tensor_tensor(out=var[:], in0=msq[:], in1=mu2[:],
                                        op=Alu.subtract)
                sd = spool.tile([D, 1], f32, tag="bnsd")
                nc.scalar.activation(out=sd[:], in_=var[:], func=Act.Sqrt,
                                     bias=bn_s[:, 5:6])
                rs = spool.tile([D, 1], f32, tag="bnrs")
                nc.vector.reciprocal(out=rs[:], in_=sd[:])
                scl = spool.tile([D, 1], f32, tag="bnscl")
                nc.vector.tensor_tensor(out=scl[:], in0=bn_s[:, 1:2],
                                        in1=rs[:], op=Alu.mult)
                t_c = spool.tile([D, 1], f32, tag="bntc")
                nc.vector.tensor_tensor(out=t_c[:], in0=mu[:], in1=scl[:],
                                        op=Alu.mult)
                shf = spool.tile([D, 1], f32, tag="bnshf")
                nc.vector.tensor_tensor(out=shf[:], in0=bn_s[:, 2:3],
                                        in1=t_c[:], op=Alu.subtract)
                return scl, shf

            def norm_elu_blockwise(dst_s, src_s, scl, shf, D, do_elu):
                for b in range(NB):
                    nsl = slice(b * BLK, (b + 1) * BLK)
                    if not do_elu:
                        nc.vector.tensor_scalar(
                            out=dst_s[:, nsl], in0=src_s[:, nsl],
                            scalar1=scl[:], scalar2=shf[:],
                            op0=Alu.mult, op1=Alu.add)
                        continue
                    hb = spool.tile([D, BLK], f32, tag="nrmh")
                    nc.vector.tensor_scalar(
                        out=hb[:], in0=src_s[:, nsl], scalar1=scl[:],
                        scalar2=shf[:], op0=Alu.mult, op1=Alu.add)
                    m_t = spool.tile([D, BLK], f32, tag="nrmm")
                    nc.vector.tensor_scalar(out=m_t[:], in0=hb[:],
                                            scalar1=0.0, scalar2=None,
                                            op0=Alu.min)
                    e_t = spool.tile([D, BLK], f32, tag="nrme")
                    nc.scalar.activation(out=e_t[:], in_=m_t[:], func=Act.Exp)
                    nc.vector.tensor_scalar(out=m_t[:], in0=hb[:],
                                            scalar1=0.0, scalar2=None,
                                            op0=Alu.max)
                    t_t = spool.tile([D, BLK], f32, tag="nrmt")
                    nc.vector.tensor_tensor(out=t_t[:], in0=e_t[:],
                                            in1=m_t[:], op=Alu.add)
                    nc.vector.tensor_scalar(out=dst_s[:, nsl], in0=t_t[:],
                                            scalar1=-1.0, scalar2=None,
                                            op0=Alu.add)

            # ================= layer 1 =================
            gemm_layer(gemm1_lhs, IN_DIM, w1h_s, b1f_s, fsfd1_loc, fsfd1_full)
            edge_layer(H1, F1, fsfd1_full, fsfd1_loc, a1_s, r1_s, bn1_s,
                       h1_s, do_elu=True)
            scl1, shf1 = bn_norm(h1_s, bn1_s, bnin[0], bnout[0], HID)
            norm_elu_blockwise(h_s, h1_s, scl1, shf1, HID, do_elu=True)

            # ================= layer 2 =================
            gemm_layer(lambda t: h_s[:, t * 128:(t + 1) * 128], HID, w2_s,
                       b2f_s, fsfd2_loc, fsfd2_full)
            edge_layer(H2, F2, fsfd2_full, fsfd2_loc, a2_s, r2_s, bn2_s,
                       h1_s, do_elu=False)  # reuse h1_s as h2 buffer
            scl2, shf2 = bn_norm(h1_s, bn2_s, bnin[1], bnout[1], OUT_DIM)
            outb = npool.tile([OUT_DIM, CORE_NODES], f16, tag="outb")
            norm_elu_blockwise(outb, h1_s, scl2, shf2, OUT_DIM, do_elu=False)
            nc.sync.dma_start(out=out_d[:, :], in_=outb[:])

    return nc


# ---------------------------------------------------------------- pjrt runner
_CACHE = {}


def _get_runtime():
    if "rt" in _CACHE:
        return _CACHE["rt"]
    for p in ("/opt/trn_rl_repo",):
        if os.path.isdir(p) and p not in sys.path:
            sys.path.insert(0, p)
    import jax
    import jax.numpy as jnp
    from jax.sharding import Mesh, PartitionSpec
    from jax.experimental.shard_map import shard_map
    from concourse import mybir
    from concourse.bass2jax import (_bass_exec_p, partition_id_tensor,
                                    install_neuronx_cc_hook)

    install_neuronx_cc_hook()
    nc = build_program()
    nc.finalize()

    partition_name = (nc.partition_id_tensor.name
                      if nc.partition_id_tensor else None)
    dbg_name = nc.dbg_addr.name if nc.dbg_addr is not None else None
    in_names, out_names, out_info = [], [], []
    for alloc in nc.m.functions[0].allocations:
        if not isinstance(alloc, mybir.MemoryLocationSet):
            continue
        name = alloc.memorylocations[0].name
        if alloc.kind == "ExternalInput":
            if name != partition_name:
                in_names.append(name)
        elif alloc.kind == "ExternalOutput":
            out_names.append(name)
            out_info.append((tuple(alloc.tensor_shape),
                             mybir.dt.np(alloc.dtype)))
    in_names_all = tuple(in_names + out_names
                         + ([partition_name] if partition_name else []))
    out_avals = tuple(jax.core.ShapedArray(s, d) for s, d in out_info)

    assert dbg_name is None and in_names == OPERAND_NAMES, (dbg_name, in_names)
    n_params = len(in_names)

    def _body(*args):
        operands = list(args)
        if partition_name is not None:
            operands.append(partition_id_tensor())
        return tuple(_bass_exec_p.bind(
            *operands, out_avals=out_avals, in_names=in_names_all,
            out_names=tuple(out_names), lowering_input_output_aliases=(),
            sim_require_finite=True, sim_require_nnan=True, nc=nc))

    devices = jax.devices()[:NCORES]
    assert len(devices) == NCORES
    mesh = Mesh(np.asarray(devices), ("core",))
    from jax.sharding import NamedSharding
    spec = NamedSharding(mesh, PartitionSpec("core"))
    fn = jax.jit(shard_map(
        _body, mesh=mesh,
        in_specs=(PartitionSpec("core"),) * (n_params + len(out_names)),
        out_specs=(PartitionSpec("core"),) * len(out_names),
        check_rep=False),
        donate_argnums=tuple(range(n_params, n_params + len(out_names))))
    # ExternalOutput staging buffers, zero-filled on device (never uploaded).
    # Donated into the bass_exec results; the kernel writes every element.
    zfn = jax.jit(
        lambda: tuple(jnp.zeros((NCORES * s[0], *s[1:]), d)
                      for s, d in out_info),
        out_shardings=(spec,) * len(out_info))
    _CACHE["rt"] = (fn, zfn, mesh)
    return _CACHE["rt"]


def kernel(**inputs) -> np.ndarray:
    import threading
    import jax
    from jax.sharding import NamedSharding, PartitionSpec

    fn, zfn, mesh = _get_runtime()
    spec = NamedSharding(mesh, PartitionSpec("core"))
    # overlap the (largest) x upload with the remaining host-side prep
    xh_g = make_xh(inputs)
    holder = {}

    def _put():
        holder["xh"] = jax.device_put(xh_g, spec)

    th = threading.Thread(target=_put)
    th.start()
    rest = make_rest(inputs)
    th.join()
    (out,) = fn(holder["xh"], *rest, *zfn())
    # threaded per-shard pull overlaps the 8 device->host transfers
    from concurrent.futures import ThreadPoolExecutor
    arr = np.empty((NCORES * OUT_DIM, CORE_NODES), np.float16)

    def _pull(shard):
        arr[shard.index] = np.asarray(shard.data)

    with ThreadPoolExecutor(NCORES) as ex:
        list(ex.map(_pull, out.addressable_shards))
    res = arr.reshape(NCORES, OUT_DIM, CORE_NODES).transpose(0, 2, 1)
    return np.ascontiguousarray(
        res.reshape(NPAD, OUT_DIM)[:N_REAL]).astype(np.float32)


if __name__ == "__main__":
    import jax
    with jax.default_device(jax.devices("cpu")[0]):
        import reference
        inputs = {k: np.asarray(v) for k, v in reference.setup_inputs().items()}
        expected = np.asarray(reference.reference(**inputs))
    actual = kernel(**inputs)
    rel = np.linalg.norm(actual - expected) / np.linalg.norm(expected)
    print("Relative error:", rel)
